# revision 13
# baseline (speedup 1.0000x reference)
"""DAWN block (moe_routing) Trainium2 kernel: 8-core SPMD, v2.

Sharding: core c = (batch b=c//2, half h=c%2). Each core handles one batch's
attention + memory block for half the queries (rows permuted so local queries
come first). Expert pools (compress_neurons / expand_pool) are pair-sharded:
each core streams only 32 of 64 experts and partial combines are AllReduced
within the pair. Causal structure: per-q-slot key-position lists + a constant
triangular mask tile + per-core bias column (full-mask blocks), so ~19% of
score/AV work is skipped and no per-key mask tensor is needed.

DMA strategy: everything is host-relaid-out so the device does few, large,
contiguous DMAs (the v1 kernel's 456 tiny combine DMAs were the bottleneck:
each DMA costs ~0.6us queue dispatch + 625ns shared HWDGE serial time).
"""
import numpy as np
import ml_dtypes

import concourse.bass as bass
import concourse.mybir as mybir
import concourse.tile as tile
from concourse import bacc

B, S, D = 4, 1024, 1024
H, DH = 16, 64
R = 128
NEXP = 64
NLOC = 32          # local experts per core (pair-sharded)
GLOC = NLOC // 4   # 8 stacked-expert groups
NK, KR = 4096, 128
TOPK = 8
N_CORES = 8
SQ = S // 2
P = 128
NT = S // P        # 8 seq tiles
QT = SQ // P       # 4 local q tiles
DT_T = D // P      # 8 d tiles

F32 = mybir.dt.float32
U32 = mybir.dt.uint32
NEG = -1.0e9
ALU = mybir.AluOpType
ACT = mybir.ActivationFunctionType
AX = None

# aux column maps
AF_A4 = 0          # [0:32, 0:128]
AF_B8 = 128        # [0:32, 128:136]
AF_BSELR = 136     # [0:1, 136:140]
AF_BSELC = 140     # [0:4, 140:141]
AF_NEGC = 141      # [0:128, 141:142]
AF_ONE = 142       # [0:1, 142:143]  value 1.0
AF_W = 144
AB_IDN = 0         # [0:128, 0:128]
AB_BMS = 128       # [0:128, 128:160]
AB_TRI = 160       # [0:128, 160:672] = [tri | zeros x3]
AB_W = 672


def bdt():
    return mybir.dt.bfloat16


def np_bdt():
    return ml_dtypes.bfloat16


def build_nc(use_cc=True):
    global AX
    AX = mybir.AxisListType.X
    DT = bdt()
    nc = bacc.Bacc("TRN2", target_bir_lowering=False, debug=False,
                   num_devices=N_CORES)
    I = {}

    def inp(name, shape, dt):
        I[name] = nc.dram_tensor(name, shape, dt, kind="ExternalInput").ap()

    inp("x", [S, D], F32)              # row-permuted batch (local q first)
    inp("imp", [P, NT], F32)           # imp[p,i] = importance[perm[128i+p]]
    inp("cnb", [4, GLOC, P, 1024], DT) # local-expert compress pool, relaid
    inp("plb", [4, GLOC, P, 1024], DT) # local-expert expand pool, relaid
    inp("wct", [P, NT * 320], DT)      # [Wc|WQ|WK|WV|Wm].T tiled (expert-perm)
    inp("wot", [P, NT * 1024], DT)     # WO.T tiled
    inp("kkt", [KR, NK], DT)           # knowledge_K.T
    inp("kv", [NK, D], DT)             # knowledge_V
    inp("gb", [4, D], F32)             # g1,b1,g2,b2 rows
    inp("auxf", [P, AF_W], F32)
    inp("auxb", [P, AB_W], DT)
    o = nc.dram_tensor("o", [SQ, D], F32, kind="ExternalOutput").ap()

    with tile.TileContext(nc) as tc:
        _body(nc, tc, I, o, use_cc)
    nc.compile()
    return nc


def _body(nc, tc, I, o, use_cc):
    DT = bdt()
    import contextlib
    ctx = contextlib.ExitStack()
    with ctx:
        pp = ctx.enter_context(tc.tile_pool(name="pers", bufs=1))
        sp = ctx.enter_context(tc.tile_pool(name="stream", bufs=2))
        st2 = ctx.enter_context(tc.tile_pool(name="strm", bufs=3))
        pst = ctx.enter_context(tc.tile_pool(name="ps_t", bufs=2, space="PSUM"))
        psa = ctx.enter_context(tc.tile_pool(name="ps_a", bufs=2, space="PSUM"))
        psv = ctx.enter_context(tc.tile_pool(name="ps_v", bufs=2, space="PSUM"))
        dr = ctx.enter_context(tc.tile_pool(name="dram", bufs=1, space="DRAM"))

        # ---------- bulk loads ----------
        xa = pp.tile([P, NT * 1024], F32, tag="xa", name="xa")
        for hh in range(2):
            src = bass.AP(I["x"].tensor, hh * 4 * P * 1024,
                          [[1024, P], [P * 1024, 4], [1, 1024]])
            nc.sync.dma_start(out=xa[:, hh * 4096:(hh + 1) * 4096], in_=src)
        wct = pp.tile([P, NT * 320], DT, tag="wct", name="wct")
        nc.sync.dma_start(out=wct[:], in_=I["wct"][:])
        impa = pp.tile([P, NT], F32, tag="impa", name="impa")
        nc.sync.dma_start(out=impa[:], in_=I["imp"][:])
        auxf = pp.tile([P, AF_W], F32, tag="auxf", name="auxf")
        nc.sync.dma_start(out=auxf[:], in_=I["auxf"][:])
        auxb = pp.tile([P, AB_W], DT, tag="auxb", name="auxb")
        nc.sync.dma_start(out=auxb[:], in_=I["auxb"][:])

        idn = auxb[:, AB_IDN:AB_IDN + P]
        bmS = auxb[:, AB_BMS:AB_BMS + 32]
        trix = auxb[:, AB_TRI:AB_TRI + 4 * P]
        A4 = auxf[0:NLOC, AF_A4:AF_A4 + P]
        B8 = auxf[0:NLOC, AF_B8:AF_B8 + GLOC]
        bselr = auxf[0:1, AF_BSELR:AF_BSELR + B]
        bselc = auxf[0:B, AF_BSELC:AF_BSELC + 1]
        negc = auxf[:, AF_NEGC:AF_NEGC + 1]
        one1 = auxf[0:1, AF_ONE:AF_ONE + 1]

        # expert pool streams in half-b chunks: [128, 4096] = 4 g-groups.
        def stream_half(tensor, b, half):
            t = st2.tile([P, 4096], DT, tag="strm", name="strm")
            srcap = bass.AP(tensor, (b * GLOC + half * 4) * P * 1024,
                            [[1024, P], [P * 1024, 4], [1, 1024]])
            nc.sync.dma_start(out=t[:], in_=srcap)
            return t

        cn1 = [[stream_half(I["cnb"].tensor, b, h) for h in range(2)]
               for b in range(4)]
        pl1 = [[stream_half(I["plb"].tensor, b, h) for h in range(2)]
               for b in range(4)]
        wota2 = []
        for h in range(2):
            t = st2.tile([P, 4096], DT, tag="strm", name="strm")
            nc.sync.dma_start(out=t[:], in_=I["wot"][:, h * 4096:(h + 1) * 4096])
            wota2.append(t)
        cn2 = [[stream_half(I["cnb"].tensor, b, h) for h in range(2)]
               for b in range(4)]

        gB = pp.tile([P, D], F32, tag="gB", name="gB")
        bB = pp.tile([P, D], F32, tag="bB", name="bB")

        def load_gb(gi, bi):
            nc.sync.dma_start(out=gB[:], in_=bass.AP(I["gb"].tensor, gi * D,
                                                     [[0, P], [1, D]]))
            nc.sync.dma_start(out=bB[:], in_=bass.AP(I["gb"].tensor, bi * D,
                                                     [[0, P], [1, D]]))

        load_gb(0, 1)

        def copy_ps(out_ap, in_ap, k):
            if k % 2 == 0:
                nc.vector.tensor_copy(out=out_ap, in_=in_ap)
            else:
                nc.scalar.activation(out=out_ap, in_=in_ap, func=ACT.Identity)

        def layernorm_tile(x_ap, pool, tag):
            # var = E[x^2] - mean^2; one Square pass on raw x overlaps the
            # DVE sum; final pass folds (x - mean) * rstd into one activation.
            stats = sp.tile([P, 6], F32, tag="ln_stats", name="ln_stats")
            mean = stats[:, 0:1]; ex2 = stats[:, 1:2]; rstd = stats[:, 2:3]
            nmr = stats[:, 3:4]; var = stats[:, 4:5]; m2 = stats[:, 5:6]
            sq = sp.tile([P, D], F32, tag="ln_sq", name="ln_sq", bufs=1)
            nc.scalar.activation(out=sq[:], in_=x_ap, func=ACT.Square,
                                 accum_out=ex2)
            nc.vector.tensor_reduce(out=mean, in_=x_ap, axis=AX, op=ALU.add)
            nc.vector.tensor_scalar(out=mean, in0=mean, scalar1=1.0 / D,
                                    scalar2=None, op0=ALU.mult)
            nc.vector.tensor_tensor(out=m2, in0=mean, in1=mean, op=ALU.mult)
            nc.vector.tensor_scalar(out=var, in0=ex2, scalar1=1.0 / D,
                                    scalar2=1e-5, op0=ALU.mult, op1=ALU.add)
            nc.vector.tensor_tensor(out=var, in0=var, in1=m2, op=ALU.subtract)
            nc.scalar.sqrt(rstd, var)
            nc.vector.reciprocal(rstd, rstd)
            nc.vector.tensor_tensor(out=nmr, in0=mean, in1=rstd, op=ALU.mult)
            nc.vector.tensor_scalar(out=nmr, in0=nmr, scalar1=-1.0,
                                    scalar2=None, op0=ALU.mult)
            out = pool.tile([P, D], DT, tag=tag)
            nc.scalar.activation(out=out[:], in_=x_ap, func=ACT.Identity,
                                 scale=rstd, bias=nmr)
            nc.vector.tensor_tensor(out=out[:], in0=out[:], in1=gB[:],
                                    op=ALU.mult)
            nc.gpsimd.tensor_tensor(out=out[:], in0=out[:], in1=bB[:],
                                    op=ALU.add)
            return out

        def softmax_pool(psum_ap, out_ap, nblk, blk, imp_col, pool_out,
                         first, last):
            # exp (no max-sub; scores are O(1)) with per-block accum, then
            # pool with 1/Z folded into the importance column.
            zs = sp.tile([P, 8], F32, tag="sm_zs", name="sm_zs")
            for bi in range(nblk):
                sl = slice(bi * blk, (bi + 1) * blk)
                nc.scalar.activation(out=out_ap[:, sl], in_=psum_ap[:, sl],
                                     func=ACT.Exp, accum_out=zs[:, bi:bi + 1])
            nc.vector.reciprocal(zs[:, 0:nblk], zs[:, 0:nblk])
            impz = sp.tile([P, 8], F32, tag="sm_iz", name="sm_iz")
            nc.vector.tensor_scalar(out=impz[:, 0:nblk], in0=zs[:, 0:nblk],
                                    scalar1=imp_col, scalar2=None,
                                    op0=ALU.mult)
            for bi in range(nblk):
                sl = slice(bi * blk, (bi + 1) * blk)
                nc.tensor.matmul(out=pool_out[:, sl],
                                 lhsT=impz[:, bi:bi + 1], rhs=out_ap[:, sl],
                                 start=first, stop=last)

        def group_cols(wcol_ap, ncols):
            """wcol [32, ncols] f32 -> wk [128, GLOC*ncols]:
            wk[p, ncols*g + c] = wcol[4g + p%4, c]."""
            rhsB = sp.tile([NLOC, GLOC * ncols], F32, tag="rhsB", name="rhsB")
            for pi in range(ncols):
                nc.vector.tensor_scalar(
                    out=rhsB[:, pi:GLOC * ncols:ncols], in0=B8,
                    scalar1=wcol_ap[:, pi:pi + 1], scalar2=None, op0=ALU.mult)
            wkp = pst.tile([P, GLOC * ncols], F32, tag="tpp", name="wkp")
            nc.tensor.matmul(out=wkp[:], lhsT=A4, rhs=rhsB[:],
                             start=True, stop=True)
            wk = sp.tile([P, GLOC * ncols], F32, tag="wkall", name="wkall")
            nc.vector.tensor_copy(out=wk[:], in_=wkp[:])
            return wk

        def combine_cn(wcol_ap, chunks, out_f32):
            """out_f32 [128, 1024] f32 partial combine of local experts.
            chunks[b][p, 1024g+128t+r] = CN[e(g,p), 128t+32b+p//4, r]."""
            wk = group_cols(wcol_ap, 1)
            lhs = []
            for g in range(GLOC):
                lg = sp.tile([P, NLOC], DT, tag=f"clh{g}", name=f"clh{g}",
                             bufs=1)
                nc.vector.tensor_scalar(out=lg[:], in0=bmS,
                                        scalar1=wk[:, g:g + 1],
                                        scalar2=None, op0=ALU.mult)
                lhs.append(lg)
            for b in range(4):
                acc = psa.tile([NLOC, 1024], F32, tag="acc", name="cacc")
                for hh in range(2):
                    for g in range(GLOC):
                        gh, gl = g // 4, g % 4
                        nc.tensor.matmul(
                            out=acc[:, hh * 512:(hh + 1) * 512],
                            lhsT=lhs[g][:],
                            rhs=chunks[b][gh][:, gl * 1024 + hh * 512:
                                              gl * 1024 + (hh + 1) * 512],
                            start=(g == 0), stop=(g == GLOC - 1))
                copy_ps(out_f32[32 * b:32 * b + 32, :], acc[:], b)

        def pair_allreduce(sb_f32, ncol):
            """AllReduce sb_f32 [128, ncol] within batch pairs (in place)."""
            if not use_cc:
                return
            cc_in = dr.tile([P, ncol], F32)
            cc_out = dr.tile([P, ncol], F32)
            nc.gpsimd.dma_start(out=cc_in[:], in_=sb_f32[:])
            nc.gpsimd.collective_compute(
                "AllReduce", ALU.add,
                replica_groups=[[0, 1], [2, 3], [4, 5], [6, 7]],
                ins=[cc_in.opt()], outs=[cc_out.opt()])
            nc.gpsimd.dma_start(out=sb_f32[:], in_=cc_out[:])

        # ---------- LN1 + transposes ----------
        ctx4 = contextlib.ExitStack()
        p4 = ctx4.enter_context(tc.tile_pool(name="ph4", bufs=1))
        with tc.tile_pool(name="ph0", bufs=1) as p0:
            nxT = [p0.tile([P, S], DT, tag=f"nxT{t}", name=f"nxT{t}")
                   for t in range(DT_T)]
            for i in range(NT):
                nx_i = layernorm_tile(xa[:, i * 1024:(i + 1) * 1024], sp, "nx")
                for t in range(DT_T):
                    tp = pst.tile([P, P], DT, tag="tpp", name="tpp")
                    nc.tensor.transpose(out=tp[:],
                                        in_=nx_i[:, t * P:(t + 1) * P],
                                        identity=idn)
                    copy_ps(nxT[t][:, i * P:(i + 1) * P], tp[:], t)

            # ---------- routers (c,q,k,v) ----------
            wpool_ps = psv.tile([1, 4 * NEXP], F32, tag="pvacc", name="pvacc")
            for i in range(NT):
                pr_ps = psa.tile([P, 4 * NEXP], F32, tag="acc", name="acc")
                for t in range(DT_T):
                    nc.tensor.matmul(out=pr_ps[:],
                                     lhsT=nxT[t][:, i * P:(i + 1) * P],
                                     rhs=wct[:, 320 * t:320 * t + 256],
                                     start=(t == 0), stop=(t == DT_T - 1))
                pref = sp.tile([P, 4 * NEXP], F32, tag="pref", name="pref")
                softmax_pool(pr_ps[:], pref[:], 4, NEXP, impa[:, i:i + 1],
                             wpool_ps, first=(i == 0), last=(i == NT - 1))

            wrow = pp.tile([1, 4 * NEXP], F32, tag="wrow", name="wrow")
            nc.vector.tensor_copy(out=wrow[:], in_=wpool_ps[:])
            for bi in range(4):
                sl = slice(bi * NEXP, (bi + 1) * NEXP)
                st = sp.tile([1, 1], F32, tag="wn_st", name="wn_st")
                nc.vector.tensor_reduce(out=st[:], in_=wrow[:, sl], axis=AX,
                                        op=ALU.add)
                nc.vector.tensor_scalar(out=st[:], in0=st[:], scalar1=1e-8,
                                        scalar2=None, op0=ALU.add)
                nc.vector.reciprocal(st[:], st[:])
                nc.vector.tensor_scalar(out=wrow[:, sl], in0=wrow[:, sl],
                                        scalar1=st[:], scalar2=None,
                                        op0=ALU.mult)
            wt0 = pst.tile([P, 1], F32, tag="tpp", name="wt0")
            nc.tensor.transpose(out=wt0[:], in_=wrow[:, 0:P], identity=one1)
            wt1 = pst.tile([P, 1], F32, tag="tpp", name="wt1")
            nc.tensor.transpose(out=wt1[:], in_=wrow[:, P:2 * P], identity=one1)
            wcolcq = pp.tile([P, 1], F32, tag="wcolcq", name="wcolcq")
            nc.vector.tensor_copy(out=wcolcq[:], in_=wt0[:])
            wcolkv = pp.tile([P, 1], F32, tag="wcolkv", name="wcolkv")
            nc.vector.tensor_copy(out=wcolkv[:], in_=wt1[:])
            wcols3 = pp.tile([NLOC, 3], F32, tag="wcols3", name="wcols3")
            nc.vector.tensor_copy(out=wcols3[:, 0:1],
                                  in_=wcolcq[NEXP:NEXP + NLOC, :])
            nc.vector.tensor_copy(out=wcols3[:, 1:2], in_=wcolkv[0:NLOC, :])
            nc.vector.tensor_copy(out=wcols3[:, 2:3],
                                  in_=wcolkv[NEXP:NEXP + NLOC, :])

            # ---------- sc combine (+pair AllReduce) ----------
            e3f = p0.tile([P, 3072], F32, tag="e3f", name="e3f")
            scf = e3f[:, 0:1024]
            combine_cn(wcolcq[0:NLOC, 0:1], cn1, scf)
            pair_allreduce(scf, 1024)
            sc_b = p0.tile([P, 1024], DT, tag="sc_b", name="sc_b")
            nc.vector.tensor_copy(out=sc_b[:], in_=scf[:])

            # ---------- e3 combine ----------
            w3 = group_cols(wcols3[:], 3)  # [128, 24]
            lhs3 = []
            for g in range(GLOC):
                lg = p0.tile([P, 96], DT, tag=f"e3lh{g}", name=f"e3lh{g}")
                for pl_i in range(3):
                    nc.vector.tensor_scalar(
                        out=lg[:, 32 * pl_i:32 * (pl_i + 1)], in0=bmS,
                        scalar1=w3[:, 3 * g + pl_i:3 * g + pl_i + 1],
                        scalar2=None, op0=ALU.mult)
                lhs3.append(lg)
            for b in range(4):
                acc = psa.tile([96, 1024], F32, tag="acc", name="eacc")
                for hh in range(2):
                    for g in range(GLOC):
                        gh, gl = g // 4, g % 4
                        nc.tensor.matmul(
                            out=acc[:, hh * 512:(hh + 1) * 512],
                            lhsT=lhs3[g][:],
                            rhs=pl1[b][gh][:, gl * 1024 + hh * 512:
                                           gl * 1024 + (hh + 1) * 512],
                            start=(g == 0), stop=(g == GLOC - 1))
                for pl_i in range(3):
                    copy_ps(e3f[32 * b:32 * b + 32,
                                1024 * pl_i:1024 * (pl_i + 1)],
                            acc[32 * pl_i:32 * pl_i + 32, :], b + pl_i)
            pair_allreduce(e3f, 3072)
            e3 = p0.tile([P, 3072], DT, tag="e3", name="e3")
            nc.vector.tensor_copy(out=e3[:, 0:1024], in_=e3f[:, 0:1024])
            nc.scalar.activation(out=e3[:, 1024:2048], in_=e3f[:, 1024:2048],
                                 func=ACT.Identity)
            nc.gpsimd.tensor_copy(out=e3[:, 2048:3072], in_=e3f[:, 2048:3072])

            # ---------- hT directly: hT[r, q] = sum_d sc[d, r] nx[q, d] ----------
            hT = p0.tile([P, S], DT, tag="hT")
            for j in range(2):
                hp = psa.tile([P, 512], F32, tag="acc", name="hacc")
                for t in range(DT_T):
                    nc.tensor.matmul(out=hp[:],
                                     lhsT=sc_b[:, t * P:(t + 1) * P],
                                     rhs=nxT[t][:, j * 512:(j + 1) * 512],
                                     start=(t == 0), stop=(t == DT_T - 1))
                copy_ps(hT[:, j * 512:(j + 1) * 512], hp[:], j)

            # ---------- K, Q, V ----------
            SCALE_Q = 1.0 / float(np.sqrt(DH))
            kT = [p4.tile([P, S], DT, tag=f"kT{t}", name=f"kT{t}")
                  for t in range(DT_T)]
            qT = [p4.tile([P, SQ], DT, tag=f"qT{t}", name=f"qT{t}")
                  for t in range(DT_T)]
            vext = [p4.tile([P, H * (DH + 1)], DT, tag=f"vx{i}", name=f"vx{i}")
                    for i in range(NT)]
            for t in range(DT_T):
                kp = psa.tile([P, S], F32, tag="acc", name="acc")
                for j in range(2):
                    nc.tensor.matmul(out=kp[:, j * 512:(j + 1) * 512],
                                     lhsT=e3[:, 1024 + t * P:1024 + t * P + P],
                                     rhs=hT[:, j * 512:(j + 1) * 512],
                                     start=True, stop=True)
                nc.scalar.activation(out=kT[t][:], in_=kp[:], func=ACT.Identity)
                qp = psv.tile([P, SQ], F32, tag="pvacc", name="qacc")
                nc.tensor.matmul(out=qp[:], lhsT=e3[:, t * P:t * P + P],
                                 rhs=hT[:, 0:SQ], start=True, stop=True)
                nc.vector.tensor_scalar(out=qT[t][:], in0=qp[:],
                                        scalar1=SCALE_Q, scalar2=None,
                                        op0=ALU.mult)
            for i in range(NT):
                vp = psa.tile([P, D], F32, tag="acc", name="acc")
                for j in range(2):
                    nc.tensor.matmul(
                        out=vp[:, j * 512:(j + 1) * 512],
                        lhsT=hT[:, i * P:(i + 1) * P],
                        rhs=e3[:, 2048 + j * 512:2048 + (j + 1) * 512],
                        start=True, stop=True)
                vv = vext[i][:].rearrange("p (hh c) -> p hh c", c=DH + 1)
                nc.vector.tensor_copy(
                    out=vv[:, :, 0:DH],
                    in_=vp[:].rearrange("p (hh c) -> p hh c", c=DH))
                nc.gpsimd.memset(vv[:, :, DH:DH + 1], 1.0)
        # ph0 (nxT, scf, e3f, lhs3) released

        # ---------- attention ----------
        # q-slot s covers local q-tile s; key positions {0..s} u {4..7}.
        # position j==s gets the constant tri mask via PE; positions 4..7 get
        # the per-core bias column (0 or -1e9) folded into the exp.
        attnT = [p4.tile([P, SQ], DT, tag=f"at{t}", name=f"at{t}")
                 for t in range(DT_T)]
        for hd in range(H):
            t4 = hd // 2
            hs = (hd % 2) * DH
            po = psv.tile([DH + 1, SQ], F32, tag="pvacc", name="poacc")
            for j in range(NT):
                qlo = j * P if j < QT else 0
                w = SQ - qlo
                sps = psa.tile([P, SQ], F32, tag="acc", name="sacc")
                if j < QT:
                    nc.tensor.matmul(out=sps[:, 0:P],
                                     lhsT=kT[t4][hs:hs + DH, j * P:(j + 1) * P],
                                     rhs=qT[t4][hs:hs + DH, qlo:qlo + P],
                                     start=True, stop=False)
                    if w > P:
                        nc.tensor.matmul(out=sps[:, P:w],
                                         lhsT=kT[t4][hs:hs + DH,
                                                     j * P:(j + 1) * P],
                                         rhs=qT[t4][hs:hs + DH, qlo + P:SQ],
                                         start=True, stop=True)
                else:
                    nc.tensor.matmul(out=sps[:, 0:w],
                                     lhsT=kT[t4][hs:hs + DH, j * P:(j + 1) * P],
                                     rhs=qT[t4][hs:hs + DH, qlo:SQ],
                                     start=True, stop=True)
                if j < QT:
                    nc.tensor.matmul(out=sps[:, 0:P], lhsT=idn,
                                     rhs=trix[:, 0:P], start=False, stop=True)
                pt = sp.tile([P, SQ], DT, tag="p_tile", name="p_tile", bufs=3)
                if j < QT:
                    nc.scalar.activation(out=pt[:, 0:w], in_=sps[:, 0:w],
                                         func=ACT.Exp)
                else:
                    nc.scalar.activation(out=pt[:, 0:w], in_=sps[:, 0:w],
                                         func=ACT.Exp, bias=negc)
                nc.tensor.matmul(
                    out=po[:, qlo:SQ],
                    lhsT=vext[j][:, hd * (DH + 1):(hd + 1) * (DH + 1)],
                    rhs=pt[:, 0:w], start=(j == 0), stop=(j == NT - 1))
            rec = sp.tile([1, SQ], F32, tag="rec", name="rec")
            nc.vector.reciprocal(rec[:], po[DH:DH + 1, :])
            recB = sp.tile([DH, SQ], F32, tag="recB", name="recB")
            nc.gpsimd.partition_broadcast(recB[:], rec[:])
            nc.vector.tensor_tensor(out=attnT[t4][hs:hs + DH, :],
                                    in0=po[0:DH, :], in1=recB[:], op=ALU.mult)

        # ---------- WO + residual (into xa) ----------
        for i in range(QT):
            wp = psa.tile([P, D], F32, tag="acc", name="acc")
            for j in range(2):
                for t in range(DT_T):
                    toff = 1024 * t + 512 * j
                    nc.tensor.matmul(
                        out=wp[:, j * 512:(j + 1) * 512],
                        lhsT=attnT[t][:, i * P:(i + 1) * P],
                        rhs=wota2[toff // 4096][:, toff % 4096:
                                                toff % 4096 + 512],
                        start=(t == 0), stop=(t == DT_T - 1))
            nc.vector.tensor_tensor(out=xa[:, i * 1024:(i + 1) * 1024],
                                    in0=wp[:], in1=xa[:, i * 1024:(i + 1) * 1024],
                                    op=ALU.add)

        ctx4.close()

        # ---------- memory block ----------
        with tc.tile_pool(name="ph6", bufs=1) as p6:
            nx2T = [p6.tile([P, SQ], DT, tag=f"n2T{t}", name=f"n2T{t}")
                    for t in range(DT_T)]
            kkt = p6.tile([KR, NK], DT, tag="kkt", name="kkt")
            nc.sync.dma_start(out=kkt[:], in_=I["kkt"][:])
            load_gb(2, 3)
            for i in range(QT):
                nx2_i = layernorm_tile(xa[:, i * 1024:(i + 1) * 1024], sp,
                                       "nx2")
                for t in range(DT_T):
                    tp = pst.tile([P, P], DT, tag="tpp", name="tpp")
                    nc.tensor.transpose(out=tp[:],
                                        in_=nx2_i[:, t * P:(t + 1) * P],
                                        identity=idn)
                    copy_ps(nx2T[t][:, i * P:(i + 1) * P], tp[:], t)

            mwp_ps = psv.tile([1, NEXP], F32, tag="pvacc", name="pvacc")
            for i in range(QT):
                pr = psa.tile([P, NEXP], F32, tag="acc", name="acc")
                for t in range(DT_T):
                    nc.tensor.matmul(out=pr[:],
                                     lhsT=nx2T[t][:, i * P:(i + 1) * P],
                                     rhs=wct[:, 320 * t + 256:320 * t + 320],
                                     start=(t == 0), stop=(t == DT_T - 1))
                prefm = sp.tile([P, NEXP], F32, tag="prefm", name="prefm")
                softmax_pool(pr[:], prefm[:], 1, NEXP, impa[:, i:i + 1],
                             mwp_ps, first=(i == 0), last=(i == QT - 1))

            mwrow = p6.tile([1, NEXP], F32, tag="mwrow", name="mwrow")
            if use_cc:
                mwr = sp.tile([1, NEXP], F32, tag="mwr", name="mwr")
                nc.vector.tensor_copy(out=mwr[:], in_=mwp_ps[:])
                ccp = psa.tile([B, NEXP], F32, tag="acc", name="acc")
                nc.tensor.matmul(out=ccp[:], lhsT=bselr, rhs=mwr[:],
                                 start=True, stop=True)
                cc_sb = sp.tile([B, NEXP], F32, tag="cc_sb", name="cc_sb")
                nc.vector.tensor_copy(out=cc_sb[:], in_=ccp[:])
                cc_in = dr.tile([B, NEXP], F32)
                cc_out = dr.tile([B, NEXP], F32)
                nc.gpsimd.dma_start(out=cc_in[:], in_=cc_sb[:])
                nc.gpsimd.collective_compute(
                    "AllReduce", ALU.add,
                    replica_groups=[list(range(N_CORES))],
                    ins=[cc_in.opt()], outs=[cc_out.opt()])
                cc_res = sp.tile([B, NEXP], F32, tag="cc_res", name="cc_res")
                nc.gpsimd.dma_start(out=cc_res[:], in_=cc_out[:])
                mwf = psa.tile([1, NEXP], F32, tag="acc", name="acc")
                nc.tensor.matmul(out=mwf[:], lhsT=bselc, rhs=cc_res[:],
                                 start=True, stop=True)
                nc.vector.tensor_copy(out=mwrow[:], in_=mwf[:])
            else:
                nc.vector.tensor_copy(out=mwrow[:], in_=mwp_ps[:])
            st = sp.tile([1, 1], F32, tag="wn_st", name="wn_st")
            nc.vector.tensor_reduce(out=st[:], in_=mwrow[:], axis=AX,
                                    op=ALU.add)
            nc.vector.tensor_scalar(out=st[:], in0=st[:], scalar1=1e-8,
                                    scalar2=None, op0=ALU.add)
            nc.vector.reciprocal(st[:], st[:])
            nc.vector.tensor_scalar(out=mwrow[:], in0=mwrow[:], scalar1=st[:],
                                    scalar2=None, op0=ALU.mult)
            mwrow_cp = sp.tile([1, NEXP], F32, tag="mwr2", name="mwr2")
            nc.vector.tensor_copy(out=mwrow_cp[:], in_=mwrow[:])
            mwt = pst.tile([NEXP, 1], F32, tag="tpp", name="mwt")
            nc.tensor.transpose(out=mwt[:], in_=mwrow_cp[:], identity=one1)
            mwcol = p6.tile([NEXP, 1], F32, tag="mwcol", name="mwcol")
            nc.vector.tensor_copy(out=mwcol[:], in_=mwt[:])

            scmf = p6.tile([P, 1024], F32, tag="scmf", name="scmf")
            combine_cn(mwcol[0:NLOC, 0:1], cn2, scmf)
            pair_allreduce(scmf, 1024)
            scm_b = p6.tile([P, 1024], DT, tag="scm_b", name="scm_b")
            nc.vector.tensor_copy(out=scm_b[:], in_=scmf[:])

            # QmT [r, sq]
            qmp = psv.tile([P, SQ], F32, tag="pvacc", name="pvacc")
            for t in range(DT_T):
                nc.tensor.matmul(out=qmp[:], lhsT=scm_b[:, t * P:(t + 1) * P],
                                 rhs=nx2T[t][:], start=(t == 0),
                                 stop=(t == DT_T - 1))
            qmT = p6.tile([P, SQ], DT, tag="qmT")
            nc.vector.tensor_scalar(out=qmT[:], in0=qmp[:],
                                    scalar1=1.0 / float(np.sqrt(KR)),
                                    scalar2=None, op0=ALU.mult)

            idx_all = p6.tile([P, QT * TOPK], U32, tag="idx_all",
                              name="idx_all")
            w8_all = p6.tile([P, QT * TOPK], F32, tag="w8_all", name="w8_all")
            for i in range(QT):
                ks = p6.tile([P, NK], DT, tag="ks_sb", name="ks_sb")
                for j in range(NK // 512):
                    ksp = psa.tile([P, 512], F32, tag="acc", name="acc")
                    nc.tensor.matmul(out=ksp[:],
                                     lhsT=qmT[:, i * P:(i + 1) * P],
                                     rhs=kkt[:, j * 512:(j + 1) * 512],
                                     start=True, stop=True)
                    eng = nc.scalar if (j % 2 == 0) else nc.vector
                    if eng is nc.scalar:
                        nc.scalar.activation(out=ks[:, j * 512:(j + 1) * 512],
                                             in_=ksp[:], func=ACT.Identity)
                    else:
                        nc.vector.tensor_copy(out=ks[:, j * 512:(j + 1) * 512],
                                              in_=ksp[:])
                tv = sp.tile([P, TOPK], F32, tag="tv", name="tv")
                nc.vector.max_with_indices(
                    out_max=tv[:],
                    out_indices=idx_all[:, i * TOPK:(i + 1) * TOPK],
                    in_=ks[:])
                st8 = sp.tile([P, 2], F32, tag="st8", name="st8")
                nm = st8[:, 0:1]; se8 = st8[:, 1:2]
                nc.vector.tensor_scalar(out=nm, in0=tv[:, 0:1], scalar1=-1.0,
                                        scalar2=None, op0=ALU.mult)
                w8 = sp.tile([P, TOPK], F32, tag="w8", name="w8")
                nc.scalar.activation(out=w8[:], in_=tv[:], func=ACT.Exp,
                                     bias=nm, accum_out=se8)
                nc.vector.reciprocal(se8, se8)
                nc.vector.tensor_scalar(out=w8_all[:, i * TOPK:(i + 1) * TOPK],
                                        in0=w8[:], scalar1=se8, scalar2=None,
                                        op0=ALU.mult)

            for i in range(QT):
                acc = p6.tile([P, D], F32, tag="mem_acc", name="mem_acc", bufs=2)
                gt8 = p6.tile([P, TOPK * D], DT, tag="gath8", name="gath8", bufs=2)
                nc.gpsimd.indirect_dma_start(
                    out=gt8[:], out_offset=None, in_=I["kv"][:],
                    in_offset=bass.IndirectOffsetOnAxis(
                        ap=idx_all[:, i * TOPK:(i + 1) * TOPK], axis=0))
                accB = p6.tile([P, D], F32, tag="mem_accB", name="mem_accB",
                               bufs=2)
                tmp = sp.tile([P, D], F32, tag="gtmp", name="gtmp")
                for k in range(TOPK):
                    g = i * TOPK + k
                    if k % 2 == 0:
                        prev = (xa[:, i * 1024:(i + 1) * 1024] if k == 0
                                else acc[:])
                        nc.vector.scalar_tensor_tensor(
                            out=acc[:], in0=gt8[:, k * D:(k + 1) * D],
                            scalar=w8_all[:, g:g + 1],
                            in1=prev, op0=ALU.mult, op1=ALU.add)
                    else:
                        dst = accB[:] if k == 1 else tmp[:]
                        nc.scalar.activation(out=dst,
                                             in_=gt8[:, k * D:(k + 1) * D],
                                             func=ACT.Identity,
                                             scale=w8_all[:, g:g + 1])
                        if k > 1:
                            nc.gpsimd.tensor_tensor(out=accB[:], in0=tmp[:],
                                                    in1=accB[:], op=ALU.add)
                nc.vector.tensor_tensor(out=acc[:], in0=acc[:], in1=accB[:],
                                        op=ALU.add)
                nc.sync.dma_start(out=o[i * P:(i + 1) * P, :], in_=acc[:])


# ---------------- PJRT SPMD runner (persistent jit) ----------------

class SpmdRunner:
    def __init__(self, nc, n_cores):
        import jax
        from jax.sharding import Mesh, PartitionSpec
        from jax.experimental.shard_map import shard_map
        from concourse import bass2jax
        bass2jax.install_neuronx_cc_hook()
        self.jax = jax
        self.nc = nc
        self.n_cores = n_cores
        partition_name = (nc.partition_id_tensor.name
                          if nc.partition_id_tensor else None)
        in_names, out_names, out_avals, zero_outs = [], [], [], []
        for alloc in nc.m.functions[0].allocations:
            if not isinstance(alloc, mybir.MemoryLocationSet):
                continue
            name = alloc.memorylocations[0].name
            if alloc.kind == "ExternalInput":
                if name != partition_name:
                    in_names.append(name)
            elif alloc.kind == "ExternalOutput":
                shape = tuple(alloc.tensor_shape)
                dtype = mybir.dt.np(alloc.dtype)
                out_names.append(name)
                out_avals.append(jax.core.ShapedArray(shape, dtype))
                zero_outs.append(np.zeros(shape, dtype))
        self.n_params = len(in_names)
        self.in_names = list(in_names)
        self.out_names = out_names
        self.out_avals = out_avals
        self.zero_outs = zero_outs
        all_in = in_names + out_names + ([partition_name] if partition_name
                                         else [])

        def _body(*args):
            operands = list(args)
            if partition_name is not None:
                operands.append(bass2jax.partition_id_tensor())
            outs = bass2jax._bass_exec_p.bind(
                *operands, out_avals=tuple(out_avals), in_names=tuple(all_in),
                out_names=tuple(out_names), lowering_input_output_aliases=(),
                sim_require_finite=True, sim_require_nnan=True, nc=nc)
            return tuple(outs)

        devices = jax.devices()[:n_cores]
        self.mesh = Mesh(np.asarray(devices), ("core",))
        nspec = self.n_params + len(out_names)
        self.sharded = jax.jit(
            shard_map(_body, mesh=self.mesh,
                      in_specs=(PartitionSpec("core"),) * nspec,
                      out_specs=(PartitionSpec("core"),) * len(out_names),
                      check_rep=False),
            keep_unused=True)

    def concat_inputs(self, in_maps):
        per_core = [[np.asarray(m[n]) for n in self.in_names] for m in in_maps]
        cat = [np.concatenate([per_core[c][i] for c in range(self.n_cores)],
                              axis=0) for i in range(self.n_params)]
        cat += [np.zeros((self.n_cores * z.shape[0], *z.shape[1:]), z.dtype)
                for z in self.zero_outs]
        return cat

    def run(self, in_maps):
        out_arrs = self.sharded(*self.concat_inputs(in_maps))
        self.jax.block_until_ready(out_arrs)
        return [
            {n: np.asarray(out_arrs[i]).reshape(
                self.n_cores, *self.out_avals[i].shape)[c]
             for i, n in enumerate(self.out_names)}
            for c in range(self.n_cores)
        ]


# ---------------- host side ----------------

_RUNNER = None


def _make_inputs(x, importance, mask, compress_neurons, expand_pool,
                 knowledge_K, knowledge_V, Wc, WQ, WK, WV, Wm, WO,
                 g1, b1, g2, b2):
    ndt = np_bdt()
    f = lambda a: np.asarray(a, np.float32)
    cn = f(compress_neurons)
    pl = f(expand_pool)
    wstack = np.concatenate([f(Wc), f(WQ), f(WK), f(WV), f(Wm)], axis=0)
    wot = np.ascontiguousarray(f(WO).T)  # [D, D] = WO.T
    wotr = np.empty((P, NT * 1024), np.float32)
    for t in range(NT):
        wotr[:, 1024 * t:1024 * (t + 1)] = wot[128 * t:128 * (t + 1), :]
    kkt = np.ascontiguousarray(f(knowledge_K).T).astype(ndt)
    kv = f(knowledge_V).astype(ndt)

    # aux (core-independent parts)
    auxb = np.zeros((P, AB_W), np.float32)
    auxb[:, AB_IDN:AB_IDN + P] = np.eye(P)
    auxb[:, AB_BMS:AB_BMS + 32] = (
        (np.arange(P)[:, None] // 4) == np.arange(32)[None, :])
    ktri = np.arange(P)
    auxb[:, AB_TRI:AB_TRI + P] = np.where(
        ktri[None, :] >= ktri[:, None], 0.0, NEG)  # tri[k, q]; rest zeros
    auxb = auxb.astype(ndt)

    gb = np.stack([f(g1).ravel(), f(b1).ravel(), f(g2).ravel(),
                   f(b2).ravel()]).reshape(4, D)

    x = f(x); importance = f(importance)
    in_maps = []
    for c in range(N_CORES):
        b, hf = c // 2, c % 2
        qr = np.arange(hf * SQ, hf * SQ + SQ)
        rest = np.arange((1 - hf) * SQ, (1 - hf) * SQ + SQ)
        perm = np.concatenate([qr, rest])
        eperm = (np.arange(NEXP) + NLOC * hf) % NEXP  # local experts first

        m = {}
        m["x"] = np.ascontiguousarray(x[b][perm])
        impc = importance[b][perm].reshape(NT, P).T  # [p, i]
        m["imp"] = np.ascontiguousarray(impc).astype(np.float32)

        # wct: [128, 8*320]; block order [Wc|WQ|WK|WV|Wm], experts permuted
        wp_ = wstack.reshape(5, NEXP, D)[:, eperm, :].reshape(5 * NEXP, D)
        wctT = wp_.T  # [D, 320]
        wcth = np.empty((P, NT * 320), np.float32)
        for t in range(NT):
            wcth[:, 320 * t:320 * (t + 1)] = wctT[128 * t:128 * (t + 1), :]
        m["wct"] = np.ascontiguousarray(wcth).astype(ndt)
        m["wot"] = wotr.astype(ndt)
        m["kkt"] = kkt
        m["kv"] = kv
        m["gb"] = gb

        # cnb[b4, g, p, 128t + r] = cn[e(g,p), 128t + 32*b4 + p//4, r]
        loc = eperm[:NLOC]
        cl = cn[loc]                      # [32, D, R]
        clr = cl.reshape(NLOC, 8, 128, R) # [n, t, dsub, r]
        # dsub = 32*b4 + p//4 ; partition p = 4*(p//4) + n%4
        cnb = np.empty((4, GLOC, P, 1024), np.float32)
        for b4 in range(4):
            blk = clr[:, :, 32 * b4:32 * (b4 + 1), :]   # [n, t, 32, r]
            for g in range(GLOC):
                for e in range(4):
                    n = 4 * g + e
                    # partition p = 4*m + e (m = dsub idx), free = 128t + r
                    cnb[b4, g, e::4, :] = blk[n].transpose(1, 0, 2).reshape(
                        32, 8 * 128)
        m["cnb"] = np.ascontiguousarray(cnb).astype(ndt)

        pll = pl[loc]                     # [32, R, D]
        plb = np.empty((4, GLOC, P, 1024), np.float32)
        for b4 in range(4):
            blk = pll[:, 32 * b4:32 * (b4 + 1), :]      # [n, 32, D]
            for g in range(GLOC):
                for e in range(4):
                    plb[b4, g, e::4, :] = blk[4 * g + e]
        m["plb"] = np.ascontiguousarray(plb).astype(ndt)

        auxf = np.zeros((P, AF_W), np.float32)
        auxf[:NLOC, AF_A4:AF_A4 + P] = (
            (np.arange(NLOC)[:, None] % 4) == (np.arange(P)[None, :] % 4))
        auxf[:NLOC, AF_B8:AF_B8 + GLOC] = (
            (np.arange(NLOC)[:, None] // 4) == np.arange(GLOC)[None, :])
        onehot = np.zeros(B, np.float32); onehot[b] = 1.0
        auxf[0:1, AF_BSELR:AF_BSELR + B] = onehot[None, :]
        auxf[0:B, AF_BSELC:AF_BSELC + 1] = onehot[:, None]
        auxf[:, AF_NEGC] = NEG if hf == 0 else 0.0
        auxf[0, AF_ONE] = 1.0
        m["auxf"] = auxf
        m["auxb"] = auxb
        in_maps.append(m)
    return in_maps


def _get_runner():
    global _RUNNER
    if _RUNNER is None:
        nc = build_nc(use_cc=True)
        _RUNNER = SpmdRunner(nc, N_CORES)
    return _RUNNER


def kernel(**inputs):
    r = _get_runner()
    in_maps = _make_inputs(**inputs)
    res = r.run(in_maps)
    out = np.empty((B, S, D), np.float32)
    for c in range(N_CORES):
        b, hf = c // 2, c % 2
        out[b, hf * SQ:(hf + 1) * SQ] = res[c]["o"]
    return out


# revision 15
# speedup vs baseline: 1.0314x; 1.0314x over previous
"""DAWN block (moe_routing) Trainium2 kernel: 8-core SPMD, v2.

Sharding: core c = (batch b=c//2, half h=c%2). Each core handles one batch's
attention + memory block for half the queries (rows permuted so local queries
come first). Expert pools (compress_neurons / expand_pool) are pair-sharded:
each core streams only 32 of 64 experts and partial combines are AllReduced
within the pair. Causal structure: per-q-slot key-position lists + a constant
triangular mask tile + per-core bias column (full-mask blocks), so ~19% of
score/AV work is skipped and no per-key mask tensor is needed.

DMA strategy: everything is host-relaid-out so the device does few, large,
contiguous DMAs (the v1 kernel's 456 tiny combine DMAs were the bottleneck:
each DMA costs ~0.6us queue dispatch + 625ns shared HWDGE serial time).
"""
import numpy as np
import ml_dtypes

import concourse.bass as bass
import concourse.mybir as mybir
import concourse.tile as tile
from concourse import bacc

B, S, D = 4, 1024, 1024
H, DH = 16, 64
R = 128
NEXP = 64
NLOC = 32          # local experts per core (pair-sharded)
GLOC = NLOC // 4   # 8 stacked-expert groups
NK, KR = 4096, 128
TOPK = 8
N_CORES = 8
SQ = S // 2
P = 128
NT = S // P        # 8 seq tiles
QT = SQ // P       # 4 local q tiles
DT_T = D // P      # 8 d tiles

F32 = mybir.dt.float32
U32 = mybir.dt.uint32
NEG = -1.0e9
ALU = mybir.AluOpType
ACT = mybir.ActivationFunctionType
AX = None

# aux column maps
AF_A4 = 0          # [0:32, 0:128]
AF_B8 = 128        # [0:32, 128:136]
AF_BSELR = 136     # [0:1, 136:140]
AF_BSELC = 140     # [0:4, 140:141]
AF_NEGC = 141      # [0:128, 141:142]
AF_ONE = 142       # [0:1, 142:143]  value 1.0
AF_ONES = 144      # [0:1, 144:272] row of ones
AF_G1 = 272        # [0:128, 272:280] g1 tiled [p, t]
AF_G2 = 280        # [0:128, 280:288] g2 tiled
AF_B1 = 288        # [0:128, 288:296] b1 tiled
AF_B2 = 296        # [0:128, 296:304] b2 tiled
AF_BR1 = 304       # [0:1, 304:560] b1 @ [Wc|WQ|WK|WV].T (perm)
AF_BR2 = 560       # [0:1, 560:624] b2 @ Wm.T (perm)
AF_W = 624
AB_IDN = 0         # [0:128, 0:128]
AB_BMS = 128       # [0:128, 128:160]
AB_TRI = 160       # [0:128, 160:672] = [tri | zeros x3]
AB_W = 672


def bdt():
    return mybir.dt.bfloat16


def np_bdt():
    return ml_dtypes.bfloat16


def build_nc(use_cc=True):
    global AX
    AX = mybir.AxisListType.X
    DT = bdt()
    nc = bacc.Bacc("TRN2", target_bir_lowering=False, debug=False,
                   num_devices=N_CORES)
    I = {}

    def inp(name, shape, dt):
        I[name] = nc.dram_tensor(name, shape, dt, kind="ExternalInput").ap()

    inp("x", [S, D], F32)              # row-permuted batch (local q first)
    inp("imp", [P, NT], F32)           # imp[p,i] = importance[perm[128i+p]]
    inp("cnb", [4, GLOC, P, 1024], DT) # local-expert compress pool, relaid
    inp("plb", [4, GLOC, P, 1024], DT) # local-expert expand pool, relaid
    inp("wct", [P, NT * 320], DT)      # [Wc|WQ|WK|WV|Wm].T tiled (expert-perm)
    inp("wot", [P, NT * 1024], DT)     # WO.T tiled
    inp("kkt", [KR, NK], DT)           # knowledge_K.T
    inp("kv", [NK, D], DT)             # knowledge_V
    inp("auxf", [P, AF_W], F32)
    inp("auxb", [P, AB_W], DT)
    o = nc.dram_tensor("o", [SQ, D], F32, kind="ExternalOutput").ap()

    with tile.TileContext(nc) as tc:
        _body(nc, tc, I, o, use_cc)
    nc.compile()
    return nc


def _body(nc, tc, I, o, use_cc):
    DT = bdt()
    import contextlib
    ctx = contextlib.ExitStack()
    with ctx:
        pp = ctx.enter_context(tc.tile_pool(name="pers", bufs=1))
        sp = ctx.enter_context(tc.tile_pool(name="stream", bufs=2))
        st2 = ctx.enter_context(tc.tile_pool(name="strm", bufs=3))
        pst = ctx.enter_context(tc.tile_pool(name="ps_t", bufs=2, space="PSUM"))
        psa = ctx.enter_context(tc.tile_pool(name="ps_a", bufs=2, space="PSUM"))
        psv = ctx.enter_context(tc.tile_pool(name="ps_v", bufs=2, space="PSUM"))
        dr = ctx.enter_context(tc.tile_pool(name="dram", bufs=1, space="DRAM"))

        # ---------- bulk loads ----------
        xa = pp.tile([P, NT * 1024], F32, tag="xa", name="xa")
        for hh in range(2):
            src = bass.AP(I["x"].tensor, hh * 4 * P * 1024,
                          [[1024, P], [P * 1024, 4], [1, 1024]])
            nc.sync.dma_start(out=xa[:, hh * 4096:(hh + 1) * 4096], in_=src)
        wct = pp.tile([P, NT * 320], DT, tag="wct", name="wct")
        nc.sync.dma_start(out=wct[:], in_=I["wct"][:])
        impa = pp.tile([P, NT], F32, tag="impa", name="impa")
        nc.sync.dma_start(out=impa[:], in_=I["imp"][:])
        auxf = pp.tile([P, AF_W], F32, tag="auxf", name="auxf")
        nc.sync.dma_start(out=auxf[:], in_=I["auxf"][:])
        auxb = pp.tile([P, AB_W], DT, tag="auxb", name="auxb")
        nc.sync.dma_start(out=auxb[:], in_=I["auxb"][:])

        idn = auxb[:, AB_IDN:AB_IDN + P]
        bmS = auxb[:, AB_BMS:AB_BMS + 32]
        trix = auxb[:, AB_TRI:AB_TRI + 4 * P]
        A4 = auxf[0:NLOC, AF_A4:AF_A4 + P]
        B8 = auxf[0:NLOC, AF_B8:AF_B8 + GLOC]
        bselr = auxf[0:1, AF_BSELR:AF_BSELR + B]
        bselc = auxf[0:B, AF_BSELC:AF_BSELC + 1]
        negc = auxf[:, AF_NEGC:AF_NEGC + 1]
        one1 = auxf[0:1, AF_ONE:AF_ONE + 1]
        ones128 = auxf[0:1, AF_ONES:AF_ONES + P]
        g1t = auxf[:, AF_G1:AF_G1 + NT]
        g2t = auxf[:, AF_G2:AF_G2 + NT]
        b1t = auxf[:, AF_B1:AF_B1 + NT]
        b2t = auxf[:, AF_B2:AF_B2 + NT]
        brow1 = auxf[0:1, AF_BR1:AF_BR1 + 256]
        brow2 = auxf[0:1, AF_BR2:AF_BR2 + NEXP]

        # expert pool streams in half-b chunks: [128, 4096] = 4 g-groups.
        def stream_half(tensor, b, half):
            t = st2.tile([P, 4096], DT, tag="strm", name="strm")
            srcap = bass.AP(tensor, (b * GLOC + half * 4) * P * 1024,
                            [[1024, P], [P * 1024, 4], [1, 1024]])
            nc.sync.dma_start(out=t[:], in_=srcap)
            return t

        cn1 = [[stream_half(I["cnb"].tensor, b, h) for h in range(2)]
               for b in range(4)]
        pl1 = [[stream_half(I["plb"].tensor, b, h) for h in range(2)]
               for b in range(4)]
        wota2 = []
        for h in range(2):
            t = st2.tile([P, 4096], DT, tag="strm", name="strm")
            nc.sync.dma_start(out=t[:], in_=I["wot"][:, h * 4096:(h + 1) * 4096])
            wota2.append(t)
        cn2 = [[stream_half(I["cnb"].tensor, b, h) for h in range(2)]
               for b in range(4)]

        def copy_ps(out_ap, in_ap, k):
            if k % 2 == 0:
                nc.vector.tensor_copy(out=out_ap, in_=in_ap)
            else:
                nc.scalar.activation(out=out_ap, in_=in_ap, func=ACT.Identity)

        def layernorm_tile(x_ap, pool, tag):
            # var = E[x^2] - mean^2; one Square pass on raw x overlaps the
            # DVE sum; final pass folds (x - mean) * rstd into one activation.
            stats = sp.tile([P, 6], F32, tag="ln_stats", name="ln_stats")
            mean = stats[:, 0:1]; ex2 = stats[:, 1:2]; rstd = stats[:, 2:3]
            nmr = stats[:, 3:4]; var = stats[:, 4:5]; m2 = stats[:, 5:6]
            sq = sp.tile([P, D], F32, tag="ln_sq", name="ln_sq", bufs=1)
            nc.scalar.activation(out=sq[:], in_=x_ap, func=ACT.Square,
                                 accum_out=ex2)
            nc.vector.tensor_reduce(out=mean, in_=x_ap, axis=AX, op=ALU.add)
            nc.vector.tensor_scalar(out=mean, in0=mean, scalar1=1.0 / D,
                                    scalar2=None, op0=ALU.mult)
            nc.vector.tensor_tensor(out=m2, in0=mean, in1=mean, op=ALU.mult)
            nc.vector.tensor_scalar(out=var, in0=ex2, scalar1=1.0 / D,
                                    scalar2=1e-5, op0=ALU.mult, op1=ALU.add)
            nc.vector.tensor_tensor(out=var, in0=var, in1=m2, op=ALU.subtract)
            nc.scalar.sqrt(rstd, var)
            nc.vector.reciprocal(rstd, rstd)
            nc.vector.tensor_tensor(out=nmr, in0=mean, in1=rstd, op=ALU.mult)
            nc.vector.tensor_scalar(out=nmr, in0=nmr, scalar1=-1.0,
                                    scalar2=None, op0=ALU.mult)
            out = pool.tile([P, D], DT, tag=tag)
            nc.scalar.activation(out=out[:], in_=x_ap, func=ACT.Identity,
                                 scale=rstd, bias=nmr)
            return out

        def softmax_pool(psum_ap, out_ap, nblk, blk, imp_col, pool_out,
                         first, last):
            # exp (no max-sub; scores are O(1)) with per-block accum, then
            # pool with 1/Z folded into the importance column.
            zs = sp.tile([P, 8], F32, tag="sm_zs", name="sm_zs")
            for bi in range(nblk):
                sl = slice(bi * blk, (bi + 1) * blk)
                nc.scalar.activation(out=out_ap[:, sl], in_=psum_ap[:, sl],
                                     func=ACT.Exp, accum_out=zs[:, bi:bi + 1])
            nc.vector.reciprocal(zs[:, 0:nblk], zs[:, 0:nblk])
            impz = sp.tile([P, 8], F32, tag="sm_iz", name="sm_iz")
            nc.vector.tensor_scalar(out=impz[:, 0:nblk], in0=zs[:, 0:nblk],
                                    scalar1=imp_col, scalar2=None,
                                    op0=ALU.mult)
            for bi in range(nblk):
                sl = slice(bi * blk, (bi + 1) * blk)
                nc.tensor.matmul(out=pool_out[:, sl],
                                 lhsT=impz[:, bi:bi + 1], rhs=out_ap[:, sl],
                                 start=first, stop=last)

        def group_cols(wcol_ap, ncols):
            """wcol [32, ncols] f32 -> wk [128, GLOC*ncols]:
            wk[p, ncols*g + c] = wcol[4g + p%4, c]."""
            rhsB = sp.tile([NLOC, GLOC * ncols], F32, tag="rhsB", name="rhsB")
            for pi in range(ncols):
                nc.vector.tensor_scalar(
                    out=rhsB[:, pi:GLOC * ncols:ncols], in0=B8,
                    scalar1=wcol_ap[:, pi:pi + 1], scalar2=None, op0=ALU.mult)
            wkp = pst.tile([P, GLOC * ncols], F32, tag="tpp", name="wkp")
            nc.tensor.matmul(out=wkp[:], lhsT=A4, rhs=rhsB[:],
                             start=True, stop=True)
            wk = sp.tile([P, GLOC * ncols], F32, tag="wkall", name="wkall")
            nc.vector.tensor_copy(out=wk[:], in_=wkp[:])
            return wk

        def combine_cn(wcol_ap, chunks, out_f32):
            """out_f32 [128, 1024] f32 partial combine of local experts.
            chunks[b][p, 1024g+128t+r] = CN[e(g,p), 128t+32b+p//4, r]."""
            wk = group_cols(wcol_ap, 1)
            lhs = []
            for g in range(GLOC):
                lg = sp.tile([P, NLOC], DT, tag=f"clh{g}", name=f"clh{g}",
                             bufs=1)
                nc.vector.tensor_scalar(out=lg[:], in0=bmS,
                                        scalar1=wk[:, g:g + 1],
                                        scalar2=None, op0=ALU.mult)
                lhs.append(lg)
            for b in range(4):
                acc = psa.tile([NLOC, 1024], F32, tag="acc", name="cacc")
                for hh in range(2):
                    for g in range(GLOC):
                        gh, gl = g // 4, g % 4
                        nc.tensor.matmul(
                            out=acc[:, hh * 512:(hh + 1) * 512],
                            lhsT=lhs[g][:],
                            rhs=chunks[b][gh][:, gl * 1024 + hh * 512:
                                              gl * 1024 + (hh + 1) * 512],
                            start=(g == 0), stop=(g == GLOC - 1))
                copy_ps(out_f32[32 * b:32 * b + 32, :], acc[:], b)

        def pair_allreduce(sb_f32, ncol):
            """AllReduce sb_f32 [128, ncol] within batch pairs (in place)."""
            if not use_cc:
                return
            cc_in = dr.tile([P, ncol], F32)
            cc_out = dr.tile([P, ncol], F32)
            nc.gpsimd.dma_start(out=cc_in[:], in_=sb_f32[:])
            nc.gpsimd.collective_compute(
                "AllReduce", ALU.add,
                replica_groups=[[0, 1], [2, 3], [4, 5], [6, 7]],
                ins=[cc_in.opt()], outs=[cc_out.opt()])
            nc.gpsimd.dma_start(out=sb_f32[:], in_=cc_out[:])

        # ---------- LN1 + transposes ----------
        ctx4 = contextlib.ExitStack()
        p4 = ctx4.enter_context(tc.tile_pool(name="ph4", bufs=1))
        with tc.tile_pool(name="ph0", bufs=1) as p0:
            nxT = [p0.tile([P, S], DT, tag=f"nxT{t}", name=f"nxT{t}")
                   for t in range(DT_T)]
            for i in range(NT):
                nx_i = layernorm_tile(xa[:, i * 1024:(i + 1) * 1024], sp, "nx")
                for t in range(DT_T):
                    tp = pst.tile([P, P], DT, tag="tpp", name="tpp")
                    nc.tensor.transpose(out=tp[:],
                                        in_=nx_i[:, t * P:(t + 1) * P],
                                        identity=idn)
                    copy_ps(nxT[t][:, i * P:(i + 1) * P], tp[:], t)

            # ---------- routers (c,q,k,v) ----------
            wpool_ps = psv.tile([1, 4 * NEXP], F32, tag="pvacc", name="pvacc")
            for i in range(NT):
                pr_ps = psa.tile([P, 4 * NEXP], F32, tag="acc", name="acc")
                for t in range(DT_T):
                    nc.tensor.matmul(out=pr_ps[:],
                                     lhsT=nxT[t][:, i * P:(i + 1) * P],
                                     rhs=wct[:, 320 * t:320 * t + 256],
                                     start=(t == 0), stop=False)
                nc.tensor.matmul(out=pr_ps[:], lhsT=ones128, rhs=brow1,
                                 start=False, stop=True)
                pref = sp.tile([P, 4 * NEXP], F32, tag="pref", name="pref")
                softmax_pool(pr_ps[:], pref[:], 4, NEXP, impa[:, i:i + 1],
                             wpool_ps, first=(i == 0), last=(i == NT - 1))

            wrow = pp.tile([1, 4 * NEXP], F32, tag="wrow", name="wrow")
            nc.vector.tensor_copy(out=wrow[:], in_=wpool_ps[:])
            for bi in range(4):
                sl = slice(bi * NEXP, (bi + 1) * NEXP)
                st = sp.tile([1, 1], F32, tag="wn_st", name="wn_st")
                nc.vector.tensor_reduce(out=st[:], in_=wrow[:, sl], axis=AX,
                                        op=ALU.add)
                nc.vector.tensor_scalar(out=st[:], in0=st[:], scalar1=1e-8,
                                        scalar2=None, op0=ALU.add)
                nc.vector.reciprocal(st[:], st[:])
                nc.vector.tensor_scalar(out=wrow[:, sl], in0=wrow[:, sl],
                                        scalar1=st[:], scalar2=None,
                                        op0=ALU.mult)
            wt0 = pst.tile([P, 1], F32, tag="tpp", name="wt0")
            nc.tensor.transpose(out=wt0[:], in_=wrow[:, 0:P], identity=one1)
            wt1 = pst.tile([P, 1], F32, tag="tpp", name="wt1")
            nc.tensor.transpose(out=wt1[:], in_=wrow[:, P:2 * P], identity=one1)
            wcolcq = pp.tile([P, 1], F32, tag="wcolcq", name="wcolcq")
            nc.vector.tensor_copy(out=wcolcq[:], in_=wt0[:])
            wcolkv = pp.tile([P, 1], F32, tag="wcolkv", name="wcolkv")
            nc.vector.tensor_copy(out=wcolkv[:], in_=wt1[:])
            wcols3 = pp.tile([NLOC, 3], F32, tag="wcols3", name="wcols3")
            nc.vector.tensor_copy(out=wcols3[:, 0:1],
                                  in_=wcolcq[NEXP:NEXP + NLOC, :])
            nc.vector.tensor_copy(out=wcols3[:, 1:2], in_=wcolkv[0:NLOC, :])
            nc.vector.tensor_copy(out=wcols3[:, 2:3],
                                  in_=wcolkv[NEXP:NEXP + NLOC, :])

            # ---------- sc combine (+pair AllReduce) ----------
            e3f = p0.tile([P, 3072], F32, tag="e3f", name="e3f")
            scf = e3f[:, 0:1024]
            combine_cn(wcolcq[0:NLOC, 0:1], cn1, scf)
            pair_allreduce(scf, 1024)
            sc_b = p0.tile([P, 1024], DT, tag="sc_b", name="sc_b")
            for t in range(DT_T):
                nc.vector.tensor_scalar(out=sc_b[:, t * P:(t + 1) * P],
                                        in0=scf[:, t * P:(t + 1) * P],
                                        scalar1=g1t[:, t:t + 1], scalar2=None,
                                        op0=ALU.mult)

            # ---------- e3 combine ----------
            w3 = group_cols(wcols3[:], 3)  # [128, 24]
            lhs3 = []
            for g in range(GLOC):
                lg = p0.tile([P, 96], DT, tag=f"e3lh{g}", name=f"e3lh{g}")
                for pl_i in range(3):
                    nc.vector.tensor_scalar(
                        out=lg[:, 32 * pl_i:32 * (pl_i + 1)], in0=bmS,
                        scalar1=w3[:, 3 * g + pl_i:3 * g + pl_i + 1],
                        scalar2=None, op0=ALU.mult)
                lhs3.append(lg)
            for b in range(4):
                acc = psa.tile([96, 1024], F32, tag="acc", name="eacc")
                for hh in range(2):
                    for g in range(GLOC):
                        gh, gl = g // 4, g % 4
                        nc.tensor.matmul(
                            out=acc[:, hh * 512:(hh + 1) * 512],
                            lhsT=lhs3[g][:],
                            rhs=pl1[b][gh][:, gl * 1024 + hh * 512:
                                           gl * 1024 + (hh + 1) * 512],
                            start=(g == 0), stop=(g == GLOC - 1))
                for pl_i in range(3):
                    copy_ps(e3f[32 * b:32 * b + 32,
                                1024 * pl_i:1024 * (pl_i + 1)],
                            acc[32 * pl_i:32 * pl_i + 32, :], b + pl_i)
            pair_allreduce(e3f, 3072)
            e3 = p0.tile([P, 3072], DT, tag="e3", name="e3")
            nc.vector.tensor_copy(out=e3[:, 0:1024], in_=e3f[:, 0:1024])
            nc.scalar.activation(out=e3[:, 1024:2048], in_=e3f[:, 1024:2048],
                                 func=ACT.Identity)
            nc.gpsimd.tensor_copy(out=e3[:, 2048:3072], in_=e3f[:, 2048:3072])

            # ---------- hT[r, q] = sum_d sc[d, r] g1[d] nx[q, d] + (b1 @ sc g1)[r]
            bsc_ps = pst.tile([1, P], F32, tag="tpp", name="bscp")
            for t in range(DT_T):
                nc.tensor.matmul(out=bsc_ps[:], lhsT=b1t[:, t:t + 1],
                                 rhs=sc_b[:, t * P:(t + 1) * P],
                                 start=(t == 0), stop=(t == DT_T - 1))
            bsc_row = sp.tile([1, P], F32, tag="bscr", name="bscr")
            nc.vector.tensor_copy(out=bsc_row[:], in_=bsc_ps[:])
            bsc_t = pst.tile([P, 1], F32, tag="tpp", name="bsct")
            nc.tensor.transpose(out=bsc_t[:], in_=bsc_row[:], identity=one1)
            bsc = sp.tile([P, 1], F32, tag="bsc", name="bsc")
            nc.vector.tensor_copy(out=bsc[:], in_=bsc_t[:])
            hT = p0.tile([P, S], DT, tag="hT")
            for j in range(2):
                hp = psa.tile([P, 512], F32, tag="acc", name="hacc")
                for t in range(DT_T):
                    nc.tensor.matmul(out=hp[:],
                                     lhsT=sc_b[:, t * P:(t + 1) * P],
                                     rhs=nxT[t][:, j * 512:(j + 1) * 512],
                                     start=(t == 0), stop=(t == DT_T - 1))
                nc.scalar.activation(out=hT[:, j * 512:(j + 1) * 512],
                                     in_=hp[:], func=ACT.Identity, bias=bsc)

            # ---------- K, Q, V ----------
            SCALE_Q = 1.0 / float(np.sqrt(DH))
            kT = [p4.tile([P, S], DT, tag=f"kT{t}", name=f"kT{t}")
                  for t in range(DT_T)]
            qT = [p4.tile([P, SQ], DT, tag=f"qT{t}", name=f"qT{t}")
                  for t in range(DT_T)]
            vext = [p4.tile([P, H * (DH + 1)], DT, tag=f"vx{i}", name=f"vx{i}")
                    for i in range(NT)]
            for t in range(DT_T):
                kp = psa.tile([P, S], F32, tag="acc", name="acc")
                for j in range(2):
                    nc.tensor.matmul(out=kp[:, j * 512:(j + 1) * 512],
                                     lhsT=e3[:, 1024 + t * P:1024 + t * P + P],
                                     rhs=hT[:, j * 512:(j + 1) * 512],
                                     start=True, stop=True)
                nc.scalar.activation(out=kT[t][:], in_=kp[:], func=ACT.Identity)
                qp = psv.tile([P, SQ], F32, tag="pvacc", name="qacc")
                nc.tensor.matmul(out=qp[:], lhsT=e3[:, t * P:t * P + P],
                                 rhs=hT[:, 0:SQ], start=True, stop=True)
                nc.vector.tensor_scalar(out=qT[t][:], in0=qp[:],
                                        scalar1=SCALE_Q, scalar2=None,
                                        op0=ALU.mult)
            for i in range(NT):
                vp = psa.tile([P, D], F32, tag="acc", name="acc")
                for j in range(2):
                    nc.tensor.matmul(
                        out=vp[:, j * 512:(j + 1) * 512],
                        lhsT=hT[:, i * P:(i + 1) * P],
                        rhs=e3[:, 2048 + j * 512:2048 + (j + 1) * 512],
                        start=True, stop=True)
                vv = vext[i][:].rearrange("p (hh c) -> p hh c", c=DH + 1)
                nc.vector.tensor_copy(
                    out=vv[:, :, 0:DH],
                    in_=vp[:].rearrange("p (hh c) -> p hh c", c=DH))
                nc.gpsimd.memset(vv[:, :, DH:DH + 1], 1.0)
        # ph0 (nxT, scf, e3f, lhs3) released

        # ---------- attention ----------
        # q-slot s covers local q-tile s; key positions {0..s} u {4..7}.
        # position j==s gets the constant tri mask via PE; positions 4..7 get
        # the per-core bias column (0 or -1e9) folded into the exp.
        attnT = [p4.tile([P, SQ], DT, tag=f"at{t}", name=f"at{t}")
                 for t in range(DT_T)]
        for hd in range(H):
            t4 = hd // 2
            hs = (hd % 2) * DH
            po = psv.tile([DH + 1, SQ], F32, tag="pvacc", name="poacc")
            for j in range(NT):
                qlo = j * P if j < QT else 0
                w = SQ - qlo
                sps = psa.tile([P, SQ], F32, tag="acc", name="sacc")
                if j < QT:
                    nc.tensor.matmul(out=sps[:, 0:P],
                                     lhsT=kT[t4][hs:hs + DH, j * P:(j + 1) * P],
                                     rhs=qT[t4][hs:hs + DH, qlo:qlo + P],
                                     start=True, stop=False)
                    if w > P:
                        nc.tensor.matmul(out=sps[:, P:w],
                                         lhsT=kT[t4][hs:hs + DH,
                                                     j * P:(j + 1) * P],
                                         rhs=qT[t4][hs:hs + DH, qlo + P:SQ],
                                         start=True, stop=True)
                else:
                    nc.tensor.matmul(out=sps[:, 0:w],
                                     lhsT=kT[t4][hs:hs + DH, j * P:(j + 1) * P],
                                     rhs=qT[t4][hs:hs + DH, qlo:SQ],
                                     start=True, stop=True)
                if j < QT:
                    nc.tensor.matmul(out=sps[:, 0:P], lhsT=idn,
                                     rhs=trix[:, 0:P], start=False, stop=True)
                pt = sp.tile([P, SQ], DT, tag="p_tile", name="p_tile", bufs=3)
                if j < QT:
                    nc.scalar.activation(out=pt[:, 0:w], in_=sps[:, 0:w],
                                         func=ACT.Exp)
                else:
                    nc.scalar.activation(out=pt[:, 0:w], in_=sps[:, 0:w],
                                         func=ACT.Exp, bias=negc)
                nc.tensor.matmul(
                    out=po[:, qlo:SQ],
                    lhsT=vext[j][:, hd * (DH + 1):(hd + 1) * (DH + 1)],
                    rhs=pt[:, 0:w], start=(j == 0), stop=(j == NT - 1))
            rec = sp.tile([1, SQ], F32, tag="rec", name="rec")
            nc.vector.reciprocal(rec[:], po[DH:DH + 1, :])
            recB = sp.tile([DH, SQ], F32, tag="recB", name="recB")
            nc.gpsimd.partition_broadcast(recB[:], rec[:])
            nc.vector.tensor_tensor(out=attnT[t4][hs:hs + DH, :],
                                    in0=po[0:DH, :], in1=recB[:], op=ALU.mult)

        # ---------- WO + residual (into xa) ----------
        for i in range(QT):
            wp = psa.tile([P, D], F32, tag="acc", name="acc")
            for j in range(2):
                for t in range(DT_T):
                    toff = 1024 * t + 512 * j
                    nc.tensor.matmul(
                        out=wp[:, j * 512:(j + 1) * 512],
                        lhsT=attnT[t][:, i * P:(i + 1) * P],
                        rhs=wota2[toff // 4096][:, toff % 4096:
                                                toff % 4096 + 512],
                        start=(t == 0), stop=(t == DT_T - 1))
            nc.vector.tensor_tensor(out=xa[:, i * 1024:(i + 1) * 1024],
                                    in0=wp[:], in1=xa[:, i * 1024:(i + 1) * 1024],
                                    op=ALU.add)

        ctx4.close()

        # ---------- memory block ----------
        with tc.tile_pool(name="ph6", bufs=1) as p6:
            nx2T = [p6.tile([P, SQ], DT, tag=f"n2T{t}", name=f"n2T{t}")
                    for t in range(DT_T)]
            kkt = p6.tile([KR, NK], DT, tag="kkt", name="kkt")
            nc.sync.dma_start(out=kkt[:], in_=I["kkt"][:])
            for i in range(QT):
                nx2_i = layernorm_tile(xa[:, i * 1024:(i + 1) * 1024], sp,
                                       "nx2")
                for t in range(DT_T):
                    tp = pst.tile([P, P], DT, tag="tpp", name="tpp")
                    nc.tensor.transpose(out=tp[:],
                                        in_=nx2_i[:, t * P:(t + 1) * P],
                                        identity=idn)
                    copy_ps(nx2T[t][:, i * P:(i + 1) * P], tp[:], t)

            mwp_ps = psv.tile([1, NEXP], F32, tag="pvacc", name="pvacc")
            for i in range(QT):
                pr = psa.tile([P, NEXP], F32, tag="acc", name="acc")
                for t in range(DT_T):
                    nc.tensor.matmul(out=pr[:],
                                     lhsT=nx2T[t][:, i * P:(i + 1) * P],
                                     rhs=wct[:, 320 * t + 256:320 * t + 320],
                                     start=(t == 0), stop=False)
                nc.tensor.matmul(out=pr[:], lhsT=ones128, rhs=brow2,
                                 start=False, stop=True)
                prefm = sp.tile([P, NEXP], F32, tag="prefm", name="prefm")
                softmax_pool(pr[:], prefm[:], 1, NEXP, impa[:, i:i + 1],
                             mwp_ps, first=(i == 0), last=(i == QT - 1))

            mwrow = p6.tile([1, NEXP], F32, tag="mwrow", name="mwrow")
            if use_cc:
                mwr = sp.tile([1, NEXP], F32, tag="mwr", name="mwr")
                nc.vector.tensor_copy(out=mwr[:], in_=mwp_ps[:])
                ccp = psa.tile([B, NEXP], F32, tag="acc", name="acc")
                nc.tensor.matmul(out=ccp[:], lhsT=bselr, rhs=mwr[:],
                                 start=True, stop=True)
                cc_sb = sp.tile([B, NEXP], F32, tag="cc_sb", name="cc_sb")
                nc.vector.tensor_copy(out=cc_sb[:], in_=ccp[:])
                cc_in = dr.tile([B, NEXP], F32)
                cc_out = dr.tile([B, NEXP], F32)
                nc.gpsimd.dma_start(out=cc_in[:], in_=cc_sb[:])
                nc.gpsimd.collective_compute(
                    "AllReduce", ALU.add,
                    replica_groups=[list(range(N_CORES))],
                    ins=[cc_in.opt()], outs=[cc_out.opt()])
                cc_res = sp.tile([B, NEXP], F32, tag="cc_res", name="cc_res")
                nc.gpsimd.dma_start(out=cc_res[:], in_=cc_out[:])
                mwf = psa.tile([1, NEXP], F32, tag="acc", name="acc")
                nc.tensor.matmul(out=mwf[:], lhsT=bselc, rhs=cc_res[:],
                                 start=True, stop=True)
                nc.vector.tensor_copy(out=mwrow[:], in_=mwf[:])
            else:
                nc.vector.tensor_copy(out=mwrow[:], in_=mwp_ps[:])
            st = sp.tile([1, 1], F32, tag="wn_st", name="wn_st")
            nc.vector.tensor_reduce(out=st[:], in_=mwrow[:], axis=AX,
                                    op=ALU.add)
            nc.vector.tensor_scalar(out=st[:], in0=st[:], scalar1=1e-8,
                                    scalar2=None, op0=ALU.add)
            nc.vector.reciprocal(st[:], st[:])
            nc.vector.tensor_scalar(out=mwrow[:], in0=mwrow[:], scalar1=st[:],
                                    scalar2=None, op0=ALU.mult)
            mwrow_cp = sp.tile([1, NEXP], F32, tag="mwr2", name="mwr2")
            nc.vector.tensor_copy(out=mwrow_cp[:], in_=mwrow[:])
            mwt = pst.tile([NEXP, 1], F32, tag="tpp", name="mwt")
            nc.tensor.transpose(out=mwt[:], in_=mwrow_cp[:], identity=one1)
            mwcol = p6.tile([NEXP, 1], F32, tag="mwcol", name="mwcol")
            nc.vector.tensor_copy(out=mwcol[:], in_=mwt[:])

            scmf = p6.tile([P, 1024], F32, tag="scmf", name="scmf")
            combine_cn(mwcol[0:NLOC, 0:1], cn2, scmf)
            pair_allreduce(scmf, 1024)
            scm_b = p6.tile([P, 1024], DT, tag="scm_b", name="scm_b")
            for t in range(DT_T):
                nc.vector.tensor_scalar(out=scm_b[:, t * P:(t + 1) * P],
                                        in0=scmf[:, t * P:(t + 1) * P],
                                        scalar1=g2t[:, t:t + 1], scalar2=None,
                                        op0=ALU.mult)

            # QmT [r, sq]
            qmp = psv.tile([P, SQ], F32, tag="pvacc", name="pvacc")
            for t in range(DT_T):
                nc.tensor.matmul(out=qmp[:], lhsT=scm_b[:, t * P:(t + 1) * P],
                                 rhs=nx2T[t][:], start=(t == 0),
                                 stop=(t == DT_T - 1))
            bscm_ps = pst.tile([1, P], F32, tag="tpp", name="bscmp")
            for t in range(DT_T):
                nc.tensor.matmul(out=bscm_ps[:], lhsT=b2t[:, t:t + 1],
                                 rhs=scm_b[:, t * P:(t + 1) * P],
                                 start=(t == 0), stop=(t == DT_T - 1))
            bscm_row = sp.tile([1, P], F32, tag="bscr", name="bscmr")
            nc.vector.tensor_scalar(out=bscm_row[:], in_=None, in0=bscm_ps[:],
                                    scalar1=1.0 / float(np.sqrt(KR)),
                                    scalar2=None, op0=ALU.mult)
            bscm_t = pst.tile([P, 1], F32, tag="tpp", name="bscmt")
            nc.tensor.transpose(out=bscm_t[:], in_=bscm_row[:], identity=one1)
            bscm = sp.tile([P, 1], F32, tag="bsc", name="bscm")
            nc.vector.tensor_copy(out=bscm[:], in_=bscm_t[:])
            qmT = p6.tile([P, SQ], DT, tag="qmT")
            nc.scalar.activation(out=qmT[:], in_=qmp[:], func=ACT.Identity,
                                 scale=1.0 / float(np.sqrt(KR)), bias=bscm)

            idx_all = p6.tile([P, QT * TOPK], U32, tag="idx_all",
                              name="idx_all")
            w8_all = p6.tile([P, QT * TOPK], F32, tag="w8_all", name="w8_all")
            for i in range(QT):
                ks = p6.tile([P, NK], F32, tag="ks_sb", name="ks_sb")
                for j in range(NK // 512):
                    ksp = psa.tile([P, 512], F32, tag="acc", name="acc")
                    nc.tensor.matmul(out=ksp[:],
                                     lhsT=qmT[:, i * P:(i + 1) * P],
                                     rhs=kkt[:, j * 512:(j + 1) * 512],
                                     start=True, stop=True)
                    eng = nc.scalar if (j % 2 == 0) else nc.vector
                    if eng is nc.scalar:
                        nc.scalar.activation(out=ks[:, j * 512:(j + 1) * 512],
                                             in_=ksp[:], func=ACT.Identity)
                    else:
                        nc.vector.tensor_copy(out=ks[:, j * 512:(j + 1) * 512],
                                              in_=ksp[:])
                tv = sp.tile([P, TOPK], F32, tag="tv", name="tv")
                nc.vector.max_with_indices(
                    out_max=tv[:],
                    out_indices=idx_all[:, i * TOPK:(i + 1) * TOPK],
                    in_=ks[:])
                st8 = sp.tile([P, 2], F32, tag="st8", name="st8")
                nm = st8[:, 0:1]; se8 = st8[:, 1:2]
                nc.vector.tensor_scalar(out=nm, in0=tv[:, 0:1], scalar1=-1.0,
                                        scalar2=None, op0=ALU.mult)
                w8 = sp.tile([P, TOPK], F32, tag="w8", name="w8")
                nc.scalar.activation(out=w8[:], in_=tv[:], func=ACT.Exp,
                                     bias=nm, accum_out=se8)
                nc.vector.reciprocal(se8, se8)
                nc.vector.tensor_scalar(out=w8_all[:, i * TOPK:(i + 1) * TOPK],
                                        in0=w8[:], scalar1=se8, scalar2=None,
                                        op0=ALU.mult)

            for i in range(QT):
                acc = p6.tile([P, D], F32, tag="mem_acc", name="mem_acc", bufs=2)
                gt8 = p6.tile([P, TOPK * D], DT, tag="gath8", name="gath8", bufs=2)
                nc.gpsimd.indirect_dma_start(
                    out=gt8[:], out_offset=None, in_=I["kv"][:],
                    in_offset=bass.IndirectOffsetOnAxis(
                        ap=idx_all[:, i * TOPK:(i + 1) * TOPK], axis=0))
                accB = p6.tile([P, D], F32, tag="mem_accB", name="mem_accB",
                               bufs=2)
                tmp = sp.tile([P, D], F32, tag="gtmp", name="gtmp")
                for k in range(TOPK):
                    g = i * TOPK + k
                    if k % 2 == 0:
                        prev = (xa[:, i * 1024:(i + 1) * 1024] if k == 0
                                else acc[:])
                        nc.vector.scalar_tensor_tensor(
                            out=acc[:], in0=gt8[:, k * D:(k + 1) * D],
                            scalar=w8_all[:, g:g + 1],
                            in1=prev, op0=ALU.mult, op1=ALU.add)
                    else:
                        dst = accB[:] if k == 1 else tmp[:]
                        nc.scalar.activation(out=dst,
                                             in_=gt8[:, k * D:(k + 1) * D],
                                             func=ACT.Identity,
                                             scale=w8_all[:, g:g + 1])
                        if k > 1:
                            nc.gpsimd.tensor_tensor(out=accB[:], in0=tmp[:],
                                                    in1=accB[:], op=ALU.add)
                nc.vector.tensor_tensor(out=acc[:], in0=acc[:], in1=accB[:],
                                        op=ALU.add)
                nc.sync.dma_start(out=o[i * P:(i + 1) * P, :], in_=acc[:])


# ---------------- PJRT SPMD runner (persistent jit) ----------------

class SpmdRunner:
    def __init__(self, nc, n_cores):
        import jax
        from jax.sharding import Mesh, PartitionSpec
        from jax.experimental.shard_map import shard_map
        from concourse import bass2jax
        bass2jax.install_neuronx_cc_hook()
        self.jax = jax
        self.nc = nc
        self.n_cores = n_cores
        partition_name = (nc.partition_id_tensor.name
                          if nc.partition_id_tensor else None)
        in_names, out_names, out_avals, zero_outs = [], [], [], []
        for alloc in nc.m.functions[0].allocations:
            if not isinstance(alloc, mybir.MemoryLocationSet):
                continue
            name = alloc.memorylocations[0].name
            if alloc.kind == "ExternalInput":
                if name != partition_name:
                    in_names.append(name)
            elif alloc.kind == "ExternalOutput":
                shape = tuple(alloc.tensor_shape)
                dtype = mybir.dt.np(alloc.dtype)
                out_names.append(name)
                out_avals.append(jax.core.ShapedArray(shape, dtype))
                zero_outs.append(np.zeros(shape, dtype))
        self.n_params = len(in_names)
        self.in_names = list(in_names)
        self.out_names = out_names
        self.out_avals = out_avals
        self.zero_outs = zero_outs
        all_in = in_names + out_names + ([partition_name] if partition_name
                                         else [])

        def _body(*args):
            operands = list(args)
            if partition_name is not None:
                operands.append(bass2jax.partition_id_tensor())
            outs = bass2jax._bass_exec_p.bind(
                *operands, out_avals=tuple(out_avals), in_names=tuple(all_in),
                out_names=tuple(out_names), lowering_input_output_aliases=(),
                sim_require_finite=True, sim_require_nnan=True, nc=nc)
            return tuple(outs)

        devices = jax.devices()[:n_cores]
        self.mesh = Mesh(np.asarray(devices), ("core",))
        nspec = self.n_params + len(out_names)
        self.sharded = jax.jit(
            shard_map(_body, mesh=self.mesh,
                      in_specs=(PartitionSpec("core"),) * nspec,
                      out_specs=(PartitionSpec("core"),) * len(out_names),
                      check_rep=False),
            keep_unused=True)

    def concat_inputs(self, in_maps):
        per_core = [[np.asarray(m[n]) for n in self.in_names] for m in in_maps]
        cat = [np.concatenate([per_core[c][i] for c in range(self.n_cores)],
                              axis=0) for i in range(self.n_params)]
        cat += [np.zeros((self.n_cores * z.shape[0], *z.shape[1:]), z.dtype)
                for z in self.zero_outs]
        return cat

    def run(self, in_maps):
        out_arrs = self.sharded(*self.concat_inputs(in_maps))
        self.jax.block_until_ready(out_arrs)
        return [
            {n: np.asarray(out_arrs[i]).reshape(
                self.n_cores, *self.out_avals[i].shape)[c]
             for i, n in enumerate(self.out_names)}
            for c in range(self.n_cores)
        ]


# ---------------- host side ----------------

_RUNNER = None


def _make_inputs(x, importance, mask, compress_neurons, expand_pool,
                 knowledge_K, knowledge_V, Wc, WQ, WK, WV, Wm, WO,
                 g1, b1, g2, b2):
    ndt = np_bdt()
    f = lambda a: np.asarray(a, np.float32)
    cn = f(compress_neurons)
    pl = f(expand_pool)
    wstack = np.concatenate([f(Wc), f(WQ), f(WK), f(WV), f(Wm)], axis=0)
    wot = np.ascontiguousarray(f(WO).T)  # [D, D] = WO.T
    wotr = np.empty((P, NT * 1024), np.float32)
    for t in range(NT):
        wotr[:, 1024 * t:1024 * (t + 1)] = wot[128 * t:128 * (t + 1), :]
    kkt = np.ascontiguousarray(f(knowledge_K).T).astype(ndt)
    kv = f(knowledge_V).astype(ndt)

    # aux (core-independent parts)
    auxb = np.zeros((P, AB_W), np.float32)
    auxb[:, AB_IDN:AB_IDN + P] = np.eye(P)
    auxb[:, AB_BMS:AB_BMS + 32] = (
        (np.arange(P)[:, None] // 4) == np.arange(32)[None, :])
    ktri = np.arange(P)
    auxb[:, AB_TRI:AB_TRI + P] = np.where(
        ktri[None, :] >= ktri[:, None], 0.0, NEG)  # tri[k, q]; rest zeros
    auxb = auxb.astype(ndt)

    x = f(x); importance = f(importance)
    in_maps = []
    for c in range(N_CORES):
        b, hf = c // 2, c % 2
        qr = np.arange(hf * SQ, hf * SQ + SQ)
        rest = np.arange((1 - hf) * SQ, (1 - hf) * SQ + SQ)
        perm = np.concatenate([qr, rest])
        eperm = (np.arange(NEXP) + NLOC * hf) % NEXP  # local experts first

        m = {}
        m["x"] = np.ascontiguousarray(x[b][perm])
        impc = importance[b][perm].reshape(NT, P).T  # [p, i]
        m["imp"] = np.ascontiguousarray(impc).astype(np.float32)

        # wct: [128, 8*320]; block order [Wc|WQ|WK|WV|Wm], experts permuted.
        # g1 folded into router1 blocks, g2 into the Wm block (LN emits the
        # plain normalized value).
        wp_ = wstack.reshape(5, NEXP, D)[:, eperm, :].reshape(5 * NEXP, D)
        wp_ = wp_ * np.concatenate([np.tile(f(g1).ravel()[None, :], (4 * NEXP, 1)),
                                    np.tile(f(g2).ravel()[None, :], (NEXP, 1))])
        wctT = wp_.T  # [D, 320]
        wcth = np.empty((P, NT * 320), np.float32)
        for t in range(NT):
            wcth[:, 320 * t:320 * (t + 1)] = wctT[128 * t:128 * (t + 1), :]
        m["wct"] = np.ascontiguousarray(wcth).astype(ndt)
        m["wot"] = wotr.astype(ndt)
        m["kkt"] = kkt
        m["kv"] = kv

        # cnb[b4, g, p, 128t + r] = cn[e(g,p), 128t + 32*b4 + p//4, r]
        loc = eperm[:NLOC]
        cl = cn[loc]                      # [32, D, R]
        clr = cl.reshape(NLOC, 8, 128, R) # [n, t, dsub, r]
        # dsub = 32*b4 + p//4 ; partition p = 4*(p//4) + n%4
        cnb = np.empty((4, GLOC, P, 1024), np.float32)
        for b4 in range(4):
            blk = clr[:, :, 32 * b4:32 * (b4 + 1), :]   # [n, t, 32, r]
            for g in range(GLOC):
                for e in range(4):
                    n = 4 * g + e
                    # partition p = 4*m + e (m = dsub idx), free = 128t + r
                    cnb[b4, g, e::4, :] = blk[n].transpose(1, 0, 2).reshape(
                        32, 8 * 128)
        m["cnb"] = np.ascontiguousarray(cnb).astype(ndt)

        pll = pl[loc]                     # [32, R, D]
        plb = np.empty((4, GLOC, P, 1024), np.float32)
        for b4 in range(4):
            blk = pll[:, 32 * b4:32 * (b4 + 1), :]      # [n, 32, D]
            for g in range(GLOC):
                for e in range(4):
                    plb[b4, g, e::4, :] = blk[4 * g + e]
        m["plb"] = np.ascontiguousarray(plb).astype(ndt)

        auxf = np.zeros((P, AF_W), np.float32)
        auxf[:NLOC, AF_A4:AF_A4 + P] = (
            (np.arange(NLOC)[:, None] % 4) == (np.arange(P)[None, :] % 4))
        auxf[:NLOC, AF_B8:AF_B8 + GLOC] = (
            (np.arange(NLOC)[:, None] // 4) == np.arange(GLOC)[None, :])
        onehot = np.zeros(B, np.float32); onehot[b] = 1.0
        auxf[0:1, AF_BSELR:AF_BSELR + B] = onehot[None, :]
        auxf[0:B, AF_BSELC:AF_BSELC + 1] = onehot[:, None]
        auxf[:, AF_NEGC] = NEG if hf == 0 else 0.0
        auxf[0, AF_ONE] = 1.0
        auxf[0, AF_ONES:AF_ONES + P] = 1.0
        auxf[:, AF_G1:AF_G1 + NT] = f(g1).reshape(NT, P).T
        auxf[:, AF_G2:AF_G2 + NT] = f(g2).reshape(NT, P).T
        auxf[:, AF_B1:AF_B1 + NT] = f(b1).reshape(NT, P).T
        auxf[:, AF_B2:AF_B2 + NT] = f(b2).reshape(NT, P).T
        wr1 = wstack.reshape(5, NEXP, D)[:4, eperm, :]  # pre-g-fold rows
        auxf[0, AF_BR1:AF_BR1 + 256] = (wr1 * (f(g1).ravel() *
                                               0 + f(g1).ravel())[None, None, :]
                                        ).reshape(256, D) @ f(b1).ravel() * 0 \
            + (wr1 * f(g1).ravel()[None, None, :]).reshape(256, D) @ f(b1).ravel()
        wr2 = wstack.reshape(5, NEXP, D)[4, eperm, :]
        auxf[0, AF_BR2:AF_BR2 + NEXP] = (wr2 * f(g2).ravel()[None, :]) @ f(b2).ravel()
        m["auxf"] = auxf
        m["auxb"] = auxb
        in_maps.append(m)
    return in_maps


def _get_runner():
    global _RUNNER
    if _RUNNER is None:
        nc = build_nc(use_cc=True)
        _RUNNER = SpmdRunner(nc, N_CORES)
    return _RUNNER


def kernel(**inputs):
    r = _get_runner()
    in_maps = _make_inputs(**inputs)
    res = r.run(in_maps)
    out = np.empty((B, S, D), np.float32)
    for c in range(N_CORES):
        b, hf = c // 2, c % 2
        out[b, hf * SQ:(hf + 1) * SQ] = res[c]["o"]
    return out


# revision 17
# speedup vs baseline: 1.0992x; 1.0658x over previous
"""DAWN block (moe_routing) Trainium2 kernel: 8-core SPMD, v2.

Sharding: core c = (batch b=c//2, half h=c%2). Each core handles one batch's
attention + memory block for half the queries (rows permuted so local queries
come first). Expert pools (compress_neurons / expand_pool) are pair-sharded:
each core streams only 32 of 64 experts and partial combines are AllReduced
within the pair. Causal structure: per-q-slot key-position lists + a constant
triangular mask tile + per-core bias column (full-mask blocks), so ~19% of
score/AV work is skipped and no per-key mask tensor is needed.

DMA strategy: everything is host-relaid-out so the device does few, large,
contiguous DMAs (the v1 kernel's 456 tiny combine DMAs were the bottleneck:
each DMA costs ~0.6us queue dispatch + 625ns shared HWDGE serial time).
"""
import numpy as np
import ml_dtypes

import concourse.bass as bass
import concourse.mybir as mybir
import concourse.tile as tile
from concourse import bacc

B, S, D = 4, 1024, 1024
H, DH = 16, 64
R = 128
NEXP = 64
NLOC = 32          # local experts per core (pair-sharded)
GLOC = NLOC // 4   # 8 stacked-expert groups
NK, KR = 4096, 128
TOPK = 8
N_CORES = 8
SQ = S // 2
P = 128
NT = S // P        # 8 seq tiles
QT = SQ // P       # 4 local q tiles
DT_T = D // P      # 8 d tiles

F32 = mybir.dt.float32
U32 = mybir.dt.uint32
NEG = -1.0e9
ALU = mybir.AluOpType
ACT = mybir.ActivationFunctionType
AX = None

# aux column maps
AF_A4 = 0          # [0:32, 0:128]
AF_B8 = 128        # [0:32, 128:136]
AF_BSELR = 136     # [0:1, 136:140]
AF_BSELC = 140     # [0:4, 140:141]
AF_NEGC = 141      # [0:128, 141:142]
AF_ONE = 142       # [0:1, 142:143]  value 1.0
AF_ONES = 144      # [0:1, 144:272] row of ones
AF_G1 = 272        # [0:128, 272:280] g1 tiled [p, t]
AF_G2 = 280        # [0:128, 280:288] g2 tiled
AF_BR1 = 304       # [0:1, 304:560] b1 @ [Wc|WQ|WK|WV].T (perm)
AF_BR2 = 560       # [0:1, 560:624] b2 @ Wm.T (perm)
AF_W = 624
AB_IDN = 0         # [0:128, 0:128]
AB_BMS = 128       # [0:128, 128:160]
AB_TRI = 160       # [0:128, 160:672] = [tri | zeros x3]
AB_B1 = 672        # [0:128, 672:680] b1 tiled [p, t]
AB_B2 = 680        # [0:128, 680:688] b2 tiled
AB_W = 688


def bdt():
    return mybir.dt.bfloat16


def np_bdt():
    return ml_dtypes.bfloat16


def build_nc(use_cc=True):
    global AX
    AX = mybir.AxisListType.X
    DT = bdt()
    nc = bacc.Bacc("TRN2", target_bir_lowering=False, debug=False,
                   num_devices=N_CORES)
    I = {}

    def inp(name, shape, dt):
        I[name] = nc.dram_tensor(name, shape, dt, kind="ExternalInput").ap()

    inp("x", [S, D], F32)              # row-permuted batch (local q first)
    inp("imp", [P, NT], F32)           # imp[p,i] = importance[perm[128i+p]]
    inp("cnb", [4, GLOC, P, 1024], DT) # local-expert compress pool, relaid
    inp("plb", [4, GLOC, P, 1024], DT) # local-expert expand pool, relaid
    inp("wct", [P, NT * 320], DT)      # [Wc|WQ|WK|WV|Wm].T tiled (expert-perm)
    inp("wot", [P, NT * 1024], DT)     # WO.T tiled
    inp("kkt", [KR, NK], DT)           # knowledge_K.T
    inp("kv", [NK, D], DT)             # knowledge_V
    inp("auxf", [P, AF_W], F32)
    inp("auxb", [P, AB_W], DT)
    o = nc.dram_tensor("o", [SQ, D], F32, kind="ExternalOutput").ap()

    with tile.TileContext(nc) as tc:
        _body(nc, tc, I, o, use_cc)
    nc.compile()
    return nc


def _body(nc, tc, I, o, use_cc):
    DT = bdt()
    import contextlib
    ctx = contextlib.ExitStack()
    with ctx:
        pp = ctx.enter_context(tc.tile_pool(name="pers", bufs=1))
        sp = ctx.enter_context(tc.tile_pool(name="stream", bufs=2))
        st2 = ctx.enter_context(tc.tile_pool(name="strm", bufs=3))
        pst = ctx.enter_context(tc.tile_pool(name="ps_t", bufs=2, space="PSUM"))
        psa = ctx.enter_context(tc.tile_pool(name="ps_a", bufs=2, space="PSUM"))
        psv = ctx.enter_context(tc.tile_pool(name="ps_v", bufs=2, space="PSUM"))
        dr = ctx.enter_context(tc.tile_pool(name="dram", bufs=1, space="DRAM"))

        # ---------- bulk loads ----------
        xa = pp.tile([P, NT * 1024], F32, tag="xa", name="xa")
        for hh in range(2):
            src = bass.AP(I["x"].tensor, hh * 4 * P * 1024,
                          [[1024, P], [P * 1024, 4], [1, 1024]])
            nc.sync.dma_start(out=xa[:, hh * 4096:(hh + 1) * 4096], in_=src)
        wct = pp.tile([P, NT * 320], DT, tag="wct", name="wct")
        nc.sync.dma_start(out=wct[:], in_=I["wct"][:])
        impa = pp.tile([P, NT], F32, tag="impa", name="impa")
        nc.sync.dma_start(out=impa[:], in_=I["imp"][:])
        auxf = pp.tile([P, AF_W], F32, tag="auxf", name="auxf")
        nc.sync.dma_start(out=auxf[:], in_=I["auxf"][:])
        auxb = pp.tile([P, AB_W], DT, tag="auxb", name="auxb")
        nc.sync.dma_start(out=auxb[:], in_=I["auxb"][:])

        idn = auxb[:, AB_IDN:AB_IDN + P]
        bmS = auxb[:, AB_BMS:AB_BMS + 32]
        trix = auxb[:, AB_TRI:AB_TRI + 4 * P]
        A4 = auxf[0:NLOC, AF_A4:AF_A4 + P]
        B8 = auxf[0:NLOC, AF_B8:AF_B8 + GLOC]
        bselr = auxf[0:1, AF_BSELR:AF_BSELR + B]
        bselc = auxf[0:B, AF_BSELC:AF_BSELC + 1]
        negc = auxf[:, AF_NEGC:AF_NEGC + 1]
        one1 = auxf[0:1, AF_ONE:AF_ONE + 1]
        ones128 = auxf[0:1, AF_ONES:AF_ONES + P]
        g1t = auxf[:, AF_G1:AF_G1 + NT]
        g2t = auxf[:, AF_G2:AF_G2 + NT]
        b1t = auxb[:, AB_B1:AB_B1 + NT]
        b2t = auxb[:, AB_B2:AB_B2 + NT]
        brow1 = auxf[0:1, AF_BR1:AF_BR1 + 256]
        brow2 = auxf[0:1, AF_BR2:AF_BR2 + NEXP]

        # expert pool streams in half-b chunks: [128, 4096] = 4 g-groups.
        def stream_half(tensor, b, half):
            t = st2.tile([P, 4096], DT, tag="strm", name="strm")
            srcap = bass.AP(tensor, (b * GLOC + half * 4) * P * 1024,
                            [[1024, P], [P * 1024, 4], [1, 1024]])
            nc.sync.dma_start(out=t[:], in_=srcap)
            return t

        cn1 = [[stream_half(I["cnb"].tensor, b, h) for h in range(2)]
               for b in range(4)]
        pl1 = [[stream_half(I["plb"].tensor, b, h) for h in range(2)]
               for b in range(4)]
        wota2 = []
        for h in range(2):
            t = st2.tile([P, 4096], DT, tag="strm", name="strm")
            nc.sync.dma_start(out=t[:], in_=I["wot"][:, h * 4096:(h + 1) * 4096])
            wota2.append(t)
        cn2 = [[stream_half(I["cnb"].tensor, b, h) for h in range(2)]
               for b in range(4)]

        def copy_ps(out_ap, in_ap, k):
            if k % 2 == 0:
                nc.vector.tensor_copy(out=out_ap, in_=in_ap)
            else:
                nc.scalar.activation(out=out_ap, in_=in_ap, func=ACT.Identity)

        def layernorm_tile(x_ap, pool, tag):
            # var = E[x^2] - mean^2; one Square pass on raw x overlaps the
            # DVE sum; final pass folds (x - mean) * rstd into one activation.
            stats = sp.tile([P, 6], F32, tag="ln_stats", name="ln_stats")
            mean = stats[:, 0:1]; ex2 = stats[:, 1:2]; rstd = stats[:, 2:3]
            nmr = stats[:, 3:4]; var = stats[:, 4:5]; m2 = stats[:, 5:6]
            sq = sp.tile([P, D], F32, tag="ln_sq", name="ln_sq", bufs=1)
            nc.scalar.activation(out=sq[:], in_=x_ap, func=ACT.Square,
                                 accum_out=ex2)
            nc.vector.tensor_reduce(out=mean, in_=x_ap, axis=AX, op=ALU.add)
            nc.vector.tensor_scalar(out=mean, in0=mean, scalar1=1.0 / D,
                                    scalar2=None, op0=ALU.mult)
            nc.vector.tensor_tensor(out=m2, in0=mean, in1=mean, op=ALU.mult)
            nc.vector.tensor_scalar(out=var, in0=ex2, scalar1=1.0 / D,
                                    scalar2=1e-5, op0=ALU.mult, op1=ALU.add)
            nc.vector.tensor_tensor(out=var, in0=var, in1=m2, op=ALU.subtract)
            nc.scalar.sqrt(rstd, var)
            nc.vector.reciprocal(rstd, rstd)
            nc.vector.tensor_tensor(out=nmr, in0=mean, in1=rstd, op=ALU.mult)
            nc.vector.tensor_scalar(out=nmr, in0=nmr, scalar1=-1.0,
                                    scalar2=None, op0=ALU.mult)
            out = pool.tile([P, D], DT, tag=tag)
            nc.scalar.activation(out=out[:], in_=x_ap, func=ACT.Identity,
                                 scale=rstd, bias=nmr)
            return out

        def softmax_pool(psum_ap, out_ap, nblk, blk, imp_col, pool_out,
                         first, last):
            # exp (no max-sub; scores are O(1)) with per-block accum, then
            # pool with 1/Z folded into the importance column.
            zs = sp.tile([P, 8], F32, tag="sm_zs", name="sm_zs")
            for bi in range(nblk):
                sl = slice(bi * blk, (bi + 1) * blk)
                nc.scalar.activation(out=out_ap[:, sl], in_=psum_ap[:, sl],
                                     func=ACT.Exp, accum_out=zs[:, bi:bi + 1])
            nc.vector.reciprocal(zs[:, 0:nblk], zs[:, 0:nblk])
            impz = sp.tile([P, 8], F32, tag="sm_iz", name="sm_iz")
            nc.vector.tensor_scalar(out=impz[:, 0:nblk], in0=zs[:, 0:nblk],
                                    scalar1=imp_col, scalar2=None,
                                    op0=ALU.mult)
            for bi in range(nblk):
                sl = slice(bi * blk, (bi + 1) * blk)
                nc.tensor.matmul(out=pool_out[:, sl],
                                 lhsT=impz[:, bi:bi + 1], rhs=out_ap[:, sl],
                                 start=first, stop=last)

        def group_cols(wcol_ap, ncols):
            """wcol [32, ncols] f32 -> wk [128, GLOC*ncols]:
            wk[p, ncols*g + c] = wcol[4g + p%4, c]."""
            rhsB = sp.tile([NLOC, GLOC * ncols], F32, tag="rhsB", name="rhsB")
            for pi in range(ncols):
                nc.vector.tensor_scalar(
                    out=rhsB[:, pi:GLOC * ncols:ncols], in0=B8,
                    scalar1=wcol_ap[:, pi:pi + 1], scalar2=None, op0=ALU.mult)
            wkp = pst.tile([P, GLOC * ncols], F32, tag="tpp", name="wkp")
            nc.tensor.matmul(out=wkp[:], lhsT=A4, rhs=rhsB[:],
                             start=True, stop=True)
            wk = sp.tile([P, GLOC * ncols], F32, tag="wkall", name="wkall")
            nc.vector.tensor_copy(out=wk[:], in_=wkp[:])
            return wk

        def combine_cn(wcol_ap, chunks, out_f32):
            """out_f32 [128, 1024] f32 partial combine of local experts.
            chunks[b][p, 1024g+128t+r] = CN[e(g,p), 128t+32b+p//4, r]."""
            wk = group_cols(wcol_ap, 1)
            lhs = []
            for g in range(GLOC):
                lg = sp.tile([P, NLOC], DT, tag=f"clh{g}", name=f"clh{g}",
                             bufs=1)
                nc.vector.tensor_scalar(out=lg[:], in0=bmS,
                                        scalar1=wk[:, g:g + 1],
                                        scalar2=None, op0=ALU.mult)
                lhs.append(lg)
            for b in range(4):
                acc = psa.tile([NLOC, 1024], F32, tag="acc", name="cacc")
                for hh in range(2):
                    for g in range(GLOC):
                        gh, gl = g // 4, g % 4
                        nc.tensor.matmul(
                            out=acc[:, hh * 512:(hh + 1) * 512],
                            lhsT=lhs[g][:],
                            rhs=chunks[b][gh][:, gl * 1024 + hh * 512:
                                              gl * 1024 + (hh + 1) * 512],
                            start=(g == 0), stop=(g == GLOC - 1))
                copy_ps(out_f32[32 * b:32 * b + 32, :], acc[:], b)

        def pair_allreduce(sb_f32, ncol):
            """AllReduce sb_f32 [128, ncol] within batch pairs (in place)."""
            if not use_cc:
                return
            cc_in = dr.tile([P, ncol], F32)
            cc_out = dr.tile([P, ncol], F32)
            nc.gpsimd.dma_start(out=cc_in[:], in_=sb_f32[:])
            nc.gpsimd.collective_compute(
                "AllReduce", ALU.add,
                replica_groups=[[0, 1], [2, 3], [4, 5], [6, 7]],
                ins=[cc_in.opt()], outs=[cc_out.opt()])
            nc.gpsimd.dma_start(out=sb_f32[:], in_=cc_out[:])

        # ---------- LN1 + transposes ----------
        ctx4 = contextlib.ExitStack()
        p4 = ctx4.enter_context(tc.tile_pool(name="ph4", bufs=1))
        with tc.tile_pool(name="ph0", bufs=1) as p0:
            nxT = [p0.tile([P, S], DT, tag=f"nxT{t}", name=f"nxT{t}")
                   for t in range(DT_T)]
            for i in range(NT):
                nx_i = layernorm_tile(xa[:, i * 1024:(i + 1) * 1024], sp, "nx")
                for t in range(DT_T):
                    tp = pst.tile([P, P], DT, tag="tpp", name="tpp")
                    nc.tensor.transpose(out=tp[:],
                                        in_=nx_i[:, t * P:(t + 1) * P],
                                        identity=idn)
                    copy_ps(nxT[t][:, i * P:(i + 1) * P], tp[:], t)

            # ---------- routers (c,q,k,v) ----------
            wpool_ps = psv.tile([1, 4 * NEXP], F32, tag="pvacc", name="pvacc")
            for i in range(NT):
                pr_ps = psa.tile([P, 4 * NEXP], F32, tag="acc", name="acc")
                for t in range(DT_T):
                    nc.tensor.matmul(out=pr_ps[:],
                                     lhsT=nxT[t][:, i * P:(i + 1) * P],
                                     rhs=wct[:, 320 * t:320 * t + 256],
                                     start=(t == 0), stop=False)
                nc.tensor.matmul(out=pr_ps[:], lhsT=ones128, rhs=brow1,
                                 start=False, stop=True)
                pref = sp.tile([P, 4 * NEXP], F32, tag="pref", name="pref")
                softmax_pool(pr_ps[:], pref[:], 4, NEXP, impa[:, i:i + 1],
                             wpool_ps, first=(i == 0), last=(i == NT - 1))

            wrow = pp.tile([1, 4 * NEXP], F32, tag="wrow", name="wrow")
            nc.vector.tensor_copy(out=wrow[:], in_=wpool_ps[:])
            for bi in range(4):
                sl = slice(bi * NEXP, (bi + 1) * NEXP)
                st = sp.tile([1, 1], F32, tag="wn_st", name="wn_st")
                nc.vector.tensor_reduce(out=st[:], in_=wrow[:, sl], axis=AX,
                                        op=ALU.add)
                nc.vector.tensor_scalar(out=st[:], in0=st[:], scalar1=1e-8,
                                        scalar2=None, op0=ALU.add)
                nc.vector.reciprocal(st[:], st[:])
                nc.vector.tensor_scalar(out=wrow[:, sl], in0=wrow[:, sl],
                                        scalar1=st[:], scalar2=None,
                                        op0=ALU.mult)
            wt0 = pst.tile([P, 1], F32, tag="tpp", name="wt0")
            nc.tensor.transpose(out=wt0[:], in_=wrow[:, 0:P], identity=one1)
            wt1 = pst.tile([P, 1], F32, tag="tpp", name="wt1")
            nc.tensor.transpose(out=wt1[:], in_=wrow[:, P:2 * P], identity=one1)
            wcolcq = pp.tile([P, 1], F32, tag="wcolcq", name="wcolcq")
            nc.vector.tensor_copy(out=wcolcq[:], in_=wt0[:])
            wcolkv = pp.tile([P, 1], F32, tag="wcolkv", name="wcolkv")
            nc.vector.tensor_copy(out=wcolkv[:], in_=wt1[:])
            wcols3 = pp.tile([NLOC, 3], F32, tag="wcols3", name="wcols3")
            nc.vector.tensor_copy(out=wcols3[:, 0:1],
                                  in_=wcolcq[NEXP:NEXP + NLOC, :])
            nc.vector.tensor_copy(out=wcols3[:, 1:2], in_=wcolkv[0:NLOC, :])
            nc.vector.tensor_copy(out=wcols3[:, 2:3],
                                  in_=wcolkv[NEXP:NEXP + NLOC, :])

            # ---------- sc combine (+pair AllReduce) ----------
            e3f = p0.tile([P, 3072], F32, tag="e3f", name="e3f")
            scf = e3f[:, 0:1024]
            combine_cn(wcolcq[0:NLOC, 0:1], cn1, scf)
            pair_allreduce(scf, 1024)
            sc_b = p0.tile([P, 1024], DT, tag="sc_b", name="sc_b")
            for t in range(DT_T):
                nc.vector.tensor_scalar(out=sc_b[:, t * P:(t + 1) * P],
                                        in0=scf[:, t * P:(t + 1) * P],
                                        scalar1=g1t[:, t:t + 1], scalar2=None,
                                        op0=ALU.mult)

            # ---------- e3 combine ----------
            w3 = group_cols(wcols3[:], 3)  # [128, 24]
            lhs3 = []
            for g in range(GLOC):
                lg = p0.tile([P, 96], DT, tag=f"e3lh{g}", name=f"e3lh{g}")
                for pl_i in range(3):
                    nc.vector.tensor_scalar(
                        out=lg[:, 32 * pl_i:32 * (pl_i + 1)], in0=bmS,
                        scalar1=w3[:, 3 * g + pl_i:3 * g + pl_i + 1],
                        scalar2=None, op0=ALU.mult)
                lhs3.append(lg)
            for b in range(4):
                acc = psa.tile([96, 1024], F32, tag="acc", name="eacc")
                for hh in range(2):
                    for g in range(GLOC):
                        gh, gl = g // 4, g % 4
                        nc.tensor.matmul(
                            out=acc[:, hh * 512:(hh + 1) * 512],
                            lhsT=lhs3[g][:],
                            rhs=pl1[b][gh][:, gl * 1024 + hh * 512:
                                           gl * 1024 + (hh + 1) * 512],
                            start=(g == 0), stop=(g == GLOC - 1))
                for pl_i in range(3):
                    copy_ps(e3f[32 * b:32 * b + 32,
                                1024 * pl_i:1024 * (pl_i + 1)],
                            acc[32 * pl_i:32 * pl_i + 32, :], b + pl_i)
            pair_allreduce(e3f, 3072)
            e3 = p0.tile([P, 3072], DT, tag="e3", name="e3")
            nc.vector.tensor_copy(out=e3[:, 0:1024], in_=e3f[:, 0:1024])
            nc.scalar.activation(out=e3[:, 1024:2048], in_=e3f[:, 1024:2048],
                                 func=ACT.Identity)
            nc.gpsimd.tensor_copy(out=e3[:, 2048:3072], in_=e3f[:, 2048:3072])

            # ---------- hT[r, q] = sum_d sc[d, r] g1[d] nx[q, d] + (b1 @ sc g1)[r]
            bsc_ps = pst.tile([1, P], F32, tag="tpp", name="bscp")
            for t in range(DT_T):
                nc.tensor.matmul(out=bsc_ps[:], lhsT=b1t[:, t:t + 1],
                                 rhs=sc_b[:, t * P:(t + 1) * P],
                                 start=(t == 0), stop=(t == DT_T - 1))
            bsc_row = sp.tile([1, P], F32, tag="bscr", name="bscr")
            nc.vector.tensor_copy(out=bsc_row[:], in_=bsc_ps[:])
            bsc_t = pst.tile([P, 1], F32, tag="tpp", name="bsct")
            nc.tensor.transpose(out=bsc_t[:], in_=bsc_row[:], identity=one1)
            bsc = sp.tile([P, 1], F32, tag="bsc", name="bsc")
            nc.vector.tensor_copy(out=bsc[:], in_=bsc_t[:])
            hT = p0.tile([P, S], DT, tag="hT")
            for j in range(2):
                hp = psa.tile([P, 512], F32, tag="acc", name="hacc")
                for t in range(DT_T):
                    nc.tensor.matmul(out=hp[:],
                                     lhsT=sc_b[:, t * P:(t + 1) * P],
                                     rhs=nxT[t][:, j * 512:(j + 1) * 512],
                                     start=(t == 0), stop=(t == DT_T - 1))
                nc.scalar.activation(out=hT[:, j * 512:(j + 1) * 512],
                                     in_=hp[:], func=ACT.Identity, bias=bsc)

            # ---------- K, Q, V ----------
            SCALE_Q = 1.0 / float(np.sqrt(DH))
            kT = [p4.tile([P, S], DT, tag=f"kT{t}", name=f"kT{t}")
                  for t in range(DT_T)]
            qT = [p4.tile([P, SQ], DT, tag=f"qT{t}", name=f"qT{t}")
                  for t in range(DT_T)]
            vext = [p4.tile([P, H * (DH + 1)], DT, tag=f"vx{i}", name=f"vx{i}")
                    for i in range(NT)]
            for t in range(DT_T):
                kp = psa.tile([P, S], F32, tag="acc", name="acc")
                for j in range(2):
                    nc.tensor.matmul(out=kp[:, j * 512:(j + 1) * 512],
                                     lhsT=e3[:, 1024 + t * P:1024 + t * P + P],
                                     rhs=hT[:, j * 512:(j + 1) * 512],
                                     start=True, stop=True)
                nc.scalar.activation(out=kT[t][:], in_=kp[:], func=ACT.Identity)
                qp = psv.tile([P, SQ], F32, tag="pvacc", name="qacc")
                nc.tensor.matmul(out=qp[:], lhsT=e3[:, t * P:t * P + P],
                                 rhs=hT[:, 0:SQ], start=True, stop=True)
                nc.vector.tensor_scalar(out=qT[t][:], in0=qp[:],
                                        scalar1=SCALE_Q, scalar2=None,
                                        op0=ALU.mult)
            for i in range(NT):
                vp = psa.tile([P, D], F32, tag="acc", name="acc")
                for j in range(2):
                    nc.tensor.matmul(
                        out=vp[:, j * 512:(j + 1) * 512],
                        lhsT=hT[:, i * P:(i + 1) * P],
                        rhs=e3[:, 2048 + j * 512:2048 + (j + 1) * 512],
                        start=True, stop=True)
                vv = vext[i][:].rearrange("p (hh c) -> p hh c", c=DH + 1)
                nc.vector.tensor_copy(
                    out=vv[:, :, 0:DH],
                    in_=vp[:].rearrange("p (hh c) -> p hh c", c=DH))
                nc.gpsimd.memset(vv[:, :, DH:DH + 1], 1.0)
        # ph0 (nxT, scf, e3f, lhs3) released

        # ---------- attention ----------
        # q-slot s covers local q-tile s; key positions {0..s} u {4..7}.
        # position j==s gets the constant tri mask via PE; positions 4..7 get
        # the per-core bias column (0 or -1e9) folded into the exp.
        attnT = [p4.tile([P, SQ], DT, tag=f"at{t}", name=f"at{t}")
                 for t in range(DT_T)]
        for hd in range(H):
            t4 = hd // 2
            hs = (hd % 2) * DH
            po = psv.tile([DH + 1, SQ], F32, tag="pvacc", name="poacc")
            for j in range(NT):
                qlo = j * P if j < QT else 0
                w = SQ - qlo
                sps = psa.tile([P, SQ], F32, tag="acc", name="sacc")
                if j < QT:
                    nc.tensor.matmul(out=sps[:, 0:P],
                                     lhsT=kT[t4][hs:hs + DH, j * P:(j + 1) * P],
                                     rhs=qT[t4][hs:hs + DH, qlo:qlo + P],
                                     start=True, stop=False)
                    if w > P:
                        nc.tensor.matmul(out=sps[:, P:w],
                                         lhsT=kT[t4][hs:hs + DH,
                                                     j * P:(j + 1) * P],
                                         rhs=qT[t4][hs:hs + DH, qlo + P:SQ],
                                         start=True, stop=True)
                else:
                    nc.tensor.matmul(out=sps[:, 0:w],
                                     lhsT=kT[t4][hs:hs + DH, j * P:(j + 1) * P],
                                     rhs=qT[t4][hs:hs + DH, qlo:SQ],
                                     start=True, stop=True)
                if j < QT:
                    nc.tensor.matmul(out=sps[:, 0:P], lhsT=idn,
                                     rhs=trix[:, 0:P], start=False, stop=True)
                pt = sp.tile([P, SQ], DT, tag="p_tile", name="p_tile", bufs=3)
                if j < QT:
                    nc.scalar.activation(out=pt[:, 0:w], in_=sps[:, 0:w],
                                         func=ACT.Exp)
                else:
                    nc.scalar.activation(out=pt[:, 0:w], in_=sps[:, 0:w],
                                         func=ACT.Exp, bias=negc)
                nc.tensor.matmul(
                    out=po[:, qlo:SQ],
                    lhsT=vext[j][:, hd * (DH + 1):(hd + 1) * (DH + 1)],
                    rhs=pt[:, 0:w], start=(j == 0), stop=(j == NT - 1))
            rec = sp.tile([1, SQ], F32, tag="rec", name="rec")
            nc.vector.reciprocal(rec[:], po[DH:DH + 1, :])
            recB = sp.tile([DH, SQ], F32, tag="recB", name="recB")
            nc.gpsimd.partition_broadcast(recB[:], rec[:])
            nc.vector.tensor_tensor(out=attnT[t4][hs:hs + DH, :],
                                    in0=po[0:DH, :], in1=recB[:], op=ALU.mult)

        # ---------- WO + residual (into xa) ----------
        for i in range(QT):
            wp = psa.tile([P, D], F32, tag="acc", name="acc")
            for j in range(2):
                for t in range(DT_T):
                    toff = 1024 * t + 512 * j
                    nc.tensor.matmul(
                        out=wp[:, j * 512:(j + 1) * 512],
                        lhsT=attnT[t][:, i * P:(i + 1) * P],
                        rhs=wota2[toff // 4096][:, toff % 4096:
                                                toff % 4096 + 512],
                        start=(t == 0), stop=(t == DT_T - 1))
            nc.vector.tensor_tensor(out=xa[:, i * 1024:(i + 1) * 1024],
                                    in0=wp[:], in1=xa[:, i * 1024:(i + 1) * 1024],
                                    op=ALU.add)

        ctx4.close()

        # ---------- memory block ----------
        with tc.tile_pool(name="ph6", bufs=1) as p6:
            nx2T = [p6.tile([P, SQ], DT, tag=f"n2T{t}", name=f"n2T{t}")
                    for t in range(DT_T)]
            kkt = p6.tile([KR, NK], DT, tag="kkt", name="kkt")
            nc.sync.dma_start(out=kkt[:], in_=I["kkt"][:])
            for i in range(QT):
                nx2_i = layernorm_tile(xa[:, i * 1024:(i + 1) * 1024], sp,
                                       "nx2")
                for t in range(DT_T):
                    tp = pst.tile([P, P], DT, tag="tpp", name="tpp")
                    nc.tensor.transpose(out=tp[:],
                                        in_=nx2_i[:, t * P:(t + 1) * P],
                                        identity=idn)
                    copy_ps(nx2T[t][:, i * P:(i + 1) * P], tp[:], t)

            mwp_ps = psv.tile([1, NEXP], F32, tag="pvacc", name="pvacc")
            for i in range(QT):
                pr = psa.tile([P, NEXP], F32, tag="acc", name="acc")
                for t in range(DT_T):
                    nc.tensor.matmul(out=pr[:],
                                     lhsT=nx2T[t][:, i * P:(i + 1) * P],
                                     rhs=wct[:, 320 * t + 256:320 * t + 320],
                                     start=(t == 0), stop=False)
                nc.tensor.matmul(out=pr[:], lhsT=ones128, rhs=brow2,
                                 start=False, stop=True)
                prefm = sp.tile([P, NEXP], F32, tag="prefm", name="prefm")
                softmax_pool(pr[:], prefm[:], 1, NEXP, impa[:, i:i + 1],
                             mwp_ps, first=(i == 0), last=(i == QT - 1))

            mwrow = p6.tile([1, NEXP], F32, tag="mwrow", name="mwrow")
            if use_cc:
                mwr = sp.tile([1, NEXP], F32, tag="mwr", name="mwr")
                nc.vector.tensor_copy(out=mwr[:], in_=mwp_ps[:])
                ccp = psa.tile([B, NEXP], F32, tag="acc", name="acc")
                nc.tensor.matmul(out=ccp[:], lhsT=bselr, rhs=mwr[:],
                                 start=True, stop=True)
                cc_sb = sp.tile([B, NEXP], F32, tag="cc_sb", name="cc_sb")
                nc.vector.tensor_copy(out=cc_sb[:], in_=ccp[:])
                cc_in = dr.tile([B, NEXP], F32)
                cc_out = dr.tile([B, NEXP], F32)
                nc.gpsimd.dma_start(out=cc_in[:], in_=cc_sb[:])
                nc.gpsimd.collective_compute(
                    "AllReduce", ALU.add,
                    replica_groups=[list(range(N_CORES))],
                    ins=[cc_in.opt()], outs=[cc_out.opt()])
                cc_res = sp.tile([B, NEXP], F32, tag="cc_res", name="cc_res")
                nc.gpsimd.dma_start(out=cc_res[:], in_=cc_out[:])
                mwf = psa.tile([1, NEXP], F32, tag="acc", name="acc")
                nc.tensor.matmul(out=mwf[:], lhsT=bselc, rhs=cc_res[:],
                                 start=True, stop=True)
                nc.vector.tensor_copy(out=mwrow[:], in_=mwf[:])
            else:
                nc.vector.tensor_copy(out=mwrow[:], in_=mwp_ps[:])
            st = sp.tile([1, 1], F32, tag="wn_st", name="wn_st")
            nc.vector.tensor_reduce(out=st[:], in_=mwrow[:], axis=AX,
                                    op=ALU.add)
            nc.vector.tensor_scalar(out=st[:], in0=st[:], scalar1=1e-8,
                                    scalar2=None, op0=ALU.add)
            nc.vector.reciprocal(st[:], st[:])
            nc.vector.tensor_scalar(out=mwrow[:], in0=mwrow[:], scalar1=st[:],
                                    scalar2=None, op0=ALU.mult)
            mwrow_cp = sp.tile([1, NEXP], F32, tag="mwr2", name="mwr2")
            nc.vector.tensor_copy(out=mwrow_cp[:], in_=mwrow[:])
            mwt = pst.tile([NEXP, 1], F32, tag="tpp", name="mwt")
            nc.tensor.transpose(out=mwt[:], in_=mwrow_cp[:], identity=one1)
            mwcol = p6.tile([NEXP, 1], F32, tag="mwcol", name="mwcol")
            nc.vector.tensor_copy(out=mwcol[:], in_=mwt[:])

            scmf = p6.tile([P, 1024], F32, tag="scmf", name="scmf")
            combine_cn(mwcol[0:NLOC, 0:1], cn2, scmf)
            pair_allreduce(scmf, 1024)
            scm_b = p6.tile([P, 1024], DT, tag="scm_b", name="scm_b")
            for t in range(DT_T):
                nc.vector.tensor_scalar(out=scm_b[:, t * P:(t + 1) * P],
                                        in0=scmf[:, t * P:(t + 1) * P],
                                        scalar1=g2t[:, t:t + 1], scalar2=None,
                                        op0=ALU.mult)

            # QmT [r, sq]
            qmp = psv.tile([P, SQ], F32, tag="pvacc", name="pvacc")
            for t in range(DT_T):
                nc.tensor.matmul(out=qmp[:], lhsT=scm_b[:, t * P:(t + 1) * P],
                                 rhs=nx2T[t][:], start=(t == 0),
                                 stop=(t == DT_T - 1))
            bscm_ps = pst.tile([1, P], F32, tag="tpp", name="bscmp")
            for t in range(DT_T):
                nc.tensor.matmul(out=bscm_ps[:], lhsT=b2t[:, t:t + 1],
                                 rhs=scm_b[:, t * P:(t + 1) * P],
                                 start=(t == 0), stop=(t == DT_T - 1))
            bscm_row = sp.tile([1, P], F32, tag="bscr", name="bscmr")
            nc.vector.tensor_scalar(out=bscm_row[:], in0=bscm_ps[:],
                                    scalar1=1.0 / float(np.sqrt(KR)),
                                    scalar2=None, op0=ALU.mult)
            bscm_t = pst.tile([P, 1], F32, tag="tpp", name="bscmt")
            nc.tensor.transpose(out=bscm_t[:], in_=bscm_row[:], identity=one1)
            bscm = sp.tile([P, 1], F32, tag="bsc", name="bscm")
            nc.vector.tensor_copy(out=bscm[:], in_=bscm_t[:])
            qmT = p6.tile([P, SQ], DT, tag="qmT")
            nc.scalar.activation(out=qmT[:], in_=qmp[:], func=ACT.Identity,
                                 scale=1.0 / float(np.sqrt(KR)), bias=bscm)

            idx_all = p6.tile([P, QT * TOPK], U32, tag="idx_all",
                              name="idx_all")
            w8_all = p6.tile([P, QT * TOPK], F32, tag="w8_all", name="w8_all")
            for i in range(QT):
                ks = p6.tile([P, NK], F32, tag="ks_sb", name="ks_sb")
                for j in range(NK // 512):
                    ksp = psa.tile([P, 512], F32, tag="acc", name="acc")
                    nc.tensor.matmul(out=ksp[:],
                                     lhsT=qmT[:, i * P:(i + 1) * P],
                                     rhs=kkt[:, j * 512:(j + 1) * 512],
                                     start=True, stop=True)
                    eng = nc.scalar if (j % 2 == 0) else nc.vector
                    if eng is nc.scalar:
                        nc.scalar.activation(out=ks[:, j * 512:(j + 1) * 512],
                                             in_=ksp[:], func=ACT.Identity)
                    else:
                        nc.vector.tensor_copy(out=ks[:, j * 512:(j + 1) * 512],
                                              in_=ksp[:])
                tv = sp.tile([P, TOPK], F32, tag="tv", name="tv")
                nc.vector.max_with_indices(
                    out_max=tv[:],
                    out_indices=idx_all[:, i * TOPK:(i + 1) * TOPK],
                    in_=ks[:])
                st8 = sp.tile([P, 2], F32, tag="st8", name="st8")
                nm = st8[:, 0:1]; se8 = st8[:, 1:2]
                nc.vector.tensor_scalar(out=nm, in0=tv[:, 0:1], scalar1=-1.0,
                                        scalar2=None, op0=ALU.mult)
                w8 = sp.tile([P, TOPK], F32, tag="w8", name="w8")
                nc.scalar.activation(out=w8[:], in_=tv[:], func=ACT.Exp,
                                     bias=nm, accum_out=se8)
                nc.vector.reciprocal(se8, se8)
                nc.vector.tensor_scalar(out=w8_all[:, i * TOPK:(i + 1) * TOPK],
                                        in0=w8[:], scalar1=se8, scalar2=None,
                                        op0=ALU.mult)

            for i in range(QT):
                acc = p6.tile([P, D], F32, tag="mem_acc", name="mem_acc", bufs=2)
                gt8 = p6.tile([P, TOPK * D], DT, tag="gath8", name="gath8", bufs=2)
                nc.gpsimd.indirect_dma_start(
                    out=gt8[:], out_offset=None, in_=I["kv"][:],
                    in_offset=bass.IndirectOffsetOnAxis(
                        ap=idx_all[:, i * TOPK:(i + 1) * TOPK], axis=0))
                accB = p6.tile([P, D], F32, tag="mem_accB", name="mem_accB",
                               bufs=2)
                tmp = sp.tile([P, D], F32, tag="gtmp", name="gtmp")
                for k in range(TOPK):
                    g = i * TOPK + k
                    if k % 2 == 0:
                        prev = (xa[:, i * 1024:(i + 1) * 1024] if k == 0
                                else acc[:])
                        nc.vector.scalar_tensor_tensor(
                            out=acc[:], in0=gt8[:, k * D:(k + 1) * D],
                            scalar=w8_all[:, g:g + 1],
                            in1=prev, op0=ALU.mult, op1=ALU.add)
                    else:
                        dst = accB[:] if k == 1 else tmp[:]
                        nc.scalar.activation(out=dst,
                                             in_=gt8[:, k * D:(k + 1) * D],
                                             func=ACT.Identity,
                                             scale=w8_all[:, g:g + 1])
                        if k > 1:
                            nc.gpsimd.tensor_tensor(out=accB[:], in0=tmp[:],
                                                    in1=accB[:], op=ALU.add)
                nc.vector.tensor_tensor(out=acc[:], in0=acc[:], in1=accB[:],
                                        op=ALU.add)
                nc.sync.dma_start(out=o[i * P:(i + 1) * P, :], in_=acc[:])


# ---------------- PJRT SPMD runner (persistent jit) ----------------

class SpmdRunner:
    def __init__(self, nc, n_cores):
        import jax
        from jax.sharding import Mesh, PartitionSpec
        from jax.experimental.shard_map import shard_map
        from concourse import bass2jax
        bass2jax.install_neuronx_cc_hook()
        self.jax = jax
        self.nc = nc
        self.n_cores = n_cores
        partition_name = (nc.partition_id_tensor.name
                          if nc.partition_id_tensor else None)
        in_names, out_names, out_avals, zero_outs = [], [], [], []
        for alloc in nc.m.functions[0].allocations:
            if not isinstance(alloc, mybir.MemoryLocationSet):
                continue
            name = alloc.memorylocations[0].name
            if alloc.kind == "ExternalInput":
                if name != partition_name:
                    in_names.append(name)
            elif alloc.kind == "ExternalOutput":
                shape = tuple(alloc.tensor_shape)
                dtype = mybir.dt.np(alloc.dtype)
                out_names.append(name)
                out_avals.append(jax.core.ShapedArray(shape, dtype))
                zero_outs.append(np.zeros(shape, dtype))
        self.n_params = len(in_names)
        self.in_names = list(in_names)
        self.out_names = out_names
        self.out_avals = out_avals
        self.zero_outs = zero_outs
        all_in = in_names + out_names + ([partition_name] if partition_name
                                         else [])

        def _body(*args):
            operands = list(args)
            if partition_name is not None:
                operands.append(bass2jax.partition_id_tensor())
            outs = bass2jax._bass_exec_p.bind(
                *operands, out_avals=tuple(out_avals), in_names=tuple(all_in),
                out_names=tuple(out_names), lowering_input_output_aliases=(),
                sim_require_finite=True, sim_require_nnan=True, nc=nc)
            return tuple(outs)

        devices = jax.devices()[:n_cores]
        self.mesh = Mesh(np.asarray(devices), ("core",))
        nspec = self.n_params + len(out_names)
        self.sharded = jax.jit(
            shard_map(_body, mesh=self.mesh,
                      in_specs=(PartitionSpec("core"),) * nspec,
                      out_specs=(PartitionSpec("core"),) * len(out_names),
                      check_rep=False),
            keep_unused=True)

    def concat_inputs(self, in_maps):
        per_core = [[np.asarray(m[n]) for n in self.in_names] for m in in_maps]
        cat = [np.concatenate([per_core[c][i] for c in range(self.n_cores)],
                              axis=0) for i in range(self.n_params)]
        cat += [np.zeros((self.n_cores * z.shape[0], *z.shape[1:]), z.dtype)
                for z in self.zero_outs]
        return cat

    def run(self, in_maps):
        out_arrs = self.sharded(*self.concat_inputs(in_maps))
        self.jax.block_until_ready(out_arrs)
        return [
            {n: np.asarray(out_arrs[i]).reshape(
                self.n_cores, *self.out_avals[i].shape)[c]
             for i, n in enumerate(self.out_names)}
            for c in range(self.n_cores)
        ]


# ---------------- host side ----------------

_RUNNER = None


def _make_inputs(x, importance, mask, compress_neurons, expand_pool,
                 knowledge_K, knowledge_V, Wc, WQ, WK, WV, Wm, WO,
                 g1, b1, g2, b2):
    ndt = np_bdt()
    f = lambda a: np.asarray(a, np.float32)
    cn = f(compress_neurons)
    pl = f(expand_pool)
    wstack = np.concatenate([f(Wc), f(WQ), f(WK), f(WV), f(Wm)], axis=0)
    wot = np.ascontiguousarray(f(WO).T)  # [D, D] = WO.T
    wotr = np.empty((P, NT * 1024), np.float32)
    for t in range(NT):
        wotr[:, 1024 * t:1024 * (t + 1)] = wot[128 * t:128 * (t + 1), :]
    kkt = np.ascontiguousarray(f(knowledge_K).T).astype(ndt)
    kv = f(knowledge_V).astype(ndt)

    # aux (core-independent parts)
    auxb = np.zeros((P, AB_W), np.float32)
    auxb[:, AB_IDN:AB_IDN + P] = np.eye(P)
    auxb[:, AB_BMS:AB_BMS + 32] = (
        (np.arange(P)[:, None] // 4) == np.arange(32)[None, :])
    ktri = np.arange(P)
    auxb[:, AB_TRI:AB_TRI + P] = np.where(
        ktri[None, :] >= ktri[:, None], 0.0, NEG)  # tri[k, q]; rest zeros
    auxb[:, AB_B1:AB_B1 + NT] = f(b1).reshape(NT, P).T
    auxb[:, AB_B2:AB_B2 + NT] = f(b2).reshape(NT, P).T
    auxb = auxb.astype(ndt)

    x = f(x); importance = f(importance)
    in_maps = []
    for c in range(N_CORES):
        b, hf = c // 2, c % 2
        qr = np.arange(hf * SQ, hf * SQ + SQ)
        rest = np.arange((1 - hf) * SQ, (1 - hf) * SQ + SQ)
        perm = np.concatenate([qr, rest])
        eperm = (np.arange(NEXP) + NLOC * hf) % NEXP  # local experts first

        m = {}
        m["x"] = np.ascontiguousarray(x[b][perm])
        impc = importance[b][perm].reshape(NT, P).T  # [p, i]
        m["imp"] = np.ascontiguousarray(impc).astype(np.float32)

        # wct: [128, 8*320]; block order [Wc|WQ|WK|WV|Wm], experts permuted.
        # g1 folded into router1 blocks, g2 into the Wm block (LN emits the
        # plain normalized value).
        wp_ = wstack.reshape(5, NEXP, D)[:, eperm, :].reshape(5 * NEXP, D)
        wp_ = wp_ * np.concatenate([np.tile(f(g1).ravel()[None, :], (4 * NEXP, 1)),
                                    np.tile(f(g2).ravel()[None, :], (NEXP, 1))])
        wctT = wp_.T  # [D, 320]
        wcth = np.empty((P, NT * 320), np.float32)
        for t in range(NT):
            wcth[:, 320 * t:320 * (t + 1)] = wctT[128 * t:128 * (t + 1), :]
        m["wct"] = np.ascontiguousarray(wcth).astype(ndt)
        m["wot"] = wotr.astype(ndt)
        m["kkt"] = kkt
        m["kv"] = kv

        # cnb[b4, g, p, 128t + r] = cn[e(g,p), 128t + 32*b4 + p//4, r]
        loc = eperm[:NLOC]
        cl = cn[loc]                      # [32, D, R]
        clr = cl.reshape(NLOC, 8, 128, R) # [n, t, dsub, r]
        # dsub = 32*b4 + p//4 ; partition p = 4*(p//4) + n%4
        cnb = np.empty((4, GLOC, P, 1024), np.float32)
        for b4 in range(4):
            blk = clr[:, :, 32 * b4:32 * (b4 + 1), :]   # [n, t, 32, r]
            for g in range(GLOC):
                for e in range(4):
                    n = 4 * g + e
                    # partition p = 4*m + e (m = dsub idx), free = 128t + r
                    cnb[b4, g, e::4, :] = blk[n].transpose(1, 0, 2).reshape(
                        32, 8 * 128)
        m["cnb"] = np.ascontiguousarray(cnb).astype(ndt)

        pll = pl[loc]                     # [32, R, D]
        plb = np.empty((4, GLOC, P, 1024), np.float32)
        for b4 in range(4):
            blk = pll[:, 32 * b4:32 * (b4 + 1), :]      # [n, 32, D]
            for g in range(GLOC):
                for e in range(4):
                    plb[b4, g, e::4, :] = blk[4 * g + e]
        m["plb"] = np.ascontiguousarray(plb).astype(ndt)

        auxf = np.zeros((P, AF_W), np.float32)
        auxf[:NLOC, AF_A4:AF_A4 + P] = (
            (np.arange(NLOC)[:, None] % 4) == (np.arange(P)[None, :] % 4))
        auxf[:NLOC, AF_B8:AF_B8 + GLOC] = (
            (np.arange(NLOC)[:, None] // 4) == np.arange(GLOC)[None, :])
        onehot = np.zeros(B, np.float32); onehot[b] = 1.0
        auxf[0:1, AF_BSELR:AF_BSELR + B] = onehot[None, :]
        auxf[0:B, AF_BSELC:AF_BSELC + 1] = onehot[:, None]
        auxf[:, AF_NEGC] = NEG if hf == 0 else 0.0
        auxf[0, AF_ONE] = 1.0
        auxf[0, AF_ONES:AF_ONES + P] = 1.0
        auxf[:, AF_G1:AF_G1 + NT] = f(g1).reshape(NT, P).T
        auxf[:, AF_G2:AF_G2 + NT] = f(g2).reshape(NT, P).T
        wr1 = wstack.reshape(5, NEXP, D)[:4, eperm, :]
        auxf[0, AF_BR1:AF_BR1 + 256] = (
            wr1.reshape(256, D) @ f(b1).ravel())
        wr2 = wstack.reshape(5, NEXP, D)[4, eperm, :]
        auxf[0, AF_BR2:AF_BR2 + NEXP] = wr2 @ f(b2).ravel()
        m["auxf"] = auxf
        m["auxb"] = auxb
        in_maps.append(m)
    return in_maps


def _get_runner():
    global _RUNNER
    if _RUNNER is None:
        nc = build_nc(use_cc=True)
        _RUNNER = SpmdRunner(nc, N_CORES)
    return _RUNNER


def kernel(**inputs):
    r = _get_runner()
    in_maps = _make_inputs(**inputs)
    res = r.run(in_maps)
    out = np.empty((B, S, D), np.float32)
    for c in range(N_CORES):
        b, hf = c // 2, c % 2
        out[b, hf * SQ:(hf + 1) * SQ] = res[c]["o"]
    return out


# revision 18
# speedup vs baseline: 1.1428x; 1.0397x over previous
"""DAWN block (moe_routing) Trainium2 kernel: 8-core SPMD, v2.

Sharding: core c = (batch b=c//2, half h=c%2). Each core handles one batch's
attention + memory block for half the queries (rows permuted so local queries
come first). Expert pools (compress_neurons / expand_pool) are pair-sharded:
each core streams only 32 of 64 experts and partial combines are AllReduced
within the pair. Causal structure: per-q-slot key-position lists + a constant
triangular mask tile + per-core bias column (full-mask blocks), so ~19% of
score/AV work is skipped and no per-key mask tensor is needed.

DMA strategy: everything is host-relaid-out so the device does few, large,
contiguous DMAs (the v1 kernel's 456 tiny combine DMAs were the bottleneck:
each DMA costs ~0.6us queue dispatch + 625ns shared HWDGE serial time).
"""
import numpy as np
import ml_dtypes

import concourse.bass as bass
import concourse.mybir as mybir
import concourse.tile as tile
from concourse import bacc

B, S, D = 4, 1024, 1024
H, DH = 16, 64
R = 128
NEXP = 64
NLOC = 32          # local experts per core (pair-sharded)
GLOC = NLOC // 4   # 8 stacked-expert groups
NK, KR = 4096, 128
TOPK = 8
N_CORES = 8
SQ = S // 2
P = 128
NT = S // P        # 8 seq tiles
QT = SQ // P       # 4 local q tiles
DT_T = D // P      # 8 d tiles

F32 = mybir.dt.float32
U32 = mybir.dt.uint32
NEG = -1.0e9
ALU = mybir.AluOpType
ACT = mybir.ActivationFunctionType
AX = None

# aux column maps
AF_A4 = 0          # [0:32, 0:128]
AF_B8 = 128        # [0:32, 128:136]
AF_BSELR = 136     # [0:1, 136:140]
AF_BSELC = 140     # [0:4, 140:141]
AF_NEGC = 141      # [0:128, 141:142]
AF_ONE = 142       # [0:1, 142:143]  value 1.0
AF_ONES = 144      # [0:1, 144:272] row of ones
AF_G1 = 272        # [0:128, 272:280] g1 tiled [p, t]
AF_G2 = 280        # [0:128, 280:288] g2 tiled
AF_BR1 = 304       # [0:1, 304:560] b1 @ [Wc|WQ|WK|WV].T (perm)
AF_BR2 = 560       # [0:1, 560:624] b2 @ Wm.T (perm)
AF_W = 624
AB_IDN = 0         # [0:128, 0:128]
AB_BMS = 128       # [0:128, 128:160]
AB_TRI = 160       # [0:128, 160:672] = [tri | zeros x3]
AB_B1 = 672        # [0:128, 672:680] b1 tiled [p, t]
AB_B2 = 680        # [0:128, 680:688] b2 tiled
AB_W = 688


def bdt():
    return mybir.dt.bfloat16


def np_bdt():
    return ml_dtypes.bfloat16


def build_nc(use_cc=True):
    global AX
    AX = mybir.AxisListType.X
    DT = bdt()
    nc = bacc.Bacc("TRN2", target_bir_lowering=False, debug=False,
                   num_devices=N_CORES)
    I = {}

    def inp(name, shape, dt):
        I[name] = nc.dram_tensor(name, shape, dt, kind="ExternalInput").ap()

    inp("x", [S, D], F32)              # row-permuted batch (local q first)
    inp("imp", [P, NT], F32)           # imp[p,i] = importance[perm[128i+p]]
    inp("cnb", [4, GLOC, P, 1024], DT) # local-expert compress pool, relaid
    inp("plb", [4, GLOC, P, 1024], DT) # local-expert expand pool, relaid
    inp("wct", [P, NT * 320], DT)      # [Wc|WQ|WK|WV|Wm].T tiled (expert-perm)
    inp("wot", [P, NT * 1024], DT)     # WO.T tiled
    inp("kkt", [KR, NK], DT)           # knowledge_K.T
    inp("kv", [NK, D], DT)             # knowledge_V
    inp("auxf", [P, AF_W], F32)
    inp("auxb", [P, AB_W], DT)
    o = nc.dram_tensor("o", [SQ, D], F32, kind="ExternalOutput").ap()

    with tile.TileContext(nc) as tc:
        _body(nc, tc, I, o, use_cc)
    nc.compile()
    return nc


def _body(nc, tc, I, o, use_cc):
    DT = bdt()
    import contextlib
    ctx = contextlib.ExitStack()
    with ctx:
        pp = ctx.enter_context(tc.tile_pool(name="pers", bufs=1))
        sp = ctx.enter_context(tc.tile_pool(name="stream", bufs=2))
        st2 = ctx.enter_context(tc.tile_pool(name="strm", bufs=3))
        pst = ctx.enter_context(tc.tile_pool(name="ps_t", bufs=2, space="PSUM"))
        psa = ctx.enter_context(tc.tile_pool(name="ps_a", bufs=2, space="PSUM"))
        psv = ctx.enter_context(tc.tile_pool(name="ps_v", bufs=2, space="PSUM"))
        dr = ctx.enter_context(tc.tile_pool(name="dram", bufs=1, space="DRAM"))

        # ---------- bulk loads ----------
        xa = pp.tile([P, NT * 1024], F32, tag="xa", name="xa")
        for hh in range(2):
            src = bass.AP(I["x"].tensor, hh * 4 * P * 1024,
                          [[1024, P], [P * 1024, 4], [1, 1024]])
            nc.sync.dma_start(out=xa[:, hh * 4096:(hh + 1) * 4096], in_=src)
        wct = pp.tile([P, NT * 320], DT, tag="wct", name="wct")
        nc.sync.dma_start(out=wct[:], in_=I["wct"][:])
        impa = pp.tile([P, NT], F32, tag="impa", name="impa")
        nc.sync.dma_start(out=impa[:], in_=I["imp"][:])
        auxf = pp.tile([P, AF_W], F32, tag="auxf", name="auxf")
        nc.sync.dma_start(out=auxf[:], in_=I["auxf"][:])
        auxb = pp.tile([P, AB_W], DT, tag="auxb", name="auxb")
        nc.sync.dma_start(out=auxb[:], in_=I["auxb"][:])

        idn = auxb[:, AB_IDN:AB_IDN + P]
        bmS = auxb[:, AB_BMS:AB_BMS + 32]
        trix = auxb[:, AB_TRI:AB_TRI + 4 * P]
        A4 = auxf[0:NLOC, AF_A4:AF_A4 + P]
        B8 = auxf[0:NLOC, AF_B8:AF_B8 + GLOC]
        bselr = auxf[0:1, AF_BSELR:AF_BSELR + B]
        bselc = auxf[0:B, AF_BSELC:AF_BSELC + 1]
        negc = auxf[:, AF_NEGC:AF_NEGC + 1]
        one1 = auxf[0:1, AF_ONE:AF_ONE + 1]
        ones128 = auxf[0:1, AF_ONES:AF_ONES + P]
        g1t = auxf[:, AF_G1:AF_G1 + NT]
        g2t = auxf[:, AF_G2:AF_G2 + NT]
        b1t = auxb[:, AB_B1:AB_B1 + NT]
        b2t = auxb[:, AB_B2:AB_B2 + NT]
        brow1 = auxf[0:1, AF_BR1:AF_BR1 + 256]
        brow2 = auxf[0:1, AF_BR2:AF_BR2 + NEXP]

        # expert pool streams in half-b chunks: [128, 4096] = 4 g-groups.
        def stream_half(tensor, b, half):
            t = st2.tile([P, 4096], DT, tag="strm", name="strm")
            srcap = bass.AP(tensor, (b * GLOC + half * 4) * P * 1024,
                            [[1024, P], [P * 1024, 4], [1, 1024]])
            nc.sync.dma_start(out=t[:], in_=srcap)
            return t

        cn1 = [[stream_half(I["cnb"].tensor, b, h) for h in range(2)]
               for b in range(4)]
        pl1 = [[stream_half(I["plb"].tensor, b, h) for h in range(2)]
               for b in range(4)]
        wota2 = []
        for h in range(2):
            t = st2.tile([P, 4096], DT, tag="strm", name="strm")
            nc.sync.dma_start(out=t[:], in_=I["wot"][:, h * 4096:(h + 1) * 4096])
            wota2.append(t)
        cn2 = [[stream_half(I["cnb"].tensor, b, h) for h in range(2)]
               for b in range(4)]

        def copy_ps(out_ap, in_ap, k):
            nc.vector.tensor_copy(out=out_ap, in_=in_ap)

        def layernorm_tile(x_ap, pool, tag):
            # bn_stats computes per-partition mean/var in one DVE pass per
            # 512-wide subgroup; the final Act pass folds (x - mean) * rstd.
            bst = sp.tile([P, 2, 6], F32, tag="ln_bst", name="ln_bst")
            xg = x_ap.rearrange("p (s f) -> p s f", f=512)
            nc.vector.bn_stats(out=bst[:, 0, :], in_=xg[:, 0, :])
            nc.vector.bn_stats(out=bst[:, 1, :], in_=xg[:, 1, :])
            stats = sp.tile([P, 4], F32, tag="ln_stats", name="ln_stats")
            mv = stats[:, 0:2]
            rstd = stats[:, 2:3]; nmr = stats[:, 3:4]
            nc.vector.bn_aggr(out=mv, in_=bst[:])
            nc.vector.tensor_scalar(out=rstd, in0=stats[:, 1:2], scalar1=1e-5,
                                    scalar2=None, op0=ALU.add)
            nc.scalar.sqrt(rstd, rstd)
            nc.vector.reciprocal(rstd, rstd)
            nc.vector.tensor_tensor(out=nmr, in0=stats[:, 0:1], in1=rstd,
                                    op=ALU.mult)
            nc.vector.tensor_scalar(out=nmr, in0=nmr, scalar1=-1.0,
                                    scalar2=None, op0=ALU.mult)
            out = pool.tile([P, D], DT, tag=tag)
            nc.scalar.activation(out=out[:], in_=x_ap, func=ACT.Identity,
                                 scale=rstd, bias=nmr)
            return out

        def softmax_pool(psum_ap, out_ap, nblk, blk, imp_col, pool_out,
                         first, last):
            # exp (no max-sub; scores are O(1)) with per-block accum, then
            # pool with 1/Z folded into the importance column.
            zs = sp.tile([P, 8], F32, tag="sm_zs", name="sm_zs")
            for bi in range(nblk):
                sl = slice(bi * blk, (bi + 1) * blk)
                nc.scalar.activation(out=out_ap[:, sl], in_=psum_ap[:, sl],
                                     func=ACT.Exp, accum_out=zs[:, bi:bi + 1])
            nc.vector.reciprocal(zs[:, 0:nblk], zs[:, 0:nblk])
            impz = sp.tile([P, 8], F32, tag="sm_iz", name="sm_iz")
            nc.vector.tensor_scalar(out=impz[:, 0:nblk], in0=zs[:, 0:nblk],
                                    scalar1=imp_col, scalar2=None,
                                    op0=ALU.mult)
            for bi in range(nblk):
                sl = slice(bi * blk, (bi + 1) * blk)
                nc.tensor.matmul(out=pool_out[:, sl],
                                 lhsT=impz[:, bi:bi + 1], rhs=out_ap[:, sl],
                                 start=first, stop=last)

        def group_cols(wcol_ap, ncols):
            """wcol [32, ncols] f32 -> wk [128, GLOC*ncols]:
            wk[p, ncols*g + c] = wcol[4g + p%4, c]."""
            rhsB = sp.tile([NLOC, GLOC * ncols], F32, tag="rhsB", name="rhsB")
            for pi in range(ncols):
                nc.vector.tensor_scalar(
                    out=rhsB[:, pi:GLOC * ncols:ncols], in0=B8,
                    scalar1=wcol_ap[:, pi:pi + 1], scalar2=None, op0=ALU.mult)
            wkp = pst.tile([P, GLOC * ncols], F32, tag="tpp", name="wkp")
            nc.tensor.matmul(out=wkp[:], lhsT=A4, rhs=rhsB[:],
                             start=True, stop=True)
            wk = sp.tile([P, GLOC * ncols], F32, tag="wkall", name="wkall")
            nc.vector.tensor_copy(out=wk[:], in_=wkp[:])
            return wk

        def combine_cn(wcol_ap, chunks, out_f32):
            """out_f32 [128, 1024] f32 partial combine of local experts.
            chunks[b][p, 1024g+128t+r] = CN[e(g,p), 128t+32b+p//4, r]."""
            wk = group_cols(wcol_ap, 1)
            lhs = []
            for g in range(GLOC):
                lg = sp.tile([P, NLOC], DT, tag=f"clh{g}", name=f"clh{g}",
                             bufs=1)
                nc.vector.tensor_scalar(out=lg[:], in0=bmS,
                                        scalar1=wk[:, g:g + 1],
                                        scalar2=None, op0=ALU.mult)
                lhs.append(lg)
            for b in range(4):
                acc = psa.tile([NLOC, 1024], F32, tag="acc", name="cacc")
                for hh in range(2):
                    for g in range(GLOC):
                        gh, gl = g // 4, g % 4
                        nc.tensor.matmul(
                            out=acc[:, hh * 512:(hh + 1) * 512],
                            lhsT=lhs[g][:],
                            rhs=chunks[b][gh][:, gl * 1024 + hh * 512:
                                              gl * 1024 + (hh + 1) * 512],
                            start=(g == 0), stop=(g == GLOC - 1))
                copy_ps(out_f32[32 * b:32 * b + 32, :], acc[:], b)

        def pair_allreduce(sb_f32, ncol):
            """AllReduce sb_f32 [128, ncol] within batch pairs (in place)."""
            if not use_cc:
                return
            cc_in = dr.tile([P, ncol], F32)
            cc_out = dr.tile([P, ncol], F32)
            nc.gpsimd.dma_start(out=cc_in[:], in_=sb_f32[:])
            nc.gpsimd.collective_compute(
                "AllReduce", ALU.add,
                replica_groups=[[0, 1], [2, 3], [4, 5], [6, 7]],
                ins=[cc_in.opt()], outs=[cc_out.opt()])
            nc.gpsimd.dma_start(out=sb_f32[:], in_=cc_out[:])

        # ---------- LN1 + transposes ----------
        ctx4 = contextlib.ExitStack()
        p4 = ctx4.enter_context(tc.tile_pool(name="ph4", bufs=1))
        with tc.tile_pool(name="ph0", bufs=1) as p0:
            nxT = [p0.tile([P, S], DT, tag=f"nxT{t}", name=f"nxT{t}")
                   for t in range(DT_T)]
            for i in range(NT):
                nx_i = layernorm_tile(xa[:, i * 1024:(i + 1) * 1024], sp, "nx")
                for t in range(DT_T):
                    tp = pst.tile([P, P], DT, tag="tpp", name="tpp")
                    nc.tensor.transpose(out=tp[:],
                                        in_=nx_i[:, t * P:(t + 1) * P],
                                        identity=idn)
                    copy_ps(nxT[t][:, i * P:(i + 1) * P], tp[:], t)

            # ---------- routers (c,q,k,v) ----------
            wpool_ps = psv.tile([1, 4 * NEXP], F32, tag="pvacc", name="pvacc")
            for i in range(NT):
                pr_ps = psa.tile([P, 4 * NEXP], F32, tag="acc", name="acc")
                for t in range(DT_T):
                    nc.tensor.matmul(out=pr_ps[:],
                                     lhsT=nxT[t][:, i * P:(i + 1) * P],
                                     rhs=wct[:, 320 * t:320 * t + 256],
                                     start=(t == 0), stop=False)
                nc.tensor.matmul(out=pr_ps[:], lhsT=ones128, rhs=brow1,
                                 start=False, stop=True)
                pref = sp.tile([P, 4 * NEXP], F32, tag="pref", name="pref")
                softmax_pool(pr_ps[:], pref[:], 4, NEXP, impa[:, i:i + 1],
                             wpool_ps, first=(i == 0), last=(i == NT - 1))

            wrow = pp.tile([1, 4 * NEXP], F32, tag="wrow", name="wrow")
            nc.vector.tensor_copy(out=wrow[:], in_=wpool_ps[:])
            for bi in range(4):
                sl = slice(bi * NEXP, (bi + 1) * NEXP)
                st = sp.tile([1, 1], F32, tag="wn_st", name="wn_st")
                nc.vector.tensor_reduce(out=st[:], in_=wrow[:, sl], axis=AX,
                                        op=ALU.add)
                nc.vector.tensor_scalar(out=st[:], in0=st[:], scalar1=1e-8,
                                        scalar2=None, op0=ALU.add)
                nc.vector.reciprocal(st[:], st[:])
                nc.vector.tensor_scalar(out=wrow[:, sl], in0=wrow[:, sl],
                                        scalar1=st[:], scalar2=None,
                                        op0=ALU.mult)
            wt0 = pst.tile([P, 1], F32, tag="tpp", name="wt0")
            nc.tensor.transpose(out=wt0[:], in_=wrow[:, 0:P], identity=one1)
            wt1 = pst.tile([P, 1], F32, tag="tpp", name="wt1")
            nc.tensor.transpose(out=wt1[:], in_=wrow[:, P:2 * P], identity=one1)
            wcolcq = pp.tile([P, 1], F32, tag="wcolcq", name="wcolcq")
            nc.vector.tensor_copy(out=wcolcq[:], in_=wt0[:])
            wcolkv = pp.tile([P, 1], F32, tag="wcolkv", name="wcolkv")
            nc.vector.tensor_copy(out=wcolkv[:], in_=wt1[:])
            wcols3 = pp.tile([NLOC, 3], F32, tag="wcols3", name="wcols3")
            nc.vector.tensor_copy(out=wcols3[:, 0:1],
                                  in_=wcolcq[NEXP:NEXP + NLOC, :])
            nc.vector.tensor_copy(out=wcols3[:, 1:2], in_=wcolkv[0:NLOC, :])
            nc.vector.tensor_copy(out=wcols3[:, 2:3],
                                  in_=wcolkv[NEXP:NEXP + NLOC, :])

            # ---------- sc combine (+pair AllReduce) ----------
            e3f = p0.tile([P, 3072], F32, tag="e3f", name="e3f")
            scf = e3f[:, 0:1024]
            combine_cn(wcolcq[0:NLOC, 0:1], cn1, scf)
            pair_allreduce(scf, 1024)
            sc_b = p0.tile([P, 1024], DT, tag="sc_b", name="sc_b")
            for t in range(DT_T):
                nc.vector.tensor_scalar(out=sc_b[:, t * P:(t + 1) * P],
                                        in0=scf[:, t * P:(t + 1) * P],
                                        scalar1=g1t[:, t:t + 1], scalar2=None,
                                        op0=ALU.mult)

            # ---------- e3 combine ----------
            w3 = group_cols(wcols3[:], 3)  # [128, 24]
            lhs3 = []
            for g in range(GLOC):
                lg = p0.tile([P, 96], DT, tag=f"e3lh{g}", name=f"e3lh{g}")
                for pl_i in range(3):
                    nc.vector.tensor_scalar(
                        out=lg[:, 32 * pl_i:32 * (pl_i + 1)], in0=bmS,
                        scalar1=w3[:, 3 * g + pl_i:3 * g + pl_i + 1],
                        scalar2=None, op0=ALU.mult)
                lhs3.append(lg)
            for b in range(4):
                acc = psa.tile([96, 1024], F32, tag="acc", name="eacc")
                for hh in range(2):
                    for g in range(GLOC):
                        gh, gl = g // 4, g % 4
                        nc.tensor.matmul(
                            out=acc[:, hh * 512:(hh + 1) * 512],
                            lhsT=lhs3[g][:],
                            rhs=pl1[b][gh][:, gl * 1024 + hh * 512:
                                           gl * 1024 + (hh + 1) * 512],
                            start=(g == 0), stop=(g == GLOC - 1))
                for pl_i in range(3):
                    copy_ps(e3f[32 * b:32 * b + 32,
                                1024 * pl_i:1024 * (pl_i + 1)],
                            acc[32 * pl_i:32 * pl_i + 32, :], b + pl_i)
            pair_allreduce(e3f, 3072)
            e3 = p0.tile([P, 3072], DT, tag="e3", name="e3")
            nc.vector.tensor_copy(out=e3[:, 0:1024], in_=e3f[:, 0:1024])
            nc.scalar.activation(out=e3[:, 1024:2048], in_=e3f[:, 1024:2048],
                                 func=ACT.Identity)
            nc.gpsimd.tensor_copy(out=e3[:, 2048:3072], in_=e3f[:, 2048:3072])

            # ---------- hT[r, q] = sum_d sc[d, r] g1[d] nx[q, d] + (b1 @ sc g1)[r]
            bsc_ps = pst.tile([1, P], F32, tag="tpp", name="bscp")
            for t in range(DT_T):
                nc.tensor.matmul(out=bsc_ps[:], lhsT=b1t[:, t:t + 1],
                                 rhs=sc_b[:, t * P:(t + 1) * P],
                                 start=(t == 0), stop=(t == DT_T - 1))
            bsc_row = sp.tile([1, P], F32, tag="bscr", name="bscr")
            nc.vector.tensor_copy(out=bsc_row[:], in_=bsc_ps[:])
            bsc_t = pst.tile([P, 1], F32, tag="tpp", name="bsct")
            nc.tensor.transpose(out=bsc_t[:], in_=bsc_row[:], identity=one1)
            bsc = sp.tile([P, 1], F32, tag="bsc", name="bsc")
            nc.vector.tensor_copy(out=bsc[:], in_=bsc_t[:])
            hT = p0.tile([P, S], DT, tag="hT")
            for j in range(2):
                hp = psa.tile([P, 512], F32, tag="acc", name="hacc")
                for t in range(DT_T):
                    nc.tensor.matmul(out=hp[:],
                                     lhsT=sc_b[:, t * P:(t + 1) * P],
                                     rhs=nxT[t][:, j * 512:(j + 1) * 512],
                                     start=(t == 0), stop=(t == DT_T - 1))
                nc.scalar.activation(out=hT[:, j * 512:(j + 1) * 512],
                                     in_=hp[:], func=ACT.Identity, bias=bsc)

            # ---------- K, Q, V ----------
            SCALE_Q = 1.0 / float(np.sqrt(DH))
            kT = [p4.tile([P, S], DT, tag=f"kT{t}", name=f"kT{t}")
                  for t in range(DT_T)]
            qT = [p4.tile([P, SQ], DT, tag=f"qT{t}", name=f"qT{t}")
                  for t in range(DT_T)]
            vext = [p4.tile([P, H * (DH + 1)], DT, tag=f"vx{i}", name=f"vx{i}")
                    for i in range(NT)]
            for t in range(DT_T):
                kp = psa.tile([P, S], F32, tag="acc", name="acc")
                for j in range(2):
                    nc.tensor.matmul(out=kp[:, j * 512:(j + 1) * 512],
                                     lhsT=e3[:, 1024 + t * P:1024 + t * P + P],
                                     rhs=hT[:, j * 512:(j + 1) * 512],
                                     start=True, stop=True)
                nc.scalar.activation(out=kT[t][:], in_=kp[:], func=ACT.Identity)
                qp = psv.tile([P, SQ], F32, tag="pvacc", name="qacc")
                nc.tensor.matmul(out=qp[:], lhsT=e3[:, t * P:t * P + P],
                                 rhs=hT[:, 0:SQ], start=True, stop=True)
                nc.vector.tensor_scalar(out=qT[t][:], in0=qp[:],
                                        scalar1=SCALE_Q, scalar2=None,
                                        op0=ALU.mult)
            for i in range(NT):
                vp = psa.tile([P, D], F32, tag="acc", name="acc")
                for j in range(2):
                    nc.tensor.matmul(
                        out=vp[:, j * 512:(j + 1) * 512],
                        lhsT=hT[:, i * P:(i + 1) * P],
                        rhs=e3[:, 2048 + j * 512:2048 + (j + 1) * 512],
                        start=True, stop=True)
                vv = vext[i][:].rearrange("p (hh c) -> p hh c", c=DH + 1)
                nc.vector.tensor_copy(
                    out=vv[:, :, 0:DH],
                    in_=vp[:].rearrange("p (hh c) -> p hh c", c=DH))
                nc.gpsimd.memset(vv[:, :, DH:DH + 1], 1.0)
        # ph0 (nxT, scf, e3f, lhs3) released

        # ---------- attention ----------
        # q-slot s covers local q-tile s; key positions {0..s} u {4..7}.
        # position j==s gets the constant tri mask via PE; positions 4..7 get
        # the per-core bias column (0 or -1e9) folded into the exp.
        attnT = [p4.tile([P, SQ], DT, tag=f"at{t}", name=f"at{t}")
                 for t in range(DT_T)]
        for hd in range(H):
            t4 = hd // 2
            hs = (hd % 2) * DH
            po = psv.tile([DH + 1, SQ], F32, tag="pvacc", name="poacc")
            for j in range(NT):
                qlo = j * P if j < QT else 0
                w = SQ - qlo
                sps = psa.tile([P, SQ], F32, tag="acc", name="sacc")
                if j < QT:
                    nc.tensor.matmul(out=sps[:, 0:P],
                                     lhsT=kT[t4][hs:hs + DH, j * P:(j + 1) * P],
                                     rhs=qT[t4][hs:hs + DH, qlo:qlo + P],
                                     start=True, stop=False)
                    if w > P:
                        nc.tensor.matmul(out=sps[:, P:w],
                                         lhsT=kT[t4][hs:hs + DH,
                                                     j * P:(j + 1) * P],
                                         rhs=qT[t4][hs:hs + DH, qlo + P:SQ],
                                         start=True, stop=True)
                else:
                    nc.tensor.matmul(out=sps[:, 0:w],
                                     lhsT=kT[t4][hs:hs + DH, j * P:(j + 1) * P],
                                     rhs=qT[t4][hs:hs + DH, qlo:SQ],
                                     start=True, stop=True)
                if j < QT:
                    nc.tensor.matmul(out=sps[:, 0:P], lhsT=idn,
                                     rhs=trix[:, 0:P], start=False, stop=True)
                pt = sp.tile([P, SQ], DT, tag="p_tile", name="p_tile", bufs=3)
                if j < QT:
                    nc.scalar.activation(out=pt[:, 0:w], in_=sps[:, 0:w],
                                         func=ACT.Exp)
                else:
                    nc.scalar.activation(out=pt[:, 0:w], in_=sps[:, 0:w],
                                         func=ACT.Exp, bias=negc)
                nc.tensor.matmul(
                    out=po[:, qlo:SQ],
                    lhsT=vext[j][:, hd * (DH + 1):(hd + 1) * (DH + 1)],
                    rhs=pt[:, 0:w], start=(j == 0), stop=(j == NT - 1))
            rec = sp.tile([1, SQ], F32, tag="rec", name="rec")
            nc.vector.reciprocal(rec[:], po[DH:DH + 1, :])
            recB = sp.tile([DH, SQ], F32, tag="recB", name="recB")
            nc.gpsimd.partition_broadcast(recB[:], rec[:])
            nc.vector.tensor_tensor(out=attnT[t4][hs:hs + DH, :],
                                    in0=po[0:DH, :], in1=recB[:], op=ALU.mult)

        # ---------- WO + residual (into xa) ----------
        for i in range(QT):
            wp = psa.tile([P, D], F32, tag="acc", name="acc")
            for j in range(2):
                for t in range(DT_T):
                    toff = 1024 * t + 512 * j
                    nc.tensor.matmul(
                        out=wp[:, j * 512:(j + 1) * 512],
                        lhsT=attnT[t][:, i * P:(i + 1) * P],
                        rhs=wota2[toff // 4096][:, toff % 4096:
                                                toff % 4096 + 512],
                        start=(t == 0), stop=(t == DT_T - 1))
            nc.vector.tensor_tensor(out=xa[:, i * 1024:(i + 1) * 1024],
                                    in0=wp[:], in1=xa[:, i * 1024:(i + 1) * 1024],
                                    op=ALU.add)

        ctx4.close()

        # ---------- memory block ----------
        with tc.tile_pool(name="ph6", bufs=1) as p6:
            nx2T = [p6.tile([P, SQ], DT, tag=f"n2T{t}", name=f"n2T{t}")
                    for t in range(DT_T)]
            kkt = p6.tile([KR, NK], DT, tag="kkt", name="kkt")
            nc.sync.dma_start(out=kkt[:], in_=I["kkt"][:])
            for i in range(QT):
                nx2_i = layernorm_tile(xa[:, i * 1024:(i + 1) * 1024], sp,
                                       "nx2")
                for t in range(DT_T):
                    tp = pst.tile([P, P], DT, tag="tpp", name="tpp")
                    nc.tensor.transpose(out=tp[:],
                                        in_=nx2_i[:, t * P:(t + 1) * P],
                                        identity=idn)
                    copy_ps(nx2T[t][:, i * P:(i + 1) * P], tp[:], t)

            mwp_ps = psv.tile([1, NEXP], F32, tag="pvacc", name="pvacc")
            for i in range(QT):
                pr = psa.tile([P, NEXP], F32, tag="acc", name="acc")
                for t in range(DT_T):
                    nc.tensor.matmul(out=pr[:],
                                     lhsT=nx2T[t][:, i * P:(i + 1) * P],
                                     rhs=wct[:, 320 * t + 256:320 * t + 320],
                                     start=(t == 0), stop=False)
                nc.tensor.matmul(out=pr[:], lhsT=ones128, rhs=brow2,
                                 start=False, stop=True)
                prefm = sp.tile([P, NEXP], F32, tag="prefm", name="prefm")
                softmax_pool(pr[:], prefm[:], 1, NEXP, impa[:, i:i + 1],
                             mwp_ps, first=(i == 0), last=(i == QT - 1))

            mwrow = p6.tile([1, NEXP], F32, tag="mwrow", name="mwrow")
            if use_cc:
                mwr = sp.tile([1, NEXP], F32, tag="mwr", name="mwr")
                nc.vector.tensor_copy(out=mwr[:], in_=mwp_ps[:])
                ccp = psa.tile([B, NEXP], F32, tag="acc", name="acc")
                nc.tensor.matmul(out=ccp[:], lhsT=bselr, rhs=mwr[:],
                                 start=True, stop=True)
                cc_sb = sp.tile([B, NEXP], F32, tag="cc_sb", name="cc_sb")
                nc.vector.tensor_copy(out=cc_sb[:], in_=ccp[:])
                cc_in = dr.tile([B, NEXP], F32)
                cc_out = dr.tile([B, NEXP], F32)
                nc.gpsimd.dma_start(out=cc_in[:], in_=cc_sb[:])
                nc.gpsimd.collective_compute(
                    "AllReduce", ALU.add,
                    replica_groups=[list(range(N_CORES))],
                    ins=[cc_in.opt()], outs=[cc_out.opt()])
                cc_res = sp.tile([B, NEXP], F32, tag="cc_res", name="cc_res")
                nc.gpsimd.dma_start(out=cc_res[:], in_=cc_out[:])
                mwf = psa.tile([1, NEXP], F32, tag="acc", name="acc")
                nc.tensor.matmul(out=mwf[:], lhsT=bselc, rhs=cc_res[:],
                                 start=True, stop=True)
                nc.vector.tensor_copy(out=mwrow[:], in_=mwf[:])
            else:
                nc.vector.tensor_copy(out=mwrow[:], in_=mwp_ps[:])
            st = sp.tile([1, 1], F32, tag="wn_st", name="wn_st")
            nc.vector.tensor_reduce(out=st[:], in_=mwrow[:], axis=AX,
                                    op=ALU.add)
            nc.vector.tensor_scalar(out=st[:], in0=st[:], scalar1=1e-8,
                                    scalar2=None, op0=ALU.add)
            nc.vector.reciprocal(st[:], st[:])
            nc.vector.tensor_scalar(out=mwrow[:], in0=mwrow[:], scalar1=st[:],
                                    scalar2=None, op0=ALU.mult)
            mwrow_cp = sp.tile([1, NEXP], F32, tag="mwr2", name="mwr2")
            nc.vector.tensor_copy(out=mwrow_cp[:], in_=mwrow[:])
            mwt = pst.tile([NEXP, 1], F32, tag="tpp", name="mwt")
            nc.tensor.transpose(out=mwt[:], in_=mwrow_cp[:], identity=one1)
            mwcol = p6.tile([NEXP, 1], F32, tag="mwcol", name="mwcol")
            nc.vector.tensor_copy(out=mwcol[:], in_=mwt[:])

            scmf = p6.tile([P, 1024], F32, tag="scmf", name="scmf")
            combine_cn(mwcol[0:NLOC, 0:1], cn2, scmf)
            pair_allreduce(scmf, 1024)
            scm_b = p6.tile([P, 1024], DT, tag="scm_b", name="scm_b")
            for t in range(DT_T):
                nc.vector.tensor_scalar(out=scm_b[:, t * P:(t + 1) * P],
                                        in0=scmf[:, t * P:(t + 1) * P],
                                        scalar1=g2t[:, t:t + 1], scalar2=None,
                                        op0=ALU.mult)

            # QmT [r, sq]
            qmp = psv.tile([P, SQ], F32, tag="pvacc", name="pvacc")
            for t in range(DT_T):
                nc.tensor.matmul(out=qmp[:], lhsT=scm_b[:, t * P:(t + 1) * P],
                                 rhs=nx2T[t][:], start=(t == 0),
                                 stop=(t == DT_T - 1))
            bscm_ps = pst.tile([1, P], F32, tag="tpp", name="bscmp")
            for t in range(DT_T):
                nc.tensor.matmul(out=bscm_ps[:], lhsT=b2t[:, t:t + 1],
                                 rhs=scm_b[:, t * P:(t + 1) * P],
                                 start=(t == 0), stop=(t == DT_T - 1))
            bscm_row = sp.tile([1, P], F32, tag="bscr", name="bscmr")
            nc.vector.tensor_scalar(out=bscm_row[:], in0=bscm_ps[:],
                                    scalar1=1.0 / float(np.sqrt(KR)),
                                    scalar2=None, op0=ALU.mult)
            bscm_t = pst.tile([P, 1], F32, tag="tpp", name="bscmt")
            nc.tensor.transpose(out=bscm_t[:], in_=bscm_row[:], identity=one1)
            bscm = sp.tile([P, 1], F32, tag="bsc", name="bscm")
            nc.vector.tensor_copy(out=bscm[:], in_=bscm_t[:])
            qmT = p6.tile([P, SQ], DT, tag="qmT")
            nc.scalar.activation(out=qmT[:], in_=qmp[:], func=ACT.Identity,
                                 scale=1.0 / float(np.sqrt(KR)), bias=bscm)

            idx_all = p6.tile([P, QT * TOPK], U32, tag="idx_all",
                              name="idx_all")
            w8_all = p6.tile([P, QT * TOPK], F32, tag="w8_all", name="w8_all")
            for i in range(QT):
                ks = p6.tile([P, NK], F32, tag="ks_sb", name="ks_sb")
                for j in range(NK // 512):
                    ksp = psa.tile([P, 512], F32, tag="acc", name="acc")
                    nc.tensor.matmul(out=ksp[:],
                                     lhsT=qmT[:, i * P:(i + 1) * P],
                                     rhs=kkt[:, j * 512:(j + 1) * 512],
                                     start=True, stop=True)
                    eng = nc.scalar if (j % 2 == 0) else nc.vector
                    if eng is nc.scalar:
                        nc.scalar.activation(out=ks[:, j * 512:(j + 1) * 512],
                                             in_=ksp[:], func=ACT.Identity)
                    else:
                        nc.vector.tensor_copy(out=ks[:, j * 512:(j + 1) * 512],
                                              in_=ksp[:])
                tv = sp.tile([P, TOPK], F32, tag="tv", name="tv")
                nc.vector.max_with_indices(
                    out_max=tv[:],
                    out_indices=idx_all[:, i * TOPK:(i + 1) * TOPK],
                    in_=ks[:])
                st8 = sp.tile([P, 2], F32, tag="st8", name="st8")
                nm = st8[:, 0:1]; se8 = st8[:, 1:2]
                nc.vector.tensor_scalar(out=nm, in0=tv[:, 0:1], scalar1=-1.0,
                                        scalar2=None, op0=ALU.mult)
                w8 = sp.tile([P, TOPK], F32, tag="w8", name="w8")
                nc.scalar.activation(out=w8[:], in_=tv[:], func=ACT.Exp,
                                     bias=nm, accum_out=se8)
                nc.vector.reciprocal(se8, se8)
                nc.vector.tensor_scalar(out=w8_all[:, i * TOPK:(i + 1) * TOPK],
                                        in0=w8[:], scalar1=se8, scalar2=None,
                                        op0=ALU.mult)

            for i in range(QT):
                acc = p6.tile([P, D], F32, tag="mem_acc", name="mem_acc", bufs=2)
                gt8 = p6.tile([P, TOPK * D], DT, tag="gath8", name="gath8", bufs=2)
                nc.gpsimd.indirect_dma_start(
                    out=gt8[:], out_offset=None, in_=I["kv"][:],
                    in_offset=bass.IndirectOffsetOnAxis(
                        ap=idx_all[:, i * TOPK:(i + 1) * TOPK], axis=0))
                accB = p6.tile([P, D], F32, tag="mem_accB", name="mem_accB",
                               bufs=2)
                tmp = sp.tile([P, D], F32, tag="gtmp", name="gtmp")
                for k in range(TOPK):
                    g = i * TOPK + k
                    if k % 2 == 0:
                        prev = (xa[:, i * 1024:(i + 1) * 1024] if k == 0
                                else acc[:])
                        nc.vector.scalar_tensor_tensor(
                            out=acc[:], in0=gt8[:, k * D:(k + 1) * D],
                            scalar=w8_all[:, g:g + 1],
                            in1=prev, op0=ALU.mult, op1=ALU.add)
                    else:
                        dst = accB[:] if k == 1 else tmp[:]
                        nc.scalar.activation(out=dst,
                                             in_=gt8[:, k * D:(k + 1) * D],
                                             func=ACT.Identity,
                                             scale=w8_all[:, g:g + 1])
                        if k > 1:
                            nc.gpsimd.tensor_tensor(out=accB[:], in0=tmp[:],
                                                    in1=accB[:], op=ALU.add)
                nc.vector.tensor_tensor(out=acc[:], in0=acc[:], in1=accB[:],
                                        op=ALU.add)
                nc.sync.dma_start(out=o[i * P:(i + 1) * P, :], in_=acc[:])


# ---------------- PJRT SPMD runner (persistent jit) ----------------

class SpmdRunner:
    def __init__(self, nc, n_cores):
        import jax
        from jax.sharding import Mesh, PartitionSpec
        from jax.experimental.shard_map import shard_map
        from concourse import bass2jax
        bass2jax.install_neuronx_cc_hook()
        self.jax = jax
        self.nc = nc
        self.n_cores = n_cores
        partition_name = (nc.partition_id_tensor.name
                          if nc.partition_id_tensor else None)
        in_names, out_names, out_avals, zero_outs = [], [], [], []
        for alloc in nc.m.functions[0].allocations:
            if not isinstance(alloc, mybir.MemoryLocationSet):
                continue
            name = alloc.memorylocations[0].name
            if alloc.kind == "ExternalInput":
                if name != partition_name:
                    in_names.append(name)
            elif alloc.kind == "ExternalOutput":
                shape = tuple(alloc.tensor_shape)
                dtype = mybir.dt.np(alloc.dtype)
                out_names.append(name)
                out_avals.append(jax.core.ShapedArray(shape, dtype))
                zero_outs.append(np.zeros(shape, dtype))
        self.n_params = len(in_names)
        self.in_names = list(in_names)
        self.out_names = out_names
        self.out_avals = out_avals
        self.zero_outs = zero_outs
        all_in = in_names + out_names + ([partition_name] if partition_name
                                         else [])

        def _body(*args):
            operands = list(args)
            if partition_name is not None:
                operands.append(bass2jax.partition_id_tensor())
            outs = bass2jax._bass_exec_p.bind(
                *operands, out_avals=tuple(out_avals), in_names=tuple(all_in),
                out_names=tuple(out_names), lowering_input_output_aliases=(),
                sim_require_finite=True, sim_require_nnan=True, nc=nc)
            return tuple(outs)

        devices = jax.devices()[:n_cores]
        self.mesh = Mesh(np.asarray(devices), ("core",))
        nspec = self.n_params + len(out_names)
        self.sharded = jax.jit(
            shard_map(_body, mesh=self.mesh,
                      in_specs=(PartitionSpec("core"),) * nspec,
                      out_specs=(PartitionSpec("core"),) * len(out_names),
                      check_rep=False),
            keep_unused=True)

    def concat_inputs(self, in_maps):
        per_core = [[np.asarray(m[n]) for n in self.in_names] for m in in_maps]
        cat = [np.concatenate([per_core[c][i] for c in range(self.n_cores)],
                              axis=0) for i in range(self.n_params)]
        cat += [np.zeros((self.n_cores * z.shape[0], *z.shape[1:]), z.dtype)
                for z in self.zero_outs]
        return cat

    def run(self, in_maps):
        out_arrs = self.sharded(*self.concat_inputs(in_maps))
        self.jax.block_until_ready(out_arrs)
        return [
            {n: np.asarray(out_arrs[i]).reshape(
                self.n_cores, *self.out_avals[i].shape)[c]
             for i, n in enumerate(self.out_names)}
            for c in range(self.n_cores)
        ]


# ---------------- host side ----------------

_RUNNER = None


def _make_inputs(x, importance, mask, compress_neurons, expand_pool,
                 knowledge_K, knowledge_V, Wc, WQ, WK, WV, Wm, WO,
                 g1, b1, g2, b2):
    ndt = np_bdt()
    f = lambda a: np.asarray(a, np.float32)
    cn = f(compress_neurons)
    pl = f(expand_pool)
    wstack = np.concatenate([f(Wc), f(WQ), f(WK), f(WV), f(Wm)], axis=0)
    wot = np.ascontiguousarray(f(WO).T)  # [D, D] = WO.T
    wotr = np.empty((P, NT * 1024), np.float32)
    for t in range(NT):
        wotr[:, 1024 * t:1024 * (t + 1)] = wot[128 * t:128 * (t + 1), :]
    kkt = np.ascontiguousarray(f(knowledge_K).T).astype(ndt)
    kv = f(knowledge_V).astype(ndt)

    # aux (core-independent parts)
    auxb = np.zeros((P, AB_W), np.float32)
    auxb[:, AB_IDN:AB_IDN + P] = np.eye(P)
    auxb[:, AB_BMS:AB_BMS + 32] = (
        (np.arange(P)[:, None] // 4) == np.arange(32)[None, :])
    ktri = np.arange(P)
    auxb[:, AB_TRI:AB_TRI + P] = np.where(
        ktri[None, :] >= ktri[:, None], 0.0, NEG)  # tri[k, q]; rest zeros
    auxb[:, AB_B1:AB_B1 + NT] = f(b1).reshape(NT, P).T
    auxb[:, AB_B2:AB_B2 + NT] = f(b2).reshape(NT, P).T
    auxb = auxb.astype(ndt)

    x = f(x); importance = f(importance)
    in_maps = []
    for c in range(N_CORES):
        b, hf = c // 2, c % 2
        qr = np.arange(hf * SQ, hf * SQ + SQ)
        rest = np.arange((1 - hf) * SQ, (1 - hf) * SQ + SQ)
        perm = np.concatenate([qr, rest])
        eperm = (np.arange(NEXP) + NLOC * hf) % NEXP  # local experts first

        m = {}
        m["x"] = np.ascontiguousarray(x[b][perm])
        impc = importance[b][perm].reshape(NT, P).T  # [p, i]
        m["imp"] = np.ascontiguousarray(impc).astype(np.float32)

        # wct: [128, 8*320]; block order [Wc|WQ|WK|WV|Wm], experts permuted.
        # g1 folded into router1 blocks, g2 into the Wm block (LN emits the
        # plain normalized value).
        wp_ = wstack.reshape(5, NEXP, D)[:, eperm, :].reshape(5 * NEXP, D)
        wp_ = wp_ * np.concatenate([np.tile(f(g1).ravel()[None, :], (4 * NEXP, 1)),
                                    np.tile(f(g2).ravel()[None, :], (NEXP, 1))])
        wctT = wp_.T  # [D, 320]
        wcth = np.empty((P, NT * 320), np.float32)
        for t in range(NT):
            wcth[:, 320 * t:320 * (t + 1)] = wctT[128 * t:128 * (t + 1), :]
        m["wct"] = np.ascontiguousarray(wcth).astype(ndt)
        m["wot"] = wotr.astype(ndt)
        m["kkt"] = kkt
        m["kv"] = kv

        # cnb[b4, g, p, 128t + r] = cn[e(g,p), 128t + 32*b4 + p//4, r]
        loc = eperm[:NLOC]
        cl = cn[loc]                      # [32, D, R]
        clr = cl.reshape(NLOC, 8, 128, R) # [n, t, dsub, r]
        # dsub = 32*b4 + p//4 ; partition p = 4*(p//4) + n%4
        cnb = np.empty((4, GLOC, P, 1024), np.float32)
        for b4 in range(4):
            blk = clr[:, :, 32 * b4:32 * (b4 + 1), :]   # [n, t, 32, r]
            for g in range(GLOC):
                for e in range(4):
                    n = 4 * g + e
                    # partition p = 4*m + e (m = dsub idx), free = 128t + r
                    cnb[b4, g, e::4, :] = blk[n].transpose(1, 0, 2).reshape(
                        32, 8 * 128)
        m["cnb"] = np.ascontiguousarray(cnb).astype(ndt)

        pll = pl[loc]                     # [32, R, D]
        plb = np.empty((4, GLOC, P, 1024), np.float32)
        for b4 in range(4):
            blk = pll[:, 32 * b4:32 * (b4 + 1), :]      # [n, 32, D]
            for g in range(GLOC):
                for e in range(4):
                    plb[b4, g, e::4, :] = blk[4 * g + e]
        m["plb"] = np.ascontiguousarray(plb).astype(ndt)

        auxf = np.zeros((P, AF_W), np.float32)
        auxf[:NLOC, AF_A4:AF_A4 + P] = (
            (np.arange(NLOC)[:, None] % 4) == (np.arange(P)[None, :] % 4))
        auxf[:NLOC, AF_B8:AF_B8 + GLOC] = (
            (np.arange(NLOC)[:, None] // 4) == np.arange(GLOC)[None, :])
        onehot = np.zeros(B, np.float32); onehot[b] = 1.0
        auxf[0:1, AF_BSELR:AF_BSELR + B] = onehot[None, :]
        auxf[0:B, AF_BSELC:AF_BSELC + 1] = onehot[:, None]
        auxf[:, AF_NEGC] = NEG if hf == 0 else 0.0
        auxf[0, AF_ONE] = 1.0
        auxf[0, AF_ONES:AF_ONES + P] = 1.0
        auxf[:, AF_G1:AF_G1 + NT] = f(g1).reshape(NT, P).T
        auxf[:, AF_G2:AF_G2 + NT] = f(g2).reshape(NT, P).T
        wr1 = wstack.reshape(5, NEXP, D)[:4, eperm, :]
        auxf[0, AF_BR1:AF_BR1 + 256] = (
            wr1.reshape(256, D) @ f(b1).ravel())
        wr2 = wstack.reshape(5, NEXP, D)[4, eperm, :]
        auxf[0, AF_BR2:AF_BR2 + NEXP] = wr2 @ f(b2).ravel()
        m["auxf"] = auxf
        m["auxb"] = auxb
        in_maps.append(m)
    return in_maps


def _get_runner():
    global _RUNNER
    if _RUNNER is None:
        nc = build_nc(use_cc=True)
        _RUNNER = SpmdRunner(nc, N_CORES)
    return _RUNNER


def kernel(**inputs):
    r = _get_runner()
    in_maps = _make_inputs(**inputs)
    res = r.run(in_maps)
    out = np.empty((B, S, D), np.float32)
    for c in range(N_CORES):
        b, hf = c // 2, c % 2
        out[b, hf * SQ:(hf + 1) * SQ] = res[c]["o"]
    return out


# revision 19
# speedup vs baseline: 1.2716x; 1.1126x over previous
"""DAWN block (moe_routing) Trainium2 kernel: 8-core SPMD, v2.

Sharding: core c = (batch b=c//2, half h=c%2). Each core handles one batch's
attention + memory block for half the queries (rows permuted so local queries
come first). Expert pools (compress_neurons / expand_pool) are pair-sharded:
each core streams only 32 of 64 experts and partial combines are AllReduced
within the pair. Causal structure: per-q-slot key-position lists + a constant
triangular mask tile + per-core bias column (full-mask blocks), so ~19% of
score/AV work is skipped and no per-key mask tensor is needed.

DMA strategy: everything is host-relaid-out so the device does few, large,
contiguous DMAs (the v1 kernel's 456 tiny combine DMAs were the bottleneck:
each DMA costs ~0.6us queue dispatch + 625ns shared HWDGE serial time).
"""
import numpy as np
import ml_dtypes

import concourse.bass as bass
import concourse.mybir as mybir
import concourse.tile as tile
from concourse import bacc

B, S, D = 4, 1024, 1024
H, DH = 16, 64
R = 128
NEXP = 64
NLOC = 32          # local experts per core (pair-sharded)
GLOC = NLOC // 4   # 8 stacked-expert groups
NK, KR = 4096, 128
TOPK = 8
N_CORES = 8
SQ = S // 2
P = 128
NT = S // P        # 8 seq tiles
QT = SQ // P       # 4 local q tiles
DT_T = D // P      # 8 d tiles

F32 = mybir.dt.float32
U32 = mybir.dt.uint32
NEG = -1.0e9
ALU = mybir.AluOpType
ACT = mybir.ActivationFunctionType
AX = None

# aux column maps
AF_A4 = 0          # [0:32, 0:128]
AF_B8 = 128        # [0:32, 128:136]
AF_BSELR = 136     # [0:1, 136:140]
AF_BSELC = 140     # [0:4, 140:141]
AF_NEGC = 141      # [0:128, 141:142]
AF_ONE = 142       # [0:1, 142:143]  value 1.0
AF_ONES = 144      # [0:1, 144:272] row of ones
AF_G1 = 272        # [0:128, 272:280] g1 tiled [p, t]
AF_G2 = 280        # [0:128, 280:288] g2 tiled
AF_BR1 = 304       # [0:1, 304:560] b1 @ [Wc|WQ|WK|WV].T (perm)
AF_BR2 = 560       # [0:1, 560:624] b2 @ Wm.T (perm)
AF_W = 624
AB_IDN = 0         # [0:128, 0:128]
AB_BMS = 128       # [0:128, 128:160]
AB_TRI = 160       # [0:128, 160:672] = [tri | zeros x3]
AB_B1 = 672        # [0:128, 672:680] b1 tiled [p, t]
AB_B2 = 680        # [0:128, 680:688] b2 tiled
AB_W = 688


def bdt():
    return mybir.dt.bfloat16


def np_bdt():
    return ml_dtypes.bfloat16


def build_nc(use_cc=True):
    global AX
    AX = mybir.AxisListType.X
    DT = bdt()
    nc = bacc.Bacc("TRN2", target_bir_lowering=False, debug=False,
                   num_devices=N_CORES)
    I = {}

    def inp(name, shape, dt):
        I[name] = nc.dram_tensor(name, shape, dt, kind="ExternalInput").ap()

    inp("x", [S, D], F32)              # row-permuted batch (local q first)
    inp("imp", [P, NT], F32)           # imp[p,i] = importance[perm[128i+p]]
    inp("cnb", [4, GLOC, P, 1024], DT) # local-expert compress pool, relaid
    inp("plb", [4, GLOC, P, 1024], DT) # local-expert expand pool, relaid
    inp("wct", [P, NT * 320], DT)      # [Wc|WQ|WK|WV|Wm].T tiled (expert-perm)
    inp("wot", [P, NT * 1024], DT)     # WO.T tiled
    inp("kkt", [KR, NK], DT)           # knowledge_K.T
    inp("kv", [NK, D], DT)             # knowledge_V
    inp("auxf", [P, AF_W], F32)
    inp("auxb", [P, AB_W], DT)
    o = nc.dram_tensor("o", [SQ, D], F32, kind="ExternalOutput").ap()

    with tile.TileContext(nc) as tc:
        _body(nc, tc, I, o, use_cc)
    nc.compile()
    return nc


def _body(nc, tc, I, o, use_cc):
    DT = bdt()
    import contextlib
    ctx = contextlib.ExitStack()
    with ctx:
        pp = ctx.enter_context(tc.tile_pool(name="pers", bufs=1))
        sp = ctx.enter_context(tc.tile_pool(name="stream", bufs=2))
        st2 = ctx.enter_context(tc.tile_pool(name="strm", bufs=5))
        pst = ctx.enter_context(tc.tile_pool(name="ps_t", bufs=2, space="PSUM"))
        psa = ctx.enter_context(tc.tile_pool(name="ps_a", bufs=2, space="PSUM"))
        psv = ctx.enter_context(tc.tile_pool(name="ps_v", bufs=2, space="PSUM"))
        dr = ctx.enter_context(tc.tile_pool(name="dram", bufs=1, space="DRAM"))

        # ---------- bulk loads ----------
        xa = pp.tile([P, NT * 1024], F32, tag="xa", name="xa")
        for hh in range(2):
            src = bass.AP(I["x"].tensor, hh * 4 * P * 1024,
                          [[1024, P], [P * 1024, 4], [1, 1024]])
            nc.sync.dma_start(out=xa[:, hh * 4096:(hh + 1) * 4096], in_=src)
        wct = pp.tile([P, NT * 320], DT, tag="wct", name="wct")
        nc.sync.dma_start(out=wct[:], in_=I["wct"][:])
        impa = pp.tile([P, NT], F32, tag="impa", name="impa")
        nc.sync.dma_start(out=impa[:], in_=I["imp"][:])
        auxf = pp.tile([P, AF_W], F32, tag="auxf", name="auxf")
        nc.sync.dma_start(out=auxf[:], in_=I["auxf"][:])
        auxb = pp.tile([P, AB_W], DT, tag="auxb", name="auxb")
        nc.sync.dma_start(out=auxb[:], in_=I["auxb"][:])

        idn = auxb[:, AB_IDN:AB_IDN + P]
        bmS = auxb[:, AB_BMS:AB_BMS + 32]
        trix = auxb[:, AB_TRI:AB_TRI + 4 * P]
        A4 = auxf[0:NLOC, AF_A4:AF_A4 + P]
        B8 = auxf[0:NLOC, AF_B8:AF_B8 + GLOC]
        bselr = auxf[0:1, AF_BSELR:AF_BSELR + B]
        bselc = auxf[0:B, AF_BSELC:AF_BSELC + 1]
        negc = auxf[:, AF_NEGC:AF_NEGC + 1]
        one1 = auxf[0:1, AF_ONE:AF_ONE + 1]
        ones128 = auxf[0:1, AF_ONES:AF_ONES + P]
        g1t = auxf[:, AF_G1:AF_G1 + NT]
        g2t = auxf[:, AF_G2:AF_G2 + NT]
        b1t = auxb[:, AB_B1:AB_B1 + NT]
        b2t = auxb[:, AB_B2:AB_B2 + NT]
        brow1 = auxf[0:1, AF_BR1:AF_BR1 + 256]
        brow2 = auxf[0:1, AF_BR2:AF_BR2 + NEXP]

        # expert pool streams in half-b chunks: [128, 4096] = 4 g-groups.
        def stream_half(tensor, b, half):
            t = st2.tile([P, 4096], DT, tag="strm", name="strm")
            srcap = bass.AP(tensor, (b * GLOC + half * 4) * P * 1024,
                            [[1024, P], [P * 1024, 4], [1, 1024]])
            nc.sync.dma_start(out=t[:], in_=srcap)
            return t

        cn1 = [[stream_half(I["cnb"].tensor, b, h) for h in range(2)]
               for b in range(4)]
        pl1 = [[stream_half(I["plb"].tensor, b, h) for h in range(2)]
               for b in range(4)]
        wota2 = []
        for h in range(2):
            t = st2.tile([P, 4096], DT, tag="strm", name="strm")
            nc.sync.dma_start(out=t[:], in_=I["wot"][:, h * 4096:(h + 1) * 4096])
            wota2.append(t)
        cn2 = [[stream_half(I["cnb"].tensor, b, h) for h in range(2)]
               for b in range(4)]

        def copy_ps(out_ap, in_ap, k):
            nc.vector.tensor_copy(out=out_ap, in_=in_ap)

        def layernorm_tile(x_ap, pool, tag):
            # bn_stats computes per-partition mean/var in one DVE pass per
            # 512-wide subgroup; the final Act pass folds (x - mean) * rstd.
            bst = sp.tile([P, 2, 6], F32, tag="ln_bst", name="ln_bst")
            xg = x_ap.rearrange("p (s f) -> p s f", f=512)
            nc.vector.bn_stats(out=bst[:, 0, :], in_=xg[:, 0, :])
            nc.vector.bn_stats(out=bst[:, 1, :], in_=xg[:, 1, :])
            stats = sp.tile([P, 4], F32, tag="ln_stats", name="ln_stats")
            mv = stats[:, 0:2]
            rstd = stats[:, 2:3]; nmr = stats[:, 3:4]
            nc.vector.bn_aggr(out=mv, in_=bst[:])
            nc.vector.tensor_scalar(out=rstd, in0=stats[:, 1:2], scalar1=1e-5,
                                    scalar2=None, op0=ALU.add)
            nc.scalar.sqrt(rstd, rstd)
            nc.vector.reciprocal(rstd, rstd)
            nc.vector.tensor_tensor(out=nmr, in0=stats[:, 0:1], in1=rstd,
                                    op=ALU.mult)
            nc.vector.tensor_scalar(out=nmr, in0=nmr, scalar1=-1.0,
                                    scalar2=None, op0=ALU.mult)
            out = pool.tile([P, D], DT, tag=tag)
            nc.scalar.activation(out=out[:], in_=x_ap, func=ACT.Identity,
                                 scale=rstd, bias=nmr)
            return out

        def softmax_pool(psum_ap, out_ap, nblk, blk, imp_col, pool_out,
                         first, last):
            # exp (no max-sub; scores are O(1)) with per-block accum, then
            # pool with 1/Z folded into the importance column.
            zs = sp.tile([P, 8], F32, tag="sm_zs", name="sm_zs")
            for bi in range(nblk):
                sl = slice(bi * blk, (bi + 1) * blk)
                nc.scalar.activation(out=out_ap[:, sl], in_=psum_ap[:, sl],
                                     func=ACT.Exp, accum_out=zs[:, bi:bi + 1])
            nc.vector.reciprocal(zs[:, 0:nblk], zs[:, 0:nblk])
            impz = sp.tile([P, 8], F32, tag="sm_iz", name="sm_iz")
            nc.vector.tensor_scalar(out=impz[:, 0:nblk], in0=zs[:, 0:nblk],
                                    scalar1=imp_col, scalar2=None,
                                    op0=ALU.mult)
            for bi in range(nblk):
                sl = slice(bi * blk, (bi + 1) * blk)
                nc.tensor.matmul(out=pool_out[:, sl],
                                 lhsT=impz[:, bi:bi + 1], rhs=out_ap[:, sl],
                                 start=first, stop=last)

        def group_cols(wcol_ap, ncols):
            """wcol [32, ncols] f32 -> wk [128, GLOC*ncols]:
            wk[p, ncols*g + c] = wcol[4g + p%4, c]."""
            rhsB = sp.tile([NLOC, GLOC * ncols], F32, tag="rhsB", name="rhsB")
            for pi in range(ncols):
                nc.vector.tensor_scalar(
                    out=rhsB[:, pi:GLOC * ncols:ncols], in0=B8,
                    scalar1=wcol_ap[:, pi:pi + 1], scalar2=None, op0=ALU.mult)
            wkp = pst.tile([P, GLOC * ncols], F32, tag="tpp", name="wkp")
            nc.tensor.matmul(out=wkp[:], lhsT=A4, rhs=rhsB[:],
                             start=True, stop=True)
            wk = sp.tile([P, GLOC * ncols], F32, tag="wkall", name="wkall")
            nc.vector.tensor_copy(out=wk[:], in_=wkp[:])
            return wk

        def combine_cn(wcol_ap, chunks, out_f32):
            """out_f32 [128, 1024] f32 partial combine of local experts.
            chunks[b][p, 1024g+128t+r] = CN[e(g,p), 128t+32b+p//4, r]."""
            wk = group_cols(wcol_ap, 1)
            lhs = []
            for g in range(GLOC):
                lg = sp.tile([P, NLOC], DT, tag=f"clh{g}", name=f"clh{g}",
                             bufs=1)
                nc.vector.tensor_scalar(out=lg[:], in0=bmS,
                                        scalar1=wk[:, g:g + 1],
                                        scalar2=None, op0=ALU.mult)
                lhs.append(lg)
            for b in range(4):
                acc = psa.tile([NLOC, 1024], F32, tag="acc", name="cacc")
                for hh in range(2):
                    for g in range(GLOC):
                        gh, gl = g // 4, g % 4
                        nc.tensor.matmul(
                            out=acc[:, hh * 512:(hh + 1) * 512],
                            lhsT=lhs[g][:],
                            rhs=chunks[b][gh][:, gl * 1024 + hh * 512:
                                              gl * 1024 + (hh + 1) * 512],
                            start=(g == 0), stop=(g == GLOC - 1))
                copy_ps(out_f32[32 * b:32 * b + 32, :], acc[:], b)

        def pair_allreduce(sb_f32, ncol):
            """AllReduce sb_f32 [128, ncol] within batch pairs (in place)."""
            if not use_cc:
                return
            cc_in = dr.tile([P, ncol], F32)
            cc_out = dr.tile([P, ncol], F32)
            nc.gpsimd.dma_start(out=cc_in[:], in_=sb_f32[:])
            nc.gpsimd.collective_compute(
                "AllReduce", ALU.add,
                replica_groups=[[0, 1], [2, 3], [4, 5], [6, 7]],
                ins=[cc_in.opt()], outs=[cc_out.opt()])
            nc.gpsimd.dma_start(out=sb_f32[:], in_=cc_out[:])

        # ---------- LN1 + transposes ----------
        ctx4 = contextlib.ExitStack()
        p4 = ctx4.enter_context(tc.tile_pool(name="ph4", bufs=1))
        with tc.tile_pool(name="ph0", bufs=1) as p0:
            nxT = [p0.tile([P, S], DT, tag=f"nxT{t}", name=f"nxT{t}")
                   for t in range(DT_T)]
            for i in range(NT):
                nx_i = layernorm_tile(xa[:, i * 1024:(i + 1) * 1024], sp, "nx")
                for t in range(DT_T):
                    tp = pst.tile([P, P], DT, tag="tpp", name="tpp")
                    nc.tensor.transpose(out=tp[:],
                                        in_=nx_i[:, t * P:(t + 1) * P],
                                        identity=idn)
                    copy_ps(nxT[t][:, i * P:(i + 1) * P], tp[:], t)

            # ---------- routers (c,q,k,v) ----------
            wpool_ps = psv.tile([1, 4 * NEXP], F32, tag="pvacc", name="pvacc")
            for i in range(NT):
                pr_ps = psa.tile([P, 4 * NEXP], F32, tag="acc", name="acc")
                for t in range(DT_T):
                    nc.tensor.matmul(out=pr_ps[:],
                                     lhsT=nxT[t][:, i * P:(i + 1) * P],
                                     rhs=wct[:, 320 * t:320 * t + 256],
                                     start=(t == 0), stop=False)
                nc.tensor.matmul(out=pr_ps[:], lhsT=ones128, rhs=brow1,
                                 start=False, stop=True)
                pref = sp.tile([P, 4 * NEXP], F32, tag="pref", name="pref")
                softmax_pool(pr_ps[:], pref[:], 4, NEXP, impa[:, i:i + 1],
                             wpool_ps, first=(i == 0), last=(i == NT - 1))

            wrow = pp.tile([1, 4 * NEXP], F32, tag="wrow", name="wrow")
            nc.vector.tensor_copy(out=wrow[:], in_=wpool_ps[:])
            for bi in range(4):
                sl = slice(bi * NEXP, (bi + 1) * NEXP)
                st = sp.tile([1, 1], F32, tag="wn_st", name="wn_st")
                nc.vector.tensor_reduce(out=st[:], in_=wrow[:, sl], axis=AX,
                                        op=ALU.add)
                nc.vector.tensor_scalar(out=st[:], in0=st[:], scalar1=1e-8,
                                        scalar2=None, op0=ALU.add)
                nc.vector.reciprocal(st[:], st[:])
                nc.vector.tensor_scalar(out=wrow[:, sl], in0=wrow[:, sl],
                                        scalar1=st[:], scalar2=None,
                                        op0=ALU.mult)
            wt0 = pst.tile([P, 1], F32, tag="tpp", name="wt0")
            nc.tensor.transpose(out=wt0[:], in_=wrow[:, 0:P], identity=one1)
            wt1 = pst.tile([P, 1], F32, tag="tpp", name="wt1")
            nc.tensor.transpose(out=wt1[:], in_=wrow[:, P:2 * P], identity=one1)
            wcolcq = pp.tile([P, 1], F32, tag="wcolcq", name="wcolcq")
            nc.vector.tensor_copy(out=wcolcq[:], in_=wt0[:])
            wcolkv = pp.tile([P, 1], F32, tag="wcolkv", name="wcolkv")
            nc.vector.tensor_copy(out=wcolkv[:], in_=wt1[:])
            wcols3 = pp.tile([NLOC, 3], F32, tag="wcols3", name="wcols3")
            nc.vector.tensor_copy(out=wcols3[:, 0:1],
                                  in_=wcolcq[NEXP:NEXP + NLOC, :])
            nc.vector.tensor_copy(out=wcols3[:, 1:2], in_=wcolkv[0:NLOC, :])
            nc.vector.tensor_copy(out=wcols3[:, 2:3],
                                  in_=wcolkv[NEXP:NEXP + NLOC, :])

            # ---------- sc combine (+pair AllReduce) ----------
            e3f = p0.tile([P, 3072], F32, tag="e3f", name="e3f")
            scf = e3f[:, 0:1024]
            combine_cn(wcolcq[0:NLOC, 0:1], cn1, scf)
            pair_allreduce(scf, 1024)
            sc_b = p0.tile([P, 1024], DT, tag="sc_b", name="sc_b")
            for t in range(DT_T):
                nc.vector.tensor_scalar(out=sc_b[:, t * P:(t + 1) * P],
                                        in0=scf[:, t * P:(t + 1) * P],
                                        scalar1=g1t[:, t:t + 1], scalar2=None,
                                        op0=ALU.mult)

            # ---------- e3 combine ----------
            w3 = group_cols(wcols3[:], 3)  # [128, 24]
            lhs3 = []
            for g in range(GLOC):
                lg = p0.tile([P, 96], DT, tag=f"e3lh{g}", name=f"e3lh{g}")
                for pl_i in range(3):
                    nc.vector.tensor_scalar(
                        out=lg[:, 32 * pl_i:32 * (pl_i + 1)], in0=bmS,
                        scalar1=w3[:, 3 * g + pl_i:3 * g + pl_i + 1],
                        scalar2=None, op0=ALU.mult)
                lhs3.append(lg)
            for b in range(4):
                acc = psa.tile([96, 1024], F32, tag="acc", name="eacc")
                for hh in range(2):
                    for g in range(GLOC):
                        gh, gl = g // 4, g % 4
                        nc.tensor.matmul(
                            out=acc[:, hh * 512:(hh + 1) * 512],
                            lhsT=lhs3[g][:],
                            rhs=pl1[b][gh][:, gl * 1024 + hh * 512:
                                           gl * 1024 + (hh + 1) * 512],
                            start=(g == 0), stop=(g == GLOC - 1))
                for pl_i in range(3):
                    copy_ps(e3f[32 * b:32 * b + 32,
                                1024 * pl_i:1024 * (pl_i + 1)],
                            acc[32 * pl_i:32 * pl_i + 32, :], b + pl_i)
            pair_allreduce(e3f, 3072)
            e3 = p0.tile([P, 3072], DT, tag="e3", name="e3")
            nc.vector.tensor_copy(out=e3[:, 0:1024], in_=e3f[:, 0:1024])
            nc.scalar.activation(out=e3[:, 1024:2048], in_=e3f[:, 1024:2048],
                                 func=ACT.Identity)
            nc.gpsimd.tensor_copy(out=e3[:, 2048:3072], in_=e3f[:, 2048:3072])

            # ---------- hT[r, q] = sum_d sc[d, r] g1[d] nx[q, d] + (b1 @ sc g1)[r]
            bsc_ps = pst.tile([1, P], F32, tag="tpp", name="bscp")
            for t in range(DT_T):
                nc.tensor.matmul(out=bsc_ps[:], lhsT=b1t[:, t:t + 1],
                                 rhs=sc_b[:, t * P:(t + 1) * P],
                                 start=(t == 0), stop=(t == DT_T - 1))
            bsc_row = sp.tile([1, P], F32, tag="bscr", name="bscr")
            nc.vector.tensor_copy(out=bsc_row[:], in_=bsc_ps[:])
            bsc_t = pst.tile([P, 1], F32, tag="tpp", name="bsct")
            nc.tensor.transpose(out=bsc_t[:], in_=bsc_row[:], identity=one1)
            bsc = sp.tile([P, 1], F32, tag="bsc", name="bsc")
            nc.vector.tensor_copy(out=bsc[:], in_=bsc_t[:])
            hT = p0.tile([P, S], DT, tag="hT")
            for j in range(2):
                hp = psa.tile([P, 512], F32, tag="acc", name="hacc")
                for t in range(DT_T):
                    nc.tensor.matmul(out=hp[:],
                                     lhsT=sc_b[:, t * P:(t + 1) * P],
                                     rhs=nxT[t][:, j * 512:(j + 1) * 512],
                                     start=(t == 0), stop=(t == DT_T - 1))
                nc.scalar.activation(out=hT[:, j * 512:(j + 1) * 512],
                                     in_=hp[:], func=ACT.Identity, bias=bsc)

            # ---------- K, Q, V ----------
            SCALE_Q = 1.0 / float(np.sqrt(DH))
            kT = [p4.tile([P, S], DT, tag=f"kT{t}", name=f"kT{t}")
                  for t in range(DT_T)]
            qT = [p4.tile([P, SQ], DT, tag=f"qT{t}", name=f"qT{t}")
                  for t in range(DT_T)]
            vext = [p4.tile([P, H * (DH + 1)], DT, tag=f"vx{i}", name=f"vx{i}")
                    for i in range(NT)]
            for t in range(DT_T):
                kp = psa.tile([P, S], F32, tag="acc", name="acc")
                for j in range(2):
                    nc.tensor.matmul(out=kp[:, j * 512:(j + 1) * 512],
                                     lhsT=e3[:, 1024 + t * P:1024 + t * P + P],
                                     rhs=hT[:, j * 512:(j + 1) * 512],
                                     start=True, stop=True)
                nc.scalar.activation(out=kT[t][:], in_=kp[:], func=ACT.Identity)
                qp = psv.tile([P, SQ], F32, tag="pvacc", name="qacc")
                nc.tensor.matmul(out=qp[:], lhsT=e3[:, t * P:t * P + P],
                                 rhs=hT[:, 0:SQ], start=True, stop=True)
                nc.vector.tensor_scalar(out=qT[t][:], in0=qp[:],
                                        scalar1=SCALE_Q, scalar2=None,
                                        op0=ALU.mult)
            for i in range(NT):
                vp = psa.tile([P, D], F32, tag="acc", name="acc")
                for j in range(2):
                    nc.tensor.matmul(
                        out=vp[:, j * 512:(j + 1) * 512],
                        lhsT=hT[:, i * P:(i + 1) * P],
                        rhs=e3[:, 2048 + j * 512:2048 + (j + 1) * 512],
                        start=True, stop=True)
                vv = vext[i][:].rearrange("p (hh c) -> p hh c", c=DH + 1)
                nc.vector.tensor_copy(
                    out=vv[:, :, 0:DH],
                    in_=vp[:].rearrange("p (hh c) -> p hh c", c=DH))
                nc.gpsimd.memset(vv[:, :, DH:DH + 1], 1.0)
        # ph0 (nxT, scf, e3f, lhs3) released

        # ---------- attention ----------
        # q-slot s covers local q-tile s; key positions {0..s} u {4..7}.
        # position j==s gets the constant tri mask via PE; positions 4..7 get
        # the per-core bias column (0 or -1e9) folded into the exp.
        attnT = [p4.tile([P, SQ], DT, tag=f"at{t}", name=f"at{t}")
                 for t in range(DT_T)]
        for hd in range(H):
            t4 = hd // 2
            hs = (hd % 2) * DH
            po = psv.tile([DH + 1, SQ], F32, tag="pvacc", name="poacc")
            for j in range(NT):
                qlo = j * P if j < QT else 0
                w = SQ - qlo
                sps = psa.tile([P, SQ], F32, tag="acc", name="sacc")
                if j < QT:
                    nc.tensor.matmul(out=sps[:, 0:P],
                                     lhsT=kT[t4][hs:hs + DH, j * P:(j + 1) * P],
                                     rhs=qT[t4][hs:hs + DH, qlo:qlo + P],
                                     start=True, stop=False)
                    if w > P:
                        nc.tensor.matmul(out=sps[:, P:w],
                                         lhsT=kT[t4][hs:hs + DH,
                                                     j * P:(j + 1) * P],
                                         rhs=qT[t4][hs:hs + DH, qlo + P:SQ],
                                         start=True, stop=True)
                else:
                    nc.tensor.matmul(out=sps[:, 0:w],
                                     lhsT=kT[t4][hs:hs + DH, j * P:(j + 1) * P],
                                     rhs=qT[t4][hs:hs + DH, qlo:SQ],
                                     start=True, stop=True)
                if j < QT:
                    nc.tensor.matmul(out=sps[:, 0:P], lhsT=idn,
                                     rhs=trix[:, 0:P], start=False, stop=True)
                pt = sp.tile([P, SQ], DT, tag="p_tile", name="p_tile", bufs=3)
                if j < QT:
                    nc.scalar.activation(out=pt[:, 0:w], in_=sps[:, 0:w],
                                         func=ACT.Exp)
                else:
                    nc.scalar.activation(out=pt[:, 0:w], in_=sps[:, 0:w],
                                         func=ACT.Exp, bias=negc)
                nc.tensor.matmul(
                    out=po[:, qlo:SQ],
                    lhsT=vext[j][:, hd * (DH + 1):(hd + 1) * (DH + 1)],
                    rhs=pt[:, 0:w], start=(j == 0), stop=(j == NT - 1))
            rec = sp.tile([1, SQ], F32, tag="rec", name="rec")
            nc.vector.reciprocal(rec[:], po[DH:DH + 1, :])
            recB = sp.tile([DH, SQ], F32, tag="recB", name="recB")
            nc.gpsimd.partition_broadcast(recB[:], rec[:])
            nc.vector.tensor_tensor(out=attnT[t4][hs:hs + DH, :],
                                    in0=po[0:DH, :], in1=recB[:], op=ALU.mult)

        # ---------- WO + residual (into xa) ----------
        for i in range(QT):
            wp = psa.tile([P, D], F32, tag="acc", name="acc")
            for j in range(2):
                for t in range(DT_T):
                    toff = 1024 * t + 512 * j
                    nc.tensor.matmul(
                        out=wp[:, j * 512:(j + 1) * 512],
                        lhsT=attnT[t][:, i * P:(i + 1) * P],
                        rhs=wota2[toff // 4096][:, toff % 4096:
                                                toff % 4096 + 512],
                        start=(t == 0), stop=(t == DT_T - 1))
            nc.vector.tensor_tensor(out=xa[:, i * 1024:(i + 1) * 1024],
                                    in0=wp[:], in1=xa[:, i * 1024:(i + 1) * 1024],
                                    op=ALU.add)

        ctx4.close()

        # ---------- memory block ----------
        with tc.tile_pool(name="ph6", bufs=1) as p6:
            nx2T = [p6.tile([P, SQ], DT, tag=f"n2T{t}", name=f"n2T{t}")
                    for t in range(DT_T)]
            kkt = p6.tile([KR, NK], DT, tag="kkt", name="kkt")
            nc.sync.dma_start(out=kkt[:], in_=I["kkt"][:])
            for i in range(QT):
                nx2_i = layernorm_tile(xa[:, i * 1024:(i + 1) * 1024], sp,
                                       "nx2")
                for t in range(DT_T):
                    tp = pst.tile([P, P], DT, tag="tpp", name="tpp")
                    nc.tensor.transpose(out=tp[:],
                                        in_=nx2_i[:, t * P:(t + 1) * P],
                                        identity=idn)
                    copy_ps(nx2T[t][:, i * P:(i + 1) * P], tp[:], t)

            mwp_ps = psv.tile([1, NEXP], F32, tag="pvacc", name="pvacc")
            for i in range(QT):
                pr = psa.tile([P, NEXP], F32, tag="acc", name="acc")
                for t in range(DT_T):
                    nc.tensor.matmul(out=pr[:],
                                     lhsT=nx2T[t][:, i * P:(i + 1) * P],
                                     rhs=wct[:, 320 * t + 256:320 * t + 320],
                                     start=(t == 0), stop=False)
                nc.tensor.matmul(out=pr[:], lhsT=ones128, rhs=brow2,
                                 start=False, stop=True)
                prefm = sp.tile([P, NEXP], F32, tag="prefm", name="prefm")
                softmax_pool(pr[:], prefm[:], 1, NEXP, impa[:, i:i + 1],
                             mwp_ps, first=(i == 0), last=(i == QT - 1))

            mwrow = p6.tile([1, NEXP], F32, tag="mwrow", name="mwrow")
            if use_cc:
                mwr = sp.tile([1, NEXP], F32, tag="mwr", name="mwr")
                nc.vector.tensor_copy(out=mwr[:], in_=mwp_ps[:])
                ccp = psa.tile([B, NEXP], F32, tag="acc", name="acc")
                nc.tensor.matmul(out=ccp[:], lhsT=bselr, rhs=mwr[:],
                                 start=True, stop=True)
                cc_sb = sp.tile([B, NEXP], F32, tag="cc_sb", name="cc_sb")
                nc.vector.tensor_copy(out=cc_sb[:], in_=ccp[:])
                cc_in = dr.tile([B, NEXP], F32)
                cc_out = dr.tile([B, NEXP], F32)
                nc.gpsimd.dma_start(out=cc_in[:], in_=cc_sb[:])
                nc.gpsimd.collective_compute(
                    "AllReduce", ALU.add,
                    replica_groups=[list(range(N_CORES))],
                    ins=[cc_in.opt()], outs=[cc_out.opt()])
                cc_res = sp.tile([B, NEXP], F32, tag="cc_res", name="cc_res")
                nc.gpsimd.dma_start(out=cc_res[:], in_=cc_out[:])
                mwf = psa.tile([1, NEXP], F32, tag="acc", name="acc")
                nc.tensor.matmul(out=mwf[:], lhsT=bselc, rhs=cc_res[:],
                                 start=True, stop=True)
                nc.vector.tensor_copy(out=mwrow[:], in_=mwf[:])
            else:
                nc.vector.tensor_copy(out=mwrow[:], in_=mwp_ps[:])
            st = sp.tile([1, 1], F32, tag="wn_st", name="wn_st")
            nc.vector.tensor_reduce(out=st[:], in_=mwrow[:], axis=AX,
                                    op=ALU.add)
            nc.vector.tensor_scalar(out=st[:], in0=st[:], scalar1=1e-8,
                                    scalar2=None, op0=ALU.add)
            nc.vector.reciprocal(st[:], st[:])
            nc.vector.tensor_scalar(out=mwrow[:], in0=mwrow[:], scalar1=st[:],
                                    scalar2=None, op0=ALU.mult)
            mwrow_cp = sp.tile([1, NEXP], F32, tag="mwr2", name="mwr2")
            nc.vector.tensor_copy(out=mwrow_cp[:], in_=mwrow[:])
            mwt = pst.tile([NEXP, 1], F32, tag="tpp", name="mwt")
            nc.tensor.transpose(out=mwt[:], in_=mwrow_cp[:], identity=one1)
            mwcol = p6.tile([NEXP, 1], F32, tag="mwcol", name="mwcol")
            nc.vector.tensor_copy(out=mwcol[:], in_=mwt[:])

            scmf = p6.tile([P, 1024], F32, tag="scmf", name="scmf")
            combine_cn(mwcol[0:NLOC, 0:1], cn2, scmf)
            pair_allreduce(scmf, 1024)
            scm_b = p6.tile([P, 1024], DT, tag="scm_b", name="scm_b")
            for t in range(DT_T):
                nc.vector.tensor_scalar(out=scm_b[:, t * P:(t + 1) * P],
                                        in0=scmf[:, t * P:(t + 1) * P],
                                        scalar1=g2t[:, t:t + 1], scalar2=None,
                                        op0=ALU.mult)

            # QmT [r, sq]
            qmp = psv.tile([P, SQ], F32, tag="pvacc", name="pvacc")
            for t in range(DT_T):
                nc.tensor.matmul(out=qmp[:], lhsT=scm_b[:, t * P:(t + 1) * P],
                                 rhs=nx2T[t][:], start=(t == 0),
                                 stop=(t == DT_T - 1))
            bscm_ps = pst.tile([1, P], F32, tag="tpp", name="bscmp")
            for t in range(DT_T):
                nc.tensor.matmul(out=bscm_ps[:], lhsT=b2t[:, t:t + 1],
                                 rhs=scm_b[:, t * P:(t + 1) * P],
                                 start=(t == 0), stop=(t == DT_T - 1))
            bscm_row = sp.tile([1, P], F32, tag="bscr", name="bscmr")
            nc.vector.tensor_scalar(out=bscm_row[:], in0=bscm_ps[:],
                                    scalar1=1.0 / float(np.sqrt(KR)),
                                    scalar2=None, op0=ALU.mult)
            bscm_t = pst.tile([P, 1], F32, tag="tpp", name="bscmt")
            nc.tensor.transpose(out=bscm_t[:], in_=bscm_row[:], identity=one1)
            bscm = sp.tile([P, 1], F32, tag="bsc", name="bscm")
            nc.vector.tensor_copy(out=bscm[:], in_=bscm_t[:])
            qmT = p6.tile([P, SQ], DT, tag="qmT")
            nc.scalar.activation(out=qmT[:], in_=qmp[:], func=ACT.Identity,
                                 scale=1.0 / float(np.sqrt(KR)), bias=bscm)

            idx_all = p6.tile([P, QT * TOPK], U32, tag="idx_all",
                              name="idx_all")
            w8_all = p6.tile([P, QT * TOPK], F32, tag="w8_all", name="w8_all")
            for i in range(QT):
                ks = p6.tile([P, NK], DT, tag="ks_sb", name="ks_sb")
                for j in range(NK // 512):
                    ksp = psa.tile([P, 512], F32, tag="acc", name="acc")
                    nc.tensor.matmul(out=ksp[:],
                                     lhsT=qmT[:, i * P:(i + 1) * P],
                                     rhs=kkt[:, j * 512:(j + 1) * 512],
                                     start=True, stop=True)
                    nc.scalar.activation(out=ks[:, j * 512:(j + 1) * 512],
                                         in_=ksp[:], func=ACT.Identity)
                tv = sp.tile([P, TOPK], F32, tag="tv", name="tv")
                nc.vector.max_with_indices(
                    out_max=tv[:],
                    out_indices=idx_all[:, i * TOPK:(i + 1) * TOPK],
                    in_=ks[:])
                st8 = sp.tile([P, 2], F32, tag="st8", name="st8")
                nm = st8[:, 0:1]; se8 = st8[:, 1:2]
                nc.vector.tensor_scalar(out=nm, in0=tv[:, 0:1], scalar1=-1.0,
                                        scalar2=None, op0=ALU.mult)
                w8 = sp.tile([P, TOPK], F32, tag="w8", name="w8")
                nc.scalar.activation(out=w8[:], in_=tv[:], func=ACT.Exp,
                                     bias=nm, accum_out=se8)
                nc.vector.reciprocal(se8, se8)
                nc.vector.tensor_scalar(out=w8_all[:, i * TOPK:(i + 1) * TOPK],
                                        in0=w8[:], scalar1=se8, scalar2=None,
                                        op0=ALU.mult)

            for i in range(QT):
                acc = p6.tile([P, D], F32, tag="mem_acc", name="mem_acc", bufs=2)
                gt8 = p6.tile([P, TOPK * D], DT, tag="gath8", name="gath8", bufs=2)
                nc.gpsimd.indirect_dma_start(
                    out=gt8[:], out_offset=None, in_=I["kv"][:],
                    in_offset=bass.IndirectOffsetOnAxis(
                        ap=idx_all[:, i * TOPK:(i + 1) * TOPK], axis=0))
                accB = p6.tile([P, D], F32, tag="mem_accB", name="mem_accB",
                               bufs=2)
                tmp = sp.tile([P, D], F32, tag="gtmp", name="gtmp")
                for k in range(TOPK):
                    g = i * TOPK + k
                    if k % 2 == 0:
                        prev = (xa[:, i * 1024:(i + 1) * 1024] if k == 0
                                else acc[:])
                        nc.vector.scalar_tensor_tensor(
                            out=acc[:], in0=gt8[:, k * D:(k + 1) * D],
                            scalar=w8_all[:, g:g + 1],
                            in1=prev, op0=ALU.mult, op1=ALU.add)
                    else:
                        dst = accB[:] if k == 1 else tmp[:]
                        nc.scalar.activation(out=dst,
                                             in_=gt8[:, k * D:(k + 1) * D],
                                             func=ACT.Identity,
                                             scale=w8_all[:, g:g + 1])
                        if k > 1:
                            nc.gpsimd.tensor_tensor(out=accB[:], in0=tmp[:],
                                                    in1=accB[:], op=ALU.add)
                nc.vector.tensor_tensor(out=acc[:], in0=acc[:], in1=accB[:],
                                        op=ALU.add)
                nc.sync.dma_start(out=o[i * P:(i + 1) * P, :], in_=acc[:])


# ---------------- PJRT SPMD runner (persistent jit) ----------------

class SpmdRunner:
    def __init__(self, nc, n_cores):
        import jax
        from jax.sharding import Mesh, PartitionSpec
        from jax.experimental.shard_map import shard_map
        from concourse import bass2jax
        bass2jax.install_neuronx_cc_hook()
        self.jax = jax
        self.nc = nc
        self.n_cores = n_cores
        partition_name = (nc.partition_id_tensor.name
                          if nc.partition_id_tensor else None)
        in_names, out_names, out_avals, zero_outs = [], [], [], []
        for alloc in nc.m.functions[0].allocations:
            if not isinstance(alloc, mybir.MemoryLocationSet):
                continue
            name = alloc.memorylocations[0].name
            if alloc.kind == "ExternalInput":
                if name != partition_name:
                    in_names.append(name)
            elif alloc.kind == "ExternalOutput":
                shape = tuple(alloc.tensor_shape)
                dtype = mybir.dt.np(alloc.dtype)
                out_names.append(name)
                out_avals.append(jax.core.ShapedArray(shape, dtype))
                zero_outs.append(np.zeros(shape, dtype))
        self.n_params = len(in_names)
        self.in_names = list(in_names)
        self.out_names = out_names
        self.out_avals = out_avals
        self.zero_outs = zero_outs
        all_in = in_names + out_names + ([partition_name] if partition_name
                                         else [])

        def _body(*args):
            operands = list(args)
            if partition_name is not None:
                operands.append(bass2jax.partition_id_tensor())
            outs = bass2jax._bass_exec_p.bind(
                *operands, out_avals=tuple(out_avals), in_names=tuple(all_in),
                out_names=tuple(out_names), lowering_input_output_aliases=(),
                sim_require_finite=True, sim_require_nnan=True, nc=nc)
            return tuple(outs)

        devices = jax.devices()[:n_cores]
        self.mesh = Mesh(np.asarray(devices), ("core",))
        nspec = self.n_params + len(out_names)
        self.sharded = jax.jit(
            shard_map(_body, mesh=self.mesh,
                      in_specs=(PartitionSpec("core"),) * nspec,
                      out_specs=(PartitionSpec("core"),) * len(out_names),
                      check_rep=False),
            keep_unused=True)

    def concat_inputs(self, in_maps):
        per_core = [[np.asarray(m[n]) for n in self.in_names] for m in in_maps]
        cat = [np.concatenate([per_core[c][i] for c in range(self.n_cores)],
                              axis=0) for i in range(self.n_params)]
        cat += [np.zeros((self.n_cores * z.shape[0], *z.shape[1:]), z.dtype)
                for z in self.zero_outs]
        return cat

    def run(self, in_maps):
        out_arrs = self.sharded(*self.concat_inputs(in_maps))
        self.jax.block_until_ready(out_arrs)
        return [
            {n: np.asarray(out_arrs[i]).reshape(
                self.n_cores, *self.out_avals[i].shape)[c]
             for i, n in enumerate(self.out_names)}
            for c in range(self.n_cores)
        ]


# ---------------- host side ----------------

_RUNNER = None


def _make_inputs(x, importance, mask, compress_neurons, expand_pool,
                 knowledge_K, knowledge_V, Wc, WQ, WK, WV, Wm, WO,
                 g1, b1, g2, b2):
    ndt = np_bdt()
    f = lambda a: np.asarray(a, np.float32)
    cn = f(compress_neurons)
    pl = f(expand_pool)
    wstack = np.concatenate([f(Wc), f(WQ), f(WK), f(WV), f(Wm)], axis=0)
    wot = np.ascontiguousarray(f(WO).T)  # [D, D] = WO.T
    wotr = np.empty((P, NT * 1024), np.float32)
    for t in range(NT):
        wotr[:, 1024 * t:1024 * (t + 1)] = wot[128 * t:128 * (t + 1), :]
    kkt = np.ascontiguousarray(f(knowledge_K).T).astype(ndt)
    kv = f(knowledge_V).astype(ndt)

    # aux (core-independent parts)
    auxb = np.zeros((P, AB_W), np.float32)
    auxb[:, AB_IDN:AB_IDN + P] = np.eye(P)
    auxb[:, AB_BMS:AB_BMS + 32] = (
        (np.arange(P)[:, None] // 4) == np.arange(32)[None, :])
    ktri = np.arange(P)
    auxb[:, AB_TRI:AB_TRI + P] = np.where(
        ktri[None, :] >= ktri[:, None], 0.0, NEG)  # tri[k, q]; rest zeros
    auxb[:, AB_B1:AB_B1 + NT] = f(b1).reshape(NT, P).T
    auxb[:, AB_B2:AB_B2 + NT] = f(b2).reshape(NT, P).T
    auxb = auxb.astype(ndt)

    x = f(x); importance = f(importance)
    in_maps = []
    for c in range(N_CORES):
        b, hf = c // 2, c % 2
        qr = np.arange(hf * SQ, hf * SQ + SQ)
        rest = np.arange((1 - hf) * SQ, (1 - hf) * SQ + SQ)
        perm = np.concatenate([qr, rest])
        eperm = (np.arange(NEXP) + NLOC * hf) % NEXP  # local experts first

        m = {}
        m["x"] = np.ascontiguousarray(x[b][perm])
        impc = importance[b][perm].reshape(NT, P).T  # [p, i]
        m["imp"] = np.ascontiguousarray(impc).astype(np.float32)

        # wct: [128, 8*320]; block order [Wc|WQ|WK|WV|Wm], experts permuted.
        # g1 folded into router1 blocks, g2 into the Wm block (LN emits the
        # plain normalized value).
        wp_ = wstack.reshape(5, NEXP, D)[:, eperm, :].reshape(5 * NEXP, D)
        wp_ = wp_ * np.concatenate([np.tile(f(g1).ravel()[None, :], (4 * NEXP, 1)),
                                    np.tile(f(g2).ravel()[None, :], (NEXP, 1))])
        wctT = wp_.T  # [D, 320]
        wcth = np.empty((P, NT * 320), np.float32)
        for t in range(NT):
            wcth[:, 320 * t:320 * (t + 1)] = wctT[128 * t:128 * (t + 1), :]
        m["wct"] = np.ascontiguousarray(wcth).astype(ndt)
        m["wot"] = wotr.astype(ndt)
        m["kkt"] = kkt
        m["kv"] = kv

        # cnb[b4, g, p, 128t + r] = cn[e(g,p), 128t + 32*b4 + p//4, r]
        loc = eperm[:NLOC]
        cl = cn[loc]                      # [32, D, R]
        clr = cl.reshape(NLOC, 8, 128, R) # [n, t, dsub, r]
        # dsub = 32*b4 + p//4 ; partition p = 4*(p//4) + n%4
        cnb = np.empty((4, GLOC, P, 1024), np.float32)
        for b4 in range(4):
            blk = clr[:, :, 32 * b4:32 * (b4 + 1), :]   # [n, t, 32, r]
            for g in range(GLOC):
                for e in range(4):
                    n = 4 * g + e
                    # partition p = 4*m + e (m = dsub idx), free = 128t + r
                    cnb[b4, g, e::4, :] = blk[n].transpose(1, 0, 2).reshape(
                        32, 8 * 128)
        m["cnb"] = np.ascontiguousarray(cnb).astype(ndt)

        pll = pl[loc]                     # [32, R, D]
        plb = np.empty((4, GLOC, P, 1024), np.float32)
        for b4 in range(4):
            blk = pll[:, 32 * b4:32 * (b4 + 1), :]      # [n, 32, D]
            for g in range(GLOC):
                for e in range(4):
                    plb[b4, g, e::4, :] = blk[4 * g + e]
        m["plb"] = np.ascontiguousarray(plb).astype(ndt)

        auxf = np.zeros((P, AF_W), np.float32)
        auxf[:NLOC, AF_A4:AF_A4 + P] = (
            (np.arange(NLOC)[:, None] % 4) == (np.arange(P)[None, :] % 4))
        auxf[:NLOC, AF_B8:AF_B8 + GLOC] = (
            (np.arange(NLOC)[:, None] // 4) == np.arange(GLOC)[None, :])
        onehot = np.zeros(B, np.float32); onehot[b] = 1.0
        auxf[0:1, AF_BSELR:AF_BSELR + B] = onehot[None, :]
        auxf[0:B, AF_BSELC:AF_BSELC + 1] = onehot[:, None]
        auxf[:, AF_NEGC] = NEG if hf == 0 else 0.0
        auxf[0, AF_ONE] = 1.0
        auxf[0, AF_ONES:AF_ONES + P] = 1.0
        auxf[:, AF_G1:AF_G1 + NT] = f(g1).reshape(NT, P).T
        auxf[:, AF_G2:AF_G2 + NT] = f(g2).reshape(NT, P).T
        wr1 = wstack.reshape(5, NEXP, D)[:4, eperm, :]
        auxf[0, AF_BR1:AF_BR1 + 256] = (
            wr1.reshape(256, D) @ f(b1).ravel())
        wr2 = wstack.reshape(5, NEXP, D)[4, eperm, :]
        auxf[0, AF_BR2:AF_BR2 + NEXP] = wr2 @ f(b2).ravel()
        m["auxf"] = auxf
        m["auxb"] = auxb
        in_maps.append(m)
    return in_maps


def _get_runner():
    global _RUNNER
    if _RUNNER is None:
        nc = build_nc(use_cc=True)
        _RUNNER = SpmdRunner(nc, N_CORES)
    return _RUNNER


def kernel(**inputs):
    r = _get_runner()
    in_maps = _make_inputs(**inputs)
    res = r.run(in_maps)
    out = np.empty((B, S, D), np.float32)
    for c in range(N_CORES):
        b, hf = c // 2, c % 2
        out[b, hf * SQ:(hf + 1) * SQ] = res[c]["o"]
    return out


# revision 21
# speedup vs baseline: 1.3037x; 1.0253x over previous
"""DAWN block (moe_routing) Trainium2 kernel: 8-core SPMD, v2.

Sharding: core c = (batch b=c//2, half h=c%2). Each core handles one batch's
attention + memory block for half the queries (rows permuted so local queries
come first). Expert pools (compress_neurons / expand_pool) are pair-sharded:
each core streams only 32 of 64 experts and partial combines are AllReduced
within the pair. Causal structure: per-q-slot key-position lists + a constant
triangular mask tile + per-core bias column (full-mask blocks), so ~19% of
score/AV work is skipped and no per-key mask tensor is needed.

DMA strategy: everything is host-relaid-out so the device does few, large,
contiguous DMAs (the v1 kernel's 456 tiny combine DMAs were the bottleneck:
each DMA costs ~0.6us queue dispatch + 625ns shared HWDGE serial time).
"""
import numpy as np
import ml_dtypes

import concourse.bass as bass
import concourse.mybir as mybir
import concourse.tile as tile
from concourse import bacc

B, S, D = 4, 1024, 1024
H, DH = 16, 64
R = 128
NEXP = 64
NLOC = 32          # local experts per core (pair-sharded)
GLOC = NLOC // 4   # 8 stacked-expert groups
NK, KR = 4096, 128
TOPK = 8
N_CORES = 8
SQ = S // 2
P = 128
NT = S // P        # 8 seq tiles
QT = SQ // P       # 4 local q tiles
DT_T = D // P      # 8 d tiles

F32 = mybir.dt.float32
U32 = mybir.dt.uint32
NEG = -1.0e9
ALU = mybir.AluOpType
ACT = mybir.ActivationFunctionType
AX = None

# aux column maps
AF_A4 = 0          # [0:32, 0:128]
AF_B8 = 128        # [0:32, 128:136]
AF_BSELR = 136     # [0:1, 136:140]
AF_BSELC = 140     # [0:4, 140:141]
AF_NEGC = 141      # [0:128, 141:142]
AF_ONE = 142       # [0:1, 142:143]  value 1.0
AF_ONES = 144      # [0:1, 144:272] row of ones
AF_G1 = 272        # [0:128, 272:280] g1 tiled [p, t]
AF_G2 = 280        # [0:128, 280:288] g2 tiled
AF_BR1 = 304       # [0:1, 304:560] b1 @ [Wc|WQ|WK|WV].T (perm)
AF_BR2 = 560       # [0:1, 560:624] b2 @ Wm.T (perm)
AF_W = 624
AB_IDN = 0         # [0:128, 0:128]
AB_BMS = 128       # [0:128, 128:160]
AB_TRI = 160       # [0:128, 160:672] = [tri | zeros x3]
AB_B1 = 672        # [0:128, 672:680] b1 tiled [p, t]
AB_B2 = 680        # [0:128, 680:688] b2 tiled
AB_W = 688


def bdt():
    return mybir.dt.bfloat16


def np_bdt():
    return ml_dtypes.bfloat16


def build_nc(use_cc=True):
    global AX
    AX = mybir.AxisListType.X
    DT = bdt()
    nc = bacc.Bacc("TRN2", target_bir_lowering=False, debug=False,
                   num_devices=N_CORES)
    I = {}

    def inp(name, shape, dt):
        I[name] = nc.dram_tensor(name, shape, dt, kind="ExternalInput").ap()

    inp("x", [S, D], F32)              # row-permuted batch (local q first)
    inp("imp", [P, NT], F32)           # imp[p,i] = importance[perm[128i+p]]
    inp("cnb", [4, GLOC, P, 1024], DT) # local-expert compress pool, relaid
    inp("plb", [4, GLOC, P, 1024], DT) # local-expert expand pool, relaid
    inp("wct", [P, NT * 320], DT)      # [Wc|WQ|WK|WV|Wm].T tiled (expert-perm)
    inp("wot", [P, NT * 1024], DT)     # WO.T tiled
    inp("kkt", [KR, NK], DT)           # knowledge_K.T
    inp("kv", [NK, D], DT)             # knowledge_V
    inp("auxf", [P, AF_W], F32)
    inp("auxb", [P, AB_W], DT)
    o = nc.dram_tensor("o", [SQ, D], F32, kind="ExternalOutput").ap()

    with tile.TileContext(nc) as tc:
        _body(nc, tc, I, o, use_cc)
    nc.compile()
    return nc


def _body(nc, tc, I, o, use_cc):
    DT = bdt()
    import contextlib
    ctx = contextlib.ExitStack()
    with ctx:
        pp = ctx.enter_context(tc.tile_pool(name="pers", bufs=1))
        sp = ctx.enter_context(tc.tile_pool(name="stream", bufs=2))
        st2 = ctx.enter_context(tc.tile_pool(name="strm", bufs=5))
        pst = ctx.enter_context(tc.tile_pool(name="ps_t", bufs=2, space="PSUM"))
        psa = ctx.enter_context(tc.tile_pool(name="ps_a", bufs=2, space="PSUM"))
        psv = ctx.enter_context(tc.tile_pool(name="ps_v", bufs=2, space="PSUM"))
        dr = ctx.enter_context(tc.tile_pool(name="dram", bufs=1, space="DRAM"))

        # ---------- bulk loads ----------
        xa = pp.tile([P, NT * 1024], F32, tag="xa", name="xa")
        for hh in range(2):
            src = bass.AP(I["x"].tensor, hh * 4 * P * 1024,
                          [[1024, P], [P * 1024, 4], [1, 1024]])
            nc.sync.dma_start(out=xa[:, hh * 4096:(hh + 1) * 4096], in_=src)
        wct = pp.tile([P, NT * 320], DT, tag="wct", name="wct")
        nc.sync.dma_start(out=wct[:], in_=I["wct"][:])
        impa = pp.tile([P, NT], F32, tag="impa", name="impa")
        nc.sync.dma_start(out=impa[:], in_=I["imp"][:])
        auxf = pp.tile([P, AF_W], F32, tag="auxf", name="auxf")
        nc.sync.dma_start(out=auxf[:], in_=I["auxf"][:])
        auxb = pp.tile([P, AB_W], DT, tag="auxb", name="auxb")
        nc.sync.dma_start(out=auxb[:], in_=I["auxb"][:])

        idn = auxb[:, AB_IDN:AB_IDN + P]
        bmS = auxb[:, AB_BMS:AB_BMS + 32]
        trix = auxb[:, AB_TRI:AB_TRI + 4 * P]
        A4 = auxf[0:NLOC, AF_A4:AF_A4 + P]
        B8 = auxf[0:NLOC, AF_B8:AF_B8 + GLOC]
        bselr = auxf[0:1, AF_BSELR:AF_BSELR + B]
        bselc = auxf[0:B, AF_BSELC:AF_BSELC + 1]
        negc = auxf[:, AF_NEGC:AF_NEGC + 1]
        one1 = auxf[0:1, AF_ONE:AF_ONE + 1]
        ones128 = auxf[0:1, AF_ONES:AF_ONES + P]
        g1t = auxf[:, AF_G1:AF_G1 + NT]
        g2t = auxf[:, AF_G2:AF_G2 + NT]
        b1t = auxb[:, AB_B1:AB_B1 + NT]
        b2t = auxb[:, AB_B2:AB_B2 + NT]
        brow1 = auxf[0:1, AF_BR1:AF_BR1 + 256]
        brow2 = auxf[0:1, AF_BR2:AF_BR2 + NEXP]

        # expert pool streams in half-b chunks: [128, 4096] = 4 g-groups.
        def stream_half(tensor, b, half):
            t = st2.tile([P, 4096], DT, tag="strm", name="strm")
            srcap = bass.AP(tensor, (b * GLOC + half * 4) * P * 1024,
                            [[1024, P], [P * 1024, 4], [1, 1024]])
            nc.sync.dma_start(out=t[:], in_=srcap)
            return t

        cn1 = [[stream_half(I["cnb"].tensor, b, h) for h in range(2)]
               for b in range(4)]
        pl1 = [[stream_half(I["plb"].tensor, b, h) for h in range(2)]
               for b in range(4)]
        wota2 = []
        for h in range(2):
            t = st2.tile([P, 4096], DT, tag="strm", name="strm")
            nc.sync.dma_start(out=t[:], in_=I["wot"][:, h * 4096:(h + 1) * 4096])
            wota2.append(t)
        cn2 = [[stream_half(I["cnb"].tensor, b, h) for h in range(2)]
               for b in range(4)]

        def copy_ps(out_ap, in_ap, k):
            nc.vector.tensor_copy(out=out_ap, in_=in_ap)

        def layernorm_tile(x_ap, pool, tag):
            # bn_stats computes per-partition mean/var in one DVE pass per
            # 512-wide subgroup; the final Act pass folds (x - mean) * rstd.
            bst = sp.tile([P, 2, 6], F32, tag="ln_bst", name="ln_bst")
            xg = x_ap.rearrange("p (s f) -> p s f", f=512)
            nc.vector.bn_stats(out=bst[:, 0, :], in_=xg[:, 0, :])
            nc.vector.bn_stats(out=bst[:, 1, :], in_=xg[:, 1, :])
            stats = sp.tile([P, 4], F32, tag="ln_stats", name="ln_stats")
            mv = stats[:, 0:2]
            rstd = stats[:, 2:3]; nmr = stats[:, 3:4]
            nc.vector.bn_aggr(out=mv, in_=bst[:])
            nc.vector.tensor_scalar(out=rstd, in0=stats[:, 1:2], scalar1=1e-5,
                                    scalar2=None, op0=ALU.add)
            nc.scalar.sqrt(rstd, rstd)
            nc.vector.reciprocal(rstd, rstd)
            nc.vector.tensor_tensor(out=nmr, in0=stats[:, 0:1], in1=rstd,
                                    op=ALU.mult)
            nc.vector.tensor_scalar(out=nmr, in0=nmr, scalar1=-1.0,
                                    scalar2=None, op0=ALU.mult)
            out = pool.tile([P, D], DT, tag=tag)
            nc.scalar.activation(out=out[:], in_=x_ap, func=ACT.Identity,
                                 scale=rstd, bias=nmr)
            return out

        def softmax_pool(psum_ap, out_ap, nblk, blk, imp_col, pool_out,
                         first, last):
            # exp (no max-sub; scores are O(1)) with per-block accum, then
            # pool with 1/Z folded into the importance column.
            zs = sp.tile([P, 8], F32, tag="sm_zs", name="sm_zs")
            for bi in range(nblk):
                sl = slice(bi * blk, (bi + 1) * blk)
                nc.scalar.activation(out=out_ap[:, sl], in_=psum_ap[:, sl],
                                     func=ACT.Exp, accum_out=zs[:, bi:bi + 1])
            nc.vector.reciprocal(zs[:, 0:nblk], zs[:, 0:nblk])
            impz = sp.tile([P, 8], F32, tag="sm_iz", name="sm_iz")
            nc.vector.tensor_scalar(out=impz[:, 0:nblk], in0=zs[:, 0:nblk],
                                    scalar1=imp_col, scalar2=None,
                                    op0=ALU.mult)
            for bi in range(nblk):
                sl = slice(bi * blk, (bi + 1) * blk)
                nc.tensor.matmul(out=pool_out[:, sl],
                                 lhsT=impz[:, bi:bi + 1], rhs=out_ap[:, sl],
                                 start=first, stop=last)

        def group_cols(wcol_ap, ncols):
            """wcol [32, ncols] f32 -> wk [128, GLOC*ncols]:
            wk[p, ncols*g + c] = wcol[4g + p%4, c]."""
            rhsB = sp.tile([NLOC, GLOC * ncols], F32, tag="rhsB", name="rhsB")
            for pi in range(ncols):
                nc.vector.tensor_scalar(
                    out=rhsB[:, pi:GLOC * ncols:ncols], in0=B8,
                    scalar1=wcol_ap[:, pi:pi + 1], scalar2=None, op0=ALU.mult)
            wkp = pst.tile([P, GLOC * ncols], F32, tag="tpp", name="wkp")
            nc.tensor.matmul(out=wkp[:], lhsT=A4, rhs=rhsB[:],
                             start=True, stop=True)
            wk = sp.tile([P, GLOC * ncols], F32, tag="wkall", name="wkall")
            nc.vector.tensor_copy(out=wk[:], in_=wkp[:])
            return wk

        def combine_cn(wcol_ap, chunks, out_f32):
            """out_f32 [128, 1024] f32 partial combine of local experts.
            chunks[b][p, 1024g+128t+r] = CN[e(g,p), 128t+32b+p//4, r]."""
            wk = group_cols(wcol_ap, 1)
            lhs = []
            for g in range(GLOC):
                lg = sp.tile([P, NLOC], DT, tag=f"clh{g}", name=f"clh{g}",
                             bufs=1)
                nc.vector.tensor_scalar(out=lg[:], in0=bmS,
                                        scalar1=wk[:, g:g + 1],
                                        scalar2=None, op0=ALU.mult)
                lhs.append(lg)
            for b in range(4):
                acc = psa.tile([NLOC, 1024], F32, tag="acc", name="cacc")
                for hh in range(2):
                    for g in range(GLOC):
                        gh, gl = g // 4, g % 4
                        nc.tensor.matmul(
                            out=acc[:, hh * 512:(hh + 1) * 512],
                            lhsT=lhs[g][:],
                            rhs=chunks[b][gh][:, gl * 1024 + hh * 512:
                                              gl * 1024 + (hh + 1) * 512],
                            start=(g == 0), stop=(g == GLOC - 1))
                copy_ps(out_f32[32 * b:32 * b + 32, :], acc[:], b)

        def pair_allreduce(sb_f32, ncol):
            """AllReduce sb_f32 [128, ncol] within batch pairs (in place)."""
            if not use_cc:
                return
            cc_in = dr.tile([P, ncol], F32)
            cc_out = dr.tile([P, ncol], F32)
            nc.gpsimd.dma_start(out=cc_in[:], in_=sb_f32[:])
            nc.gpsimd.collective_compute(
                "AllReduce", ALU.add,
                replica_groups=[[0, 1], [2, 3], [4, 5], [6, 7]],
                ins=[cc_in.opt()], outs=[cc_out.opt()])
            nc.gpsimd.dma_start(out=sb_f32[:], in_=cc_out[:])

        # ---------- LN1 + transposes ----------
        ctx4 = contextlib.ExitStack()
        p4 = ctx4.enter_context(tc.tile_pool(name="ph4", bufs=1))
        with tc.tile_pool(name="ph0", bufs=1) as p0:
            nxT = [p0.tile([P, S], DT, tag=f"nxT{t}", name=f"nxT{t}")
                   for t in range(DT_T)]
            for i in range(NT):
                nx_i = layernorm_tile(xa[:, i * 1024:(i + 1) * 1024], sp, "nx")
                for t in range(DT_T):
                    tp = pst.tile([P, P], DT, tag="tpp", name="tpp")
                    nc.tensor.transpose(out=tp[:],
                                        in_=nx_i[:, t * P:(t + 1) * P],
                                        identity=idn)
                    copy_ps(nxT[t][:, i * P:(i + 1) * P], tp[:], t)

            # ---------- routers (c,q,k,v) ----------
            wpool_ps = psv.tile([1, 4 * NEXP], F32, tag="pvacc", name="pvacc")
            for i in range(NT):
                pr_ps = psa.tile([P, 4 * NEXP], F32, tag="acc", name="acc")
                for t in range(DT_T):
                    nc.tensor.matmul(out=pr_ps[:],
                                     lhsT=nxT[t][:, i * P:(i + 1) * P],
                                     rhs=wct[:, 320 * t:320 * t + 256],
                                     start=(t == 0), stop=False)
                nc.tensor.matmul(out=pr_ps[:], lhsT=ones128, rhs=brow1,
                                 start=False, stop=True)
                pref = sp.tile([P, 4 * NEXP], F32, tag="pref", name="pref")
                softmax_pool(pr_ps[:], pref[:], 4, NEXP, impa[:, i:i + 1],
                             wpool_ps, first=(i == 0), last=(i == NT - 1))

            wrow = pp.tile([1, 4 * NEXP], F32, tag="wrow", name="wrow")
            nc.vector.tensor_copy(out=wrow[:], in_=wpool_ps[:])
            for bi in range(4):
                sl = slice(bi * NEXP, (bi + 1) * NEXP)
                st = sp.tile([1, 1], F32, tag="wn_st", name="wn_st")
                nc.vector.tensor_reduce(out=st[:], in_=wrow[:, sl], axis=AX,
                                        op=ALU.add)
                nc.vector.tensor_scalar(out=st[:], in0=st[:], scalar1=1e-8,
                                        scalar2=None, op0=ALU.add)
                nc.vector.reciprocal(st[:], st[:])
                nc.vector.tensor_scalar(out=wrow[:, sl], in0=wrow[:, sl],
                                        scalar1=st[:], scalar2=None,
                                        op0=ALU.mult)
            wt0 = pst.tile([P, 1], F32, tag="tpp", name="wt0")
            nc.tensor.transpose(out=wt0[:], in_=wrow[:, 0:P], identity=one1)
            wt1 = pst.tile([P, 1], F32, tag="tpp", name="wt1")
            nc.tensor.transpose(out=wt1[:], in_=wrow[:, P:2 * P], identity=one1)
            wcolcq = pp.tile([P, 1], F32, tag="wcolcq", name="wcolcq")
            nc.vector.tensor_copy(out=wcolcq[:], in_=wt0[:])
            wcolkv = pp.tile([P, 1], F32, tag="wcolkv", name="wcolkv")
            nc.vector.tensor_copy(out=wcolkv[:], in_=wt1[:])
            wcols3 = pp.tile([NLOC, 3], F32, tag="wcols3", name="wcols3")
            nc.vector.tensor_copy(out=wcols3[:, 0:1],
                                  in_=wcolcq[NEXP:NEXP + NLOC, :])
            nc.vector.tensor_copy(out=wcols3[:, 1:2], in_=wcolkv[0:NLOC, :])
            nc.vector.tensor_copy(out=wcols3[:, 2:3],
                                  in_=wcolkv[NEXP:NEXP + NLOC, :])

            # ---------- sc combine (+pair AllReduce) ----------
            e3f = p0.tile([P, 3072], F32, tag="e3f", name="e3f")
            scf = e3f[:, 0:1024]
            combine_cn(wcolcq[0:NLOC, 0:1], cn1, scf)
            pair_allreduce(scf, 1024)
            sc_b = p0.tile([P, 1024], DT, tag="sc_b", name="sc_b")
            for t in range(DT_T):
                nc.scalar.activation(out=sc_b[:, t * P:(t + 1) * P],
                                     in_=scf[:, t * P:(t + 1) * P],
                                     func=ACT.Identity,
                                     scale=g1t[:, t:t + 1])

            # ---------- e3 combine ----------
            w3 = group_cols(wcols3[:], 3)  # [128, 24]
            lhs3 = []
            for g in range(GLOC):
                lg = p0.tile([P, 96], DT, tag=f"e3lh{g}", name=f"e3lh{g}")
                for pl_i in range(3):
                    nc.vector.tensor_scalar(
                        out=lg[:, 32 * pl_i:32 * (pl_i + 1)], in0=bmS,
                        scalar1=w3[:, 3 * g + pl_i:3 * g + pl_i + 1],
                        scalar2=None, op0=ALU.mult)
                lhs3.append(lg)
            for b in range(4):
                acc = psa.tile([96, 1024], F32, tag="acc", name="eacc")
                for hh in range(2):
                    for g in range(GLOC):
                        gh, gl = g // 4, g % 4
                        nc.tensor.matmul(
                            out=acc[:, hh * 512:(hh + 1) * 512],
                            lhsT=lhs3[g][:],
                            rhs=pl1[b][gh][:, gl * 1024 + hh * 512:
                                           gl * 1024 + (hh + 1) * 512],
                            start=(g == 0), stop=(g == GLOC - 1))
                for pl_i in range(3):
                    copy_ps(e3f[32 * b:32 * b + 32,
                                1024 * pl_i:1024 * (pl_i + 1)],
                            acc[32 * pl_i:32 * pl_i + 32, :], b + pl_i)
            pair_allreduce(e3f, 3072)
            e3 = p0.tile([P, 3072], DT, tag="e3", name="e3")
            nc.vector.tensor_copy(out=e3[:, 0:1024], in_=e3f[:, 0:1024])
            nc.scalar.activation(out=e3[:, 1024:2048], in_=e3f[:, 1024:2048],
                                 func=ACT.Identity)
            nc.gpsimd.tensor_copy(out=e3[:, 2048:3072], in_=e3f[:, 2048:3072])

            # ---------- hT[r, q] = sum_d sc[d, r] g1[d] nx[q, d] + (b1 @ sc g1)[r]
            bsc_ps = pst.tile([1, P], F32, tag="tpp", name="bscp")
            for t in range(DT_T):
                nc.tensor.matmul(out=bsc_ps[:], lhsT=b1t[:, t:t + 1],
                                 rhs=sc_b[:, t * P:(t + 1) * P],
                                 start=(t == 0), stop=(t == DT_T - 1))
            bsc_row = sp.tile([1, P], F32, tag="bscr", name="bscr")
            nc.vector.tensor_copy(out=bsc_row[:], in_=bsc_ps[:])
            bsc_t = pst.tile([P, 1], F32, tag="tpp", name="bsct")
            nc.tensor.transpose(out=bsc_t[:], in_=bsc_row[:], identity=one1)
            bsc = sp.tile([P, 1], F32, tag="bsc", name="bsc")
            nc.vector.tensor_copy(out=bsc[:], in_=bsc_t[:])
            hT = p0.tile([P, S], DT, tag="hT")
            for j in range(2):
                hp = psa.tile([P, 512], F32, tag="acc", name="hacc")
                for t in range(DT_T):
                    nc.tensor.matmul(out=hp[:],
                                     lhsT=sc_b[:, t * P:(t + 1) * P],
                                     rhs=nxT[t][:, j * 512:(j + 1) * 512],
                                     start=(t == 0), stop=(t == DT_T - 1))
                nc.scalar.activation(out=hT[:, j * 512:(j + 1) * 512],
                                     in_=hp[:], func=ACT.Identity, bias=bsc)

            # ---------- K, Q, V ----------
            SCALE_Q = 1.0 / float(np.sqrt(DH))
            kT = [p4.tile([P, S], DT, tag=f"kT{t}", name=f"kT{t}")
                  for t in range(DT_T)]
            qT = [p4.tile([P, SQ], DT, tag=f"qT{t}", name=f"qT{t}")
                  for t in range(DT_T)]
            vext = [p4.tile([P, H * (DH + 1)], DT, tag=f"vx{i}", name=f"vx{i}")
                    for i in range(NT)]
            for t in range(DT_T):
                kp = psa.tile([P, S], F32, tag="acc", name="acc")
                for j in range(2):
                    nc.tensor.matmul(out=kp[:, j * 512:(j + 1) * 512],
                                     lhsT=e3[:, 1024 + t * P:1024 + t * P + P],
                                     rhs=hT[:, j * 512:(j + 1) * 512],
                                     start=True, stop=True)
                nc.scalar.activation(out=kT[t][:], in_=kp[:], func=ACT.Identity)
                qp = psv.tile([P, SQ], F32, tag="pvacc", name="qacc")
                nc.tensor.matmul(out=qp[:], lhsT=e3[:, t * P:t * P + P],
                                 rhs=hT[:, 0:SQ], start=True, stop=True)
                nc.vector.tensor_scalar(out=qT[t][:], in0=qp[:],
                                        scalar1=SCALE_Q, scalar2=None,
                                        op0=ALU.mult)
            for i in range(NT):
                vp = psa.tile([P, D], F32, tag="acc", name="acc")
                for j in range(2):
                    nc.tensor.matmul(
                        out=vp[:, j * 512:(j + 1) * 512],
                        lhsT=hT[:, i * P:(i + 1) * P],
                        rhs=e3[:, 2048 + j * 512:2048 + (j + 1) * 512],
                        start=True, stop=True)
                vv = vext[i][:].rearrange("p (hh c) -> p hh c", c=DH + 1)
                nc.vector.tensor_copy(
                    out=vv[:, :, 0:DH],
                    in_=vp[:].rearrange("p (hh c) -> p hh c", c=DH))
                nc.gpsimd.memset(vv[:, :, DH:DH + 1], 1.0)
        # ph0 (nxT, scf, e3f, lhs3) released

        # ---------- attention ----------
        # q-slot s covers local q-tile s; key positions {0..s} u {4..7}.
        # position j==s gets the constant tri mask via PE; positions 4..7 get
        # the per-core bias column (0 or -1e9) folded into the exp.
        attnT = [p4.tile([P, SQ], DT, tag=f"at{t}", name=f"at{t}")
                 for t in range(DT_T)]
        for hd in range(H):
            t4 = hd // 2
            hs = (hd % 2) * DH
            po = psv.tile([DH + 1, SQ], F32, tag="pvacc", name="poacc")
            for j in range(QT):
                qlo = j * P
                w = SQ - qlo
                sps = psa.tile([P, SQ], F32, tag="acc", name="sacc")
                nc.tensor.matmul(out=sps[:, 0:P],
                                 lhsT=kT[t4][hs:hs + DH, j * P:(j + 1) * P],
                                 rhs=qT[t4][hs:hs + DH, qlo:qlo + P],
                                 start=True, stop=False)
                if w > P:
                    nc.tensor.matmul(out=sps[:, P:w],
                                     lhsT=kT[t4][hs:hs + DH,
                                                 j * P:(j + 1) * P],
                                     rhs=qT[t4][hs:hs + DH, qlo + P:SQ],
                                     start=True, stop=True)
                nc.tensor.matmul(out=sps[:, 0:P], lhsT=idn,
                                 rhs=trix[:, 0:P], start=False, stop=True)
                pt = sp.tile([P, SQ], DT, tag="p_tile", name="p_tile", bufs=3)
                nc.scalar.activation(out=pt[:, 0:w], in_=sps[:, 0:w],
                                     func=ACT.Exp)
                nc.tensor.matmul(
                    out=po[:, qlo:SQ],
                    lhsT=vext[j][:, hd * (DH + 1):(hd + 1) * (DH + 1)],
                    rhs=pt[:, 0:w], start=(j == 0), stop=False)
            for jp in range(2):
                j0 = QT + 2 * jp
                sps = psa.tile([P, 2 * SQ], F32, tag="acc", name="sacc2")
                for jj in range(2):
                    nc.tensor.matmul(
                        out=sps[:, jj * SQ:(jj + 1) * SQ],
                        lhsT=kT[t4][hs:hs + DH, (j0 + jj) * P:(j0 + jj + 1) * P],
                        rhs=qT[t4][hs:hs + DH, :],
                        start=True, stop=True)
                pt = sp.tile([P, 2 * SQ], DT, tag="p_tile2", name="p_tile2",
                             bufs=2)
                nc.scalar.activation(out=pt[:], in_=sps[:], func=ACT.Exp,
                                     bias=negc)
                for jj in range(2):
                    nc.tensor.matmul(
                        out=po[:],
                        lhsT=vext[j0 + jj][:,
                                           hd * (DH + 1):(hd + 1) * (DH + 1)],
                        rhs=pt[:, jj * SQ:(jj + 1) * SQ],
                        start=False, stop=(jp == 1 and jj == 1))
            rec = sp.tile([1, SQ], F32, tag="rec", name="rec")
            nc.vector.reciprocal(rec[:], po[DH:DH + 1, :])
            recB = sp.tile([DH, SQ], F32, tag="recB", name="recB")
            nc.gpsimd.partition_broadcast(recB[:], rec[:])
            nc.vector.tensor_tensor(out=attnT[t4][hs:hs + DH, :],
                                    in0=po[0:DH, :], in1=recB[:], op=ALU.mult)

        # ---------- WO + residual (into xa) ----------
        for i in range(QT):
            wp = psa.tile([P, D], F32, tag="acc", name="acc")
            for j in range(2):
                for t in range(DT_T):
                    toff = 1024 * t + 512 * j
                    nc.tensor.matmul(
                        out=wp[:, j * 512:(j + 1) * 512],
                        lhsT=attnT[t][:, i * P:(i + 1) * P],
                        rhs=wota2[toff // 4096][:, toff % 4096:
                                                toff % 4096 + 512],
                        start=(t == 0), stop=(t == DT_T - 1))
            nc.vector.tensor_tensor(out=xa[:, i * 1024:(i + 1) * 1024],
                                    in0=wp[:], in1=xa[:, i * 1024:(i + 1) * 1024],
                                    op=ALU.add)

        ctx4.close()

        # ---------- memory block ----------
        with tc.tile_pool(name="ph6", bufs=1) as p6:
            nx2T = [p6.tile([P, SQ], DT, tag=f"n2T{t}", name=f"n2T{t}")
                    for t in range(DT_T)]
            kkt = p6.tile([KR, NK], DT, tag="kkt", name="kkt")
            nc.sync.dma_start(out=kkt[:], in_=I["kkt"][:])
            for i in range(QT):
                nx2_i = layernorm_tile(xa[:, i * 1024:(i + 1) * 1024], sp,
                                       "nx2")
                for t in range(DT_T):
                    tp = pst.tile([P, P], DT, tag="tpp", name="tpp")
                    nc.tensor.transpose(out=tp[:],
                                        in_=nx2_i[:, t * P:(t + 1) * P],
                                        identity=idn)
                    copy_ps(nx2T[t][:, i * P:(i + 1) * P], tp[:], t)

            mwp_ps = psv.tile([1, NEXP], F32, tag="pvacc", name="pvacc")
            for i in range(QT):
                pr = psa.tile([P, NEXP], F32, tag="acc", name="acc")
                for t in range(DT_T):
                    nc.tensor.matmul(out=pr[:],
                                     lhsT=nx2T[t][:, i * P:(i + 1) * P],
                                     rhs=wct[:, 320 * t + 256:320 * t + 320],
                                     start=(t == 0), stop=False)
                nc.tensor.matmul(out=pr[:], lhsT=ones128, rhs=brow2,
                                 start=False, stop=True)
                prefm = sp.tile([P, NEXP], F32, tag="prefm", name="prefm")
                softmax_pool(pr[:], prefm[:], 1, NEXP, impa[:, i:i + 1],
                             mwp_ps, first=(i == 0), last=(i == QT - 1))

            mwrow = p6.tile([1, NEXP], F32, tag="mwrow", name="mwrow")
            if use_cc:
                mwr = sp.tile([1, NEXP], F32, tag="mwr", name="mwr")
                nc.vector.tensor_copy(out=mwr[:], in_=mwp_ps[:])
                ccp = psa.tile([B, NEXP], F32, tag="acc", name="acc")
                nc.tensor.matmul(out=ccp[:], lhsT=bselr, rhs=mwr[:],
                                 start=True, stop=True)
                cc_sb = sp.tile([B, NEXP], F32, tag="cc_sb", name="cc_sb")
                nc.vector.tensor_copy(out=cc_sb[:], in_=ccp[:])
                cc_in = dr.tile([B, NEXP], F32)
                cc_out = dr.tile([B, NEXP], F32)
                nc.gpsimd.dma_start(out=cc_in[:], in_=cc_sb[:])
                nc.gpsimd.collective_compute(
                    "AllReduce", ALU.add,
                    replica_groups=[list(range(N_CORES))],
                    ins=[cc_in.opt()], outs=[cc_out.opt()])
                cc_res = sp.tile([B, NEXP], F32, tag="cc_res", name="cc_res")
                nc.gpsimd.dma_start(out=cc_res[:], in_=cc_out[:])
                mwf = psa.tile([1, NEXP], F32, tag="acc", name="acc")
                nc.tensor.matmul(out=mwf[:], lhsT=bselc, rhs=cc_res[:],
                                 start=True, stop=True)
                nc.vector.tensor_copy(out=mwrow[:], in_=mwf[:])
            else:
                nc.vector.tensor_copy(out=mwrow[:], in_=mwp_ps[:])
            st = sp.tile([1, 1], F32, tag="wn_st", name="wn_st")
            nc.vector.tensor_reduce(out=st[:], in_=mwrow[:], axis=AX,
                                    op=ALU.add)
            nc.vector.tensor_scalar(out=st[:], in0=st[:], scalar1=1e-8,
                                    scalar2=None, op0=ALU.add)
            nc.vector.reciprocal(st[:], st[:])
            nc.vector.tensor_scalar(out=mwrow[:], in0=mwrow[:], scalar1=st[:],
                                    scalar2=None, op0=ALU.mult)
            mwrow_cp = sp.tile([1, NEXP], F32, tag="mwr2", name="mwr2")
            nc.vector.tensor_copy(out=mwrow_cp[:], in_=mwrow[:])
            mwt = pst.tile([NEXP, 1], F32, tag="tpp", name="mwt")
            nc.tensor.transpose(out=mwt[:], in_=mwrow_cp[:], identity=one1)
            mwcol = p6.tile([NEXP, 1], F32, tag="mwcol", name="mwcol")
            nc.vector.tensor_copy(out=mwcol[:], in_=mwt[:])

            scmf = p6.tile([P, 1024], F32, tag="scmf", name="scmf")
            combine_cn(mwcol[0:NLOC, 0:1], cn2, scmf)
            pair_allreduce(scmf, 1024)
            scm_b = p6.tile([P, 1024], DT, tag="scm_b", name="scm_b")
            for t in range(DT_T):
                nc.scalar.activation(out=scm_b[:, t * P:(t + 1) * P],
                                     in_=scmf[:, t * P:(t + 1) * P],
                                     func=ACT.Identity,
                                     scale=g2t[:, t:t + 1])

            # QmT [r, sq]
            qmp = psv.tile([P, SQ], F32, tag="pvacc", name="pvacc")
            for t in range(DT_T):
                nc.tensor.matmul(out=qmp[:], lhsT=scm_b[:, t * P:(t + 1) * P],
                                 rhs=nx2T[t][:], start=(t == 0),
                                 stop=(t == DT_T - 1))
            bscm_ps = pst.tile([1, P], F32, tag="tpp", name="bscmp")
            for t in range(DT_T):
                nc.tensor.matmul(out=bscm_ps[:], lhsT=b2t[:, t:t + 1],
                                 rhs=scm_b[:, t * P:(t + 1) * P],
                                 start=(t == 0), stop=(t == DT_T - 1))
            bscm_row = sp.tile([1, P], F32, tag="bscr", name="bscmr")
            nc.vector.tensor_scalar(out=bscm_row[:], in0=bscm_ps[:],
                                    scalar1=1.0 / float(np.sqrt(KR)),
                                    scalar2=None, op0=ALU.mult)
            bscm_t = pst.tile([P, 1], F32, tag="tpp", name="bscmt")
            nc.tensor.transpose(out=bscm_t[:], in_=bscm_row[:], identity=one1)
            bscm = sp.tile([P, 1], F32, tag="bsc", name="bscm")
            nc.vector.tensor_copy(out=bscm[:], in_=bscm_t[:])
            qmT = p6.tile([P, SQ], DT, tag="qmT")
            nc.scalar.activation(out=qmT[:], in_=qmp[:], func=ACT.Identity,
                                 scale=1.0 / float(np.sqrt(KR)), bias=bscm)

            idx_all = p6.tile([P, QT * TOPK], U32, tag="idx_all",
                              name="idx_all")
            w8_all = p6.tile([P, QT * TOPK], F32, tag="w8_all", name="w8_all")
            for i in range(QT):
                ks = p6.tile([P, NK], DT, tag="ks_sb", name="ks_sb")
                for j in range(NK // 512):
                    ksp = psa.tile([P, 512], F32, tag="acc", name="acc")
                    nc.tensor.matmul(out=ksp[:],
                                     lhsT=qmT[:, i * P:(i + 1) * P],
                                     rhs=kkt[:, j * 512:(j + 1) * 512],
                                     start=True, stop=True)
                    nc.scalar.activation(out=ks[:, j * 512:(j + 1) * 512],
                                         in_=ksp[:], func=ACT.Identity)
                tv = sp.tile([P, TOPK], F32, tag="tv", name="tv")
                nc.vector.max_with_indices(
                    out_max=tv[:],
                    out_indices=idx_all[:, i * TOPK:(i + 1) * TOPK],
                    in_=ks[:])
                st8 = sp.tile([P, 2], F32, tag="st8", name="st8")
                nm = st8[:, 0:1]; se8 = st8[:, 1:2]
                nc.vector.tensor_scalar(out=nm, in0=tv[:, 0:1], scalar1=-1.0,
                                        scalar2=None, op0=ALU.mult)
                w8 = sp.tile([P, TOPK], F32, tag="w8", name="w8")
                nc.scalar.activation(out=w8[:], in_=tv[:], func=ACT.Exp,
                                     bias=nm, accum_out=se8)
                nc.vector.reciprocal(se8, se8)
                nc.vector.tensor_scalar(out=w8_all[:, i * TOPK:(i + 1) * TOPK],
                                        in0=w8[:], scalar1=se8, scalar2=None,
                                        op0=ALU.mult)

            for i in range(QT):
                acc = p6.tile([P, D], F32, tag="mem_acc", name="mem_acc", bufs=2)
                gt8 = p6.tile([P, TOPK * D], DT, tag="gath8", name="gath8", bufs=2)
                nc.gpsimd.indirect_dma_start(
                    out=gt8[:], out_offset=None, in_=I["kv"][:],
                    in_offset=bass.IndirectOffsetOnAxis(
                        ap=idx_all[:, i * TOPK:(i + 1) * TOPK], axis=0))
                accB = p6.tile([P, D], F32, tag="mem_accB", name="mem_accB",
                               bufs=2)
                tmp = sp.tile([P, D], F32, tag="gtmp", name="gtmp")
                for k in range(TOPK):
                    g = i * TOPK + k
                    if k % 2 == 0:
                        prev = (xa[:, i * 1024:(i + 1) * 1024] if k == 0
                                else acc[:])
                        nc.vector.scalar_tensor_tensor(
                            out=acc[:], in0=gt8[:, k * D:(k + 1) * D],
                            scalar=w8_all[:, g:g + 1],
                            in1=prev, op0=ALU.mult, op1=ALU.add)
                    else:
                        dst = accB[:] if k == 1 else tmp[:]
                        nc.scalar.activation(out=dst,
                                             in_=gt8[:, k * D:(k + 1) * D],
                                             func=ACT.Identity,
                                             scale=w8_all[:, g:g + 1])
                        if k > 1:
                            nc.gpsimd.tensor_tensor(out=accB[:], in0=tmp[:],
                                                    in1=accB[:], op=ALU.add)
                nc.vector.tensor_tensor(out=acc[:], in0=acc[:], in1=accB[:],
                                        op=ALU.add)
                nc.sync.dma_start(out=o[i * P:(i + 1) * P, :], in_=acc[:])


# ---------------- PJRT SPMD runner (persistent jit) ----------------

class SpmdRunner:
    def __init__(self, nc, n_cores):
        import jax
        from jax.sharding import Mesh, PartitionSpec
        from jax.experimental.shard_map import shard_map
        from concourse import bass2jax
        bass2jax.install_neuronx_cc_hook()
        self.jax = jax
        self.nc = nc
        self.n_cores = n_cores
        partition_name = (nc.partition_id_tensor.name
                          if nc.partition_id_tensor else None)
        in_names, out_names, out_avals, zero_outs = [], [], [], []
        for alloc in nc.m.functions[0].allocations:
            if not isinstance(alloc, mybir.MemoryLocationSet):
                continue
            name = alloc.memorylocations[0].name
            if alloc.kind == "ExternalInput":
                if name != partition_name:
                    in_names.append(name)
            elif alloc.kind == "ExternalOutput":
                shape = tuple(alloc.tensor_shape)
                dtype = mybir.dt.np(alloc.dtype)
                out_names.append(name)
                out_avals.append(jax.core.ShapedArray(shape, dtype))
                zero_outs.append(np.zeros(shape, dtype))
        self.n_params = len(in_names)
        self.in_names = list(in_names)
        self.out_names = out_names
        self.out_avals = out_avals
        self.zero_outs = zero_outs
        all_in = in_names + out_names + ([partition_name] if partition_name
                                         else [])

        def _body(*args):
            operands = list(args)
            if partition_name is not None:
                operands.append(bass2jax.partition_id_tensor())
            outs = bass2jax._bass_exec_p.bind(
                *operands, out_avals=tuple(out_avals), in_names=tuple(all_in),
                out_names=tuple(out_names), lowering_input_output_aliases=(),
                sim_require_finite=True, sim_require_nnan=True, nc=nc)
            return tuple(outs)

        devices = jax.devices()[:n_cores]
        self.mesh = Mesh(np.asarray(devices), ("core",))
        nspec = self.n_params + len(out_names)
        self.sharded = jax.jit(
            shard_map(_body, mesh=self.mesh,
                      in_specs=(PartitionSpec("core"),) * nspec,
                      out_specs=(PartitionSpec("core"),) * len(out_names),
                      check_rep=False),
            keep_unused=True)

    def concat_inputs(self, in_maps):
        per_core = [[np.asarray(m[n]) for n in self.in_names] for m in in_maps]
        cat = [np.concatenate([per_core[c][i] for c in range(self.n_cores)],
                              axis=0) for i in range(self.n_params)]
        cat += [np.zeros((self.n_cores * z.shape[0], *z.shape[1:]), z.dtype)
                for z in self.zero_outs]
        return cat

    def run(self, in_maps):
        out_arrs = self.sharded(*self.concat_inputs(in_maps))
        self.jax.block_until_ready(out_arrs)
        return [
            {n: np.asarray(out_arrs[i]).reshape(
                self.n_cores, *self.out_avals[i].shape)[c]
             for i, n in enumerate(self.out_names)}
            for c in range(self.n_cores)
        ]


# ---------------- host side ----------------

_RUNNER = None


def _make_inputs(x, importance, mask, compress_neurons, expand_pool,
                 knowledge_K, knowledge_V, Wc, WQ, WK, WV, Wm, WO,
                 g1, b1, g2, b2):
    ndt = np_bdt()
    f = lambda a: np.asarray(a, np.float32)
    cn = f(compress_neurons)
    pl = f(expand_pool)
    wstack = np.concatenate([f(Wc), f(WQ), f(WK), f(WV), f(Wm)], axis=0)
    wot = np.ascontiguousarray(f(WO).T)  # [D, D] = WO.T
    wotr = np.empty((P, NT * 1024), np.float32)
    for t in range(NT):
        wotr[:, 1024 * t:1024 * (t + 1)] = wot[128 * t:128 * (t + 1), :]
    kkt = np.ascontiguousarray(f(knowledge_K).T).astype(ndt)
    kv = f(knowledge_V).astype(ndt)

    # aux (core-independent parts)
    auxb = np.zeros((P, AB_W), np.float32)
    auxb[:, AB_IDN:AB_IDN + P] = np.eye(P)
    auxb[:, AB_BMS:AB_BMS + 32] = (
        (np.arange(P)[:, None] // 4) == np.arange(32)[None, :])
    ktri = np.arange(P)
    auxb[:, AB_TRI:AB_TRI + P] = np.where(
        ktri[None, :] >= ktri[:, None], 0.0, NEG)  # tri[k, q]; rest zeros
    auxb[:, AB_B1:AB_B1 + NT] = f(b1).reshape(NT, P).T
    auxb[:, AB_B2:AB_B2 + NT] = f(b2).reshape(NT, P).T
    auxb = auxb.astype(ndt)

    x = f(x); importance = f(importance)
    in_maps = []
    for c in range(N_CORES):
        b, hf = c // 2, c % 2
        qr = np.arange(hf * SQ, hf * SQ + SQ)
        rest = np.arange((1 - hf) * SQ, (1 - hf) * SQ + SQ)
        perm = np.concatenate([qr, rest])
        eperm = (np.arange(NEXP) + NLOC * hf) % NEXP  # local experts first

        m = {}
        m["x"] = np.ascontiguousarray(x[b][perm])
        impc = importance[b][perm].reshape(NT, P).T  # [p, i]
        m["imp"] = np.ascontiguousarray(impc).astype(np.float32)

        # wct: [128, 8*320]; block order [Wc|WQ|WK|WV|Wm], experts permuted.
        # g1 folded into router1 blocks, g2 into the Wm block (LN emits the
        # plain normalized value).
        wp_ = wstack.reshape(5, NEXP, D)[:, eperm, :].reshape(5 * NEXP, D)
        wp_ = wp_ * np.concatenate([np.tile(f(g1).ravel()[None, :], (4 * NEXP, 1)),
                                    np.tile(f(g2).ravel()[None, :], (NEXP, 1))])
        wctT = wp_.T  # [D, 320]
        wcth = np.empty((P, NT * 320), np.float32)
        for t in range(NT):
            wcth[:, 320 * t:320 * (t + 1)] = wctT[128 * t:128 * (t + 1), :]
        m["wct"] = np.ascontiguousarray(wcth).astype(ndt)
        m["wot"] = wotr.astype(ndt)
        m["kkt"] = kkt
        m["kv"] = kv

        # cnb[b4, g, p, 128t + r] = cn[e(g,p), 128t + 32*b4 + p//4, r]
        loc = eperm[:NLOC]
        cl = cn[loc]                      # [32, D, R]
        clr = cl.reshape(NLOC, 8, 128, R) # [n, t, dsub, r]
        # dsub = 32*b4 + p//4 ; partition p = 4*(p//4) + n%4
        cnb = np.empty((4, GLOC, P, 1024), np.float32)
        for b4 in range(4):
            blk = clr[:, :, 32 * b4:32 * (b4 + 1), :]   # [n, t, 32, r]
            for g in range(GLOC):
                for e in range(4):
                    n = 4 * g + e
                    # partition p = 4*m + e (m = dsub idx), free = 128t + r
                    cnb[b4, g, e::4, :] = blk[n].transpose(1, 0, 2).reshape(
                        32, 8 * 128)
        m["cnb"] = np.ascontiguousarray(cnb).astype(ndt)

        pll = pl[loc]                     # [32, R, D]
        plb = np.empty((4, GLOC, P, 1024), np.float32)
        for b4 in range(4):
            blk = pll[:, 32 * b4:32 * (b4 + 1), :]      # [n, 32, D]
            for g in range(GLOC):
                for e in range(4):
                    plb[b4, g, e::4, :] = blk[4 * g + e]
        m["plb"] = np.ascontiguousarray(plb).astype(ndt)

        auxf = np.zeros((P, AF_W), np.float32)
        auxf[:NLOC, AF_A4:AF_A4 + P] = (
            (np.arange(NLOC)[:, None] % 4) == (np.arange(P)[None, :] % 4))
        auxf[:NLOC, AF_B8:AF_B8 + GLOC] = (
            (np.arange(NLOC)[:, None] // 4) == np.arange(GLOC)[None, :])
        onehot = np.zeros(B, np.float32); onehot[b] = 1.0
        auxf[0:1, AF_BSELR:AF_BSELR + B] = onehot[None, :]
        auxf[0:B, AF_BSELC:AF_BSELC + 1] = onehot[:, None]
        auxf[:, AF_NEGC] = NEG if hf == 0 else 0.0
        auxf[0, AF_ONE] = 1.0
        auxf[0, AF_ONES:AF_ONES + P] = 1.0
        auxf[:, AF_G1:AF_G1 + NT] = f(g1).reshape(NT, P).T
        auxf[:, AF_G2:AF_G2 + NT] = f(g2).reshape(NT, P).T
        wr1 = wstack.reshape(5, NEXP, D)[:4, eperm, :]
        auxf[0, AF_BR1:AF_BR1 + 256] = (
            wr1.reshape(256, D) @ f(b1).ravel())
        wr2 = wstack.reshape(5, NEXP, D)[4, eperm, :]
        auxf[0, AF_BR2:AF_BR2 + NEXP] = wr2 @ f(b2).ravel()
        m["auxf"] = auxf
        m["auxb"] = auxb
        in_maps.append(m)
    return in_maps


def _get_runner():
    global _RUNNER
    if _RUNNER is None:
        nc = build_nc(use_cc=True)
        _RUNNER = SpmdRunner(nc, N_CORES)
    return _RUNNER


def kernel(**inputs):
    r = _get_runner()
    in_maps = _make_inputs(**inputs)
    res = r.run(in_maps)
    out = np.empty((B, S, D), np.float32)
    for c in range(N_CORES):
        b, hf = c // 2, c % 2
        out[b, hf * SQ:(hf + 1) * SQ] = res[c]["o"]
    return out
